# revision 1
# baseline (speedup 1.0000x reference)
"""Trainium2 Bass kernel for nn_BenchGNN_29300266893894 (3-layer GCN with
global-feature concat + global mean/max pooling readout + MLP head).

Self-contained: host-side sharding/packing prep + SPMD Bass/Tile program on
8 NeuronCores via run_bass_kernel_spmd.

Sharding: nodes are split into 8 contiguous shards of 6250 (padded to 6272 =
49*128). Edges are partitioned by dst owner and sorted by dst window; weight
matrices are replicated. Per layer, each core computes h = x_cat @ W for its
nodes, builds a bf16 "gather table" of h' = h*dinv rows (padded to 256 B),
AllGathers it, then aggregates its incident edges with hardware dma_gather +
one-hot matmuls accumulating per-128-dst-window PSUM tiles. Graph pooling is
done with DVE segmented scans + ap_gather boundary extraction and AllReduce.
"""
import sys
import numpy as np
import ml_dtypes

sys.path.insert(0, "/opt/trn_rl_repo")

bf16 = ml_dtypes.bfloat16
f32 = np.float32

# ---------------- problem geometry (hardcoded) ----------------
CFG = dict(N=50000, E=800000, G=512, F_IN=128, HID=96, OUT=10, C=8)

WIN = 128
HALF = 32768
CHUNK_WINDOWS = 4
CONV_CHUNK = 512
PAD_SLOT = 999.0
PAD_VAL = -60000.0


class Geo:
    def __init__(self, cfg):
        self.__dict__.update(cfg)
        assert self.N % self.C == 0
        self.NL = self.N // self.C
        self.NWIN = -(-self.NL // WIN)
        self.NLP = self.NWIN * WIN
        # pooling pad-column tricks need at least one pad column per core
        assert self.NLP > self.NL
        self.NP = self.NLP * self.C
        self.conv_chunks = [(o, min(CONV_CHUNK, self.NLP - o))
                            for o in range(0, self.NLP, CONV_CHUNK)]


def _wrap16(idx, channels):
    idx = np.asarray(idx, np.int16)
    assert len(idx) % 16 == 0
    idx16 = idx.reshape(-1, 16).T
    return np.ascontiguousarray(np.tile(idx16, (channels // 16, 1)))


def prep(geo, x, edge_index, batch, W1, b1, W2, b2, W3, b3,
         Wl1, bl1, Wl2, bl2, Wl3, bl3):
    g = geo
    x = np.asarray(x, f32)
    src = np.asarray(edge_index[0], np.int64)
    dst = np.asarray(edge_index[1], np.int64)
    batch = np.asarray(batch, np.int64)

    def gid_of(n):
        return (n // g.NL) * g.NLP + (n % g.NL)

    deg = np.bincount(dst, minlength=g.N).astype(f32) + 1.0
    dinv = (1.0 / np.sqrt(deg)).astype(f32)
    counts = np.bincount(batch, minlength=g.G).astype(f32)
    invc = (1.0 / np.maximum(counts, 1.0)).astype(f32)

    sums1 = np.zeros((g.G, g.F_IN), f32)
    np.add.at(sums1, batch, x)
    mean1T = np.ascontiguousarray((sums1 * invc[:, None]).T)

    src_g = gid_of(src)

    core_win = []
    for k in range(g.C):
        lo_n, hi_n = k * g.NL, (k + 1) * g.NL
        sel = (dst >= lo_n) & (dst < hi_n)
        es, ed = src_g[sel], (dst[sel] - lo_n)
        vloc = np.arange(g.NL, dtype=np.int64)
        es = np.concatenate([es, gid_of(vloc + lo_n)])
        ed = np.concatenate([ed, vloc])
        order = np.argsort(ed, kind="stable")
        es, ed = es[order], ed[order]
        wstart = np.searchsorted(ed, np.arange(0, g.NLP + 1, WIN))
        wins = []
        for w in range(g.NWIN):
            a, b = wstart[w], wstart[w + 1]
            ws, wd = es[a:b], ed[a:b] - w * WIN
            lo_m = ws < HALF
            wins.append((ws[lo_m], wd[lo_m], ws[~lo_m] - HALF, wd[~lo_m]))
        core_win.append(wins)

    T_lo = [max(1, max(-(-len(core_win[k][w][0]) // 128) for k in range(g.C)))
            for w in range(g.NWIN)]
    T_hi = [max(0, max(-(-len(core_win[k][w][2]) // 128) for k in range(g.C)))
            for w in range(g.NWIN)]

    # chunk plan: groups of CHUNK_WINDOWS windows; per chunk pack
    # [lo tiles of each window][hi tiles of each window]
    chunk_plan = []   # (w0, nwin, [T_lo..], [T_hi..])
    for w0 in range(0, g.NWIN, CHUNK_WINDOWS):
        nw = min(CHUNK_WINDOWS, g.NWIN - w0)
        chunk_plan.append((w0, nw, T_lo[w0:w0 + nw], T_hi[w0:w0 + nw]))

    per_core = []
    for k in range(g.C):
        lo_n, hi_n = k * g.NL, (k + 1) * g.NL
        idx_list, slot_list = [], []
        for (w0, nw, tls, ths) in chunk_plan:
            for half in (0, 2):   # lo block then hi block
                Ts = tls if half == 0 else ths
                for wi in range(nw):
                    s_arr, d_arr = (core_win[k][w0 + wi][half],
                                    core_win[k][w0 + wi][half + 1])
                    T = Ts[wi]
                    pad = T * 128 - len(s_arr)
                    idx_list.append(np.concatenate(
                        [s_arr, np.zeros(pad, np.int64)]))
                    slot_list.append(np.concatenate(
                        [d_arr, np.full(pad, PAD_SLOT)]))
        idx_all = np.concatenate(idx_list)
        slot_all = np.concatenate(slot_list).astype(f32)
        idx_sb = _wrap16(idx_all, 128)
        dst_sb = np.ascontiguousarray(
            slot_all.reshape(-1, 128).T.astype(bf16))

        bl = batch[lo_n:hi_n]
        bcol = np.searchsorted(bl, np.arange(g.G), side="right") - 1
        bprev = np.concatenate([[-1], bcol[:-1]])
        gap_last = _wrap16(np.where(bcol < 0, 0, bcol + 1), 96)
        gap_prev = _wrap16(np.where(bprev < 0, 0, bprev + 1), 96)
        present = np.zeros(g.G, bool)
        present[np.unique(bl)] = True
        maxcol = _wrap16(np.where(present, bcol, g.NLP - 1), 96)

        gstart = np.searchsorted(bl, np.arange(g.G), side="left")
        maskneg = np.zeros(g.NLP, f32)
        maskneg[np.unique(gstart[present])] = -1e30
        maskneg[g.NL] = -1e30
        maskneg_row = maskneg.reshape(1, g.NLP).astype(bf16)

        bidx = np.zeros(g.NLP, np.int64)
        bidx[:g.NL] = bl
        batch_idx = _wrap16(bidx, 128)

        xT = np.zeros((g.F_IN, g.NLP), f32)
        xT[:, :g.NL] = x[lo_n:hi_n].T
        dinv_loc = np.zeros(g.NLP, f32)
        dinv_loc[:g.NL] = dinv[lo_n:hi_n]
        dinv_sb = np.ascontiguousarray(dinv_loc.reshape(g.NWIN, WIN).T)

        H, H2, O = g.HID, g.HID // 2, g.OUT
        inp = {
            "xT": xT.astype(bf16),
            "mean1T": mean1T,
            "batch_idx": batch_idx,
            "gap_last": gap_last, "gap_prev": gap_prev, "maxcol": maxcol,
            "maskneg_row": maskneg_row,
            "eidx": idx_sb,
            "dstslot": dst_sb,
            "dinv_sb": dinv_sb,
            "iota128": np.tile(np.arange(WIN, dtype=f32), (128, 1)).astype(bf16),
            "id96": np.eye(96, dtype=f32),
            "id128b": np.eye(128, dtype=bf16),
            "invc_row": invc.reshape(1, g.G),
            "W1a": np.asarray(W1[:g.F_IN], bf16),
            "W1b": np.asarray(W1[g.F_IN:], bf16),
            "W2a": np.asarray(W2[:H], bf16), "W2b": np.asarray(W2[H:], bf16),
            "W3a": np.asarray(W3[:H], bf16), "W3b": np.asarray(W3[H:], bf16),
            "b1_row": np.asarray(b1, f32).reshape(1, H),
            "b2_row": np.asarray(b2, f32).reshape(1, H),
            "b3_row": np.asarray(b3, f32).reshape(1, H),
            "Wl1a": np.asarray(Wl1[:H], f32), "Wl1b": np.asarray(Wl1[H:], f32),
            "Wl2": np.asarray(Wl2, f32), "Wl3": np.asarray(Wl3, f32),
            "bl1": np.asarray(bl1, f32).reshape(H, 1),
            "bl2": np.asarray(bl2, f32).reshape(H2, 1),
            "bl3": np.asarray(bl3, f32).reshape(O, 1),
        }
        per_core.append(inp)

    meta = {"chunk_plan": chunk_plan,
            "Ttot": sum(T_lo) + sum(T_hi)}
    return per_core, meta


# ---------------- device program ----------------


def build_program(geo, meta, n_cores):
    import os
    dbg_layers = int(os.environ.get("KN_LAYERS", "3"))
    dbg_stage = os.environ.get("KN_STAGE", "full")
    dbg_agg = os.environ.get("KN_AGG", "ep")
    import concourse.bacc as bacc
    import concourse.bass as bass
    import concourse.mybir as mybir
    import concourse.tile as tile

    g = geo
    H, H2, O = g.HID, g.HID // 2, g.OUT
    dt = mybir.dt
    Alu = mybir.AluOpType
    Act = mybir.ActivationFunctionType
    chunk_plan = meta["chunk_plan"]
    Ttot = meta["Ttot"]
    Tmax = max(sum(tls) + sum(ths) for (_, _, tls, ths) in chunk_plan)
    Tblk = max(max(max(tls), max(ths)) for (_, _, tls, ths) in chunk_plan)

    nc = bacc.Bacc("TRN2", target_bir_lowering=False, debug=False,
                   num_devices=n_cores)
    rg = [list(range(n_cores))]

    def din(name, shape, dtype):
        return nc.dram_tensor(name, list(shape), dtype, kind="ExternalInput")

    xT_d = din("xT", [g.F_IN, g.NLP], dt.bfloat16)
    mean1T_d = din("mean1T", [g.F_IN, g.G], dt.float32)
    batch_idx_d = din("batch_idx", [128, g.NLP // 16], dt.int16)
    gap_last_d = din("gap_last", [96, g.G // 16], dt.int16)
    gap_prev_d = din("gap_prev", [96, g.G // 16], dt.int16)
    maxcol_d = din("maxcol", [96, g.G // 16], dt.int16)
    maskneg_d = din("maskneg_row", [1, g.NLP], dt.bfloat16)
    eidx_d = din("eidx", [128, Ttot * 8], dt.int16)
    dstslot_d = din("dstslot", [128, Ttot], dt.bfloat16)
    dinv_d = din("dinv_sb", [128, g.NWIN], dt.float32)
    iota_d = din("iota128", [128, 128], dt.bfloat16)
    id96_d = din("id96", [96, 96], dt.float32)
    id128b_d = din("id128b", [128, 128], dt.bfloat16)
    invc_d = din("invc_row", [1, g.G], dt.float32)
    W_d = {n: din(n, [g.F_IN if n[1] == "1" else H, H], dt.bfloat16)
           for n in ("W1a", "W1b", "W2a", "W2b", "W3a", "W3b")}
    b_d = {n: din(n, [1, H], dt.float32)
           for n in ("b1_row", "b2_row", "b3_row")}
    Wl1a_d = din("Wl1a", [H, H], dt.float32)
    Wl1b_d = din("Wl1b", [H, H], dt.float32)
    Wl2_d = din("Wl2", [H, H2], dt.float32)
    Wl3_d = din("Wl3", [H2, O], dt.float32)
    bl1_d = din("bl1", [H, 1], dt.float32)
    bl2_d = din("bl2", [H2, 1], dt.float32)
    bl3_d = din("bl3", [O, 1], dt.float32)

    out_d = nc.dram_tensor("out", [O, g.G], dt.float32, kind="ExternalOutput")

    # internal DRAM
    tshard = nc.dram_tensor("tshard", [g.NLP, 128], dt.bfloat16,
                            kind="Internal")
    table = nc.dram_tensor("table", [g.NP, 128], dt.bfloat16,
                           kind="Internal", addr_space="Shared")
    gap_in = [nc.dram_tensor(f"gap_in{i}", [96, g.G], dt.float32,
                             kind="Internal") for i in range(3)]
    gap_out = [nc.dram_tensor(f"gap_out{i}", [96, g.G], dt.float32,
                              kind="Internal", addr_space="Shared")
               for i in range(3)]
    gmp_in = nc.dram_tensor("gmp_in", [96, 3 * g.G], dt.float32,
                            kind="Internal")
    gmp_out = nc.dram_tensor("gmp_out", [96, 3 * g.G], dt.float32,
                             kind="Internal", addr_space="Shared")

    with tile.TileContext(nc) as tc:
        import contextlib
        stk = contextlib.ExitStack()
        pp = stk.enter_context(tc.tile_pool(name="persist", bufs=1))
        wk = stk.enter_context(tc.tile_pool(name="work", bufs=2))
        ep = stk.enter_context(tc.tile_pool(name="epil", bufs=2))
        ps_conv = stk.enter_context(
            tc.tile_pool(name="ps_conv", bufs=2, space="PSUM"))
        ps_tr = stk.enter_context(
            tc.tile_pool(name="ps_tr", bufs=2, space="PSUM"))
        ps_agg = stk.enter_context(
            tc.tile_pool(name="ps_agg", bufs=4, space="PSUM"))

        def load(pool, dram, shape, dtype, tag, bcast=None):
            t = pool.tile(shape, dtype, tag=tag, name=tag)
            src = dram.ap() if bcast is None else dram.ap().to_broadcast(bcast)
            nc.sync.dma_start(out=t[:], in_=src)
            return t

        # constants
        iota_sb = load(pp, iota_d, [128, 128], dt.bfloat16, "iota")
        id96_sb = load(pp, id96_d, [96, 96], dt.float32, "id96")
        id128b_sb = load(pp, id128b_d, [128, 128], dt.bfloat16, "id128b")
        dinv_sbT = load(pp, dinv_d, [128, g.NWIN], dt.float32, "dinv")
        invc_bc = load(pp, invc_d, [96, g.G], dt.float32, "invc",
                       bcast=(96, g.G))
        maskneg_sb = load(pp, maskneg_d, [96, g.NLP], dt.bfloat16, "maskn",
                          bcast=(96, g.NLP))
        eidx_sb = load(pp, eidx_d, [128, Ttot * 8], dt.int16, "eidx")
        dst_sb = load(pp, dstslot_d, [128, Ttot], dt.bfloat16, "dstslot")
        batch_idx_sb = load(pp, batch_idx_d, [128, g.NLP // 16], dt.int16,
                            "bidx")
        gap_last_sb = load(pp, gap_last_d, [96, g.G // 16], dt.int16, "glast")
        gap_prev_sb = load(pp, gap_prev_d, [96, g.G // 16], dt.int16, "gprev")
        maxcol_sb = load(pp, maxcol_d, [96, g.G // 16], dt.int16, "maxcol")
        mean1_sb = load(pp, mean1T_d, [g.F_IN, g.G], dt.float32, "mean1")
        W_sb = {n: load(pp, W_d[n], list(W_d[n].shape), dt.bfloat16, n)
                for n in W_d}
        b_bc = {n: load(pp, b_d[n], [128, H], dt.float32, n, bcast=(128, H))
                for n in b_d}
        Wl1a_sb = load(pp, Wl1a_d, [H, H], dt.float32, "Wl1a")
        Wl1b_sb = load(pp, Wl1b_d, [H, H], dt.float32, "Wl1b")
        Wl2_sb = load(pp, Wl2_d, [H, H2], dt.float32, "Wl2")
        Wl3_sb = load(pp, Wl3_d, [H2, O], dt.float32, "Wl3")
        bl1_sb = load(pp, bl1_d, [H, 1], dt.float32, "bl1")
        bl2_sb = load(pp, bl2_d, [H2, 1], dt.float32, "bl2")
        bl3_sb = load(pp, bl3_d, [O, 1], dt.float32, "bl3")

        xbuf = [pp.tile([128, g.NLP], dt.bfloat16, tag=f"xbuf{i}", name=f"xbuf{i}")
                for i in range(2)]
        nc.sync.dma_start(out=xbuf[0][:g.F_IN, :], in_=xT_d.ap())

        scano = pp.tile([96, g.NLP], dt.float32, tag="scano", name="scano")
        scanin = pp.tile([96, g.NLP], dt.bfloat16, tag="scanin", name="scanin")
        gmp_all = pp.tile([96, 3 * g.G], dt.float32, tag="gmp_all", name="gmp_all")
        mg_sb = [pp.tile([96, g.G], dt.float32, tag=f"mg{i}", name=f"mg{i}")
                 for i in range(3)]
        gapar_sb = [pp.tile([96, g.G], dt.float32, tag=f"gapar{i}", name=f"gapar{i}")
                    for i in range(3)]

        nc.gpsimd.memset(scanin[:, 0:1], 0.0)
        if g.NL + 1 < g.NLP:
            nc.gpsimd.memset(scanin[:, g.NL + 1:g.NLP], 0.0)
        zinit = wk.tile([128, Tmax, 128], dt.bfloat16, tag="gath", name="zinit")
        nc.gpsimd.memset(zinit[:, :g.NWIN, :], 0.0)
        nc.sync.dma_start(
            out=tshard.ap().rearrange("(w p) c -> p w c", p=128),
            in_=zinit[:, :g.NWIN, :])

        layer_W = [("W1a", "W1b", "b1_row"), ("W2a", "W2b", "b2_row"),
                   ("W3a", "W3b", "b3_row")]

        for l in range(dbg_layers):
            F = g.F_IN if l == 0 else H
            Wa = W_sb[layer_W[l][0]]
            Wb = W_sb[layer_W[l][1]]
            bias = b_bc[layer_W[l][2]]
            x_src, x_dst = xbuf[l % 2], xbuf[(l + 1) % 2]
            meanT = mean1_sb if l == 0 else mg_sb[l - 1]

            # ---- conv + gather-table build ----
            for (cs, cw) in g.conv_chunks:
                xg_f = wk.tile([128, CONV_CHUNK], dt.float32, tag="xgf", name="xgf")
                nc.gpsimd.ap_gather(
                    xg_f[:F, :cw], meanT[:F, :],
                    batch_idx_sb[:F, cs // 16:(cs + cw) // 16],
                    channels=F, num_elems=g.G, d=1, num_idxs=cw)
                xg_b = wk.tile([128, CONV_CHUNK], dt.bfloat16, tag="xgb", name="xgb")
                nc.scalar.copy(out=xg_b[:F, :cw], in_=xg_f[:F, :cw])
                psc = ps_conv.tile([96, CONV_CHUNK], dt.float32, tag="psc", name="psc")
                nc.tensor.matmul(out=psc[:, :cw], lhsT=Wa[:],
                                 rhs=x_src[:F, cs:cs + cw],
                                 start=True, stop=False)
                nc.tensor.matmul(out=psc[:, :cw], lhsT=Wb[:],
                                 rhs=xg_b[:F, :cw], start=False, stop=True)
                hT = wk.tile([96, CONV_CHUNK], dt.float32, tag="hT", name="hT")
                nc.vector.tensor_copy(out=hT[:, :cw], in_=psc[:, :cw])
                for wo in range(0, cw, 128):
                    w = (cs + wo) // 128
                    pt = ps_tr.tile([128, 128], dt.float32, tag="ptr", name="ptr")
                    nc.tensor.transpose(out=pt[:, :96],
                                        in_=hT[:, wo:wo + 128],
                                        identity=id96_sb[:])
                    tab = wk.tile([128, 128], dt.bfloat16, tag="tab", name="tab")
                    nc.scalar.activation(out=tab[:, :96], in_=pt[:, :96],
                                         func=Act.Copy,
                                         scale=dinv_sbT[:, w:w + 1])
                    nc.sync.dma_start(
                        out=tshard.ap()[w * 128:(w + 1) * 128, :96],
                        in_=tab[:, :96])

            if dbg_stage == "conv":
                break
            # ---- all-gather the table ----
            nc.gpsimd.collective_compute(
                "AllGather", Alu.bypass, replica_groups=rg,
                ins=[tshard.ap()], outs=[table.ap()])

            if dbg_stage == "ag":
                break
            # ---- edge aggregation ----
            t_base = 0
            for (w0, nw, tls, ths) in chunk_plan:
                Tlo, Thi = sum(tls), sum(ths)
                T = Tlo + Thi
                gath = wk.tile([128, Tmax, 128], dt.bfloat16, tag="gath", name="gath")
                # dma_gather is limited to 1024 indices (8 tiles) per call
                for (goff, tstart, cnt, hi) in (
                        (0, t_base, Tlo, False),
                        (Tlo, t_base + Tlo, Thi, True)):
                    if cnt == 0:
                        continue
                    tbl_ap = table.ap()[HALF:, :] if hi else table.ap()
                    done = 0
                    while done < cnt:
                        nt = min(8, cnt - done)
                        nc.gpsimd.dma_gather(
                            gath[:, goff + done:goff + done + nt, :],
                            tbl_ap,
                            eidx_sb[:, 8 * (tstart + done):
                                    8 * (tstart + done + nt)],
                            num_idxs=nt * 128, num_idxs_reg=nt * 128,
                            elem_size=128)
                        done += nt
                if dbg_agg == "gather":
                    t_base += T
                    continue
                # blocks of tiles: (window-in-chunk, tile offset, count)
                blocks = [(wi, sum(tls[:wi]), tls[wi]) for wi in range(nw)] \
                    + [(wi, Tlo + sum(ths[:wi]), ths[wi]) for wi in range(nw)
                       if ths[wi] > 0]
                pags = [ps_agg.tile([96, 128], dt.float32, tag="pag", name="pag")
                        for _ in range(nw)]
                remaining = [tls[wi] + ths[wi] for wi in range(nw)]
                started = [False] * nw
                for (wi, toff, Tb) in blocks:
                    oh = wk.tile([128, Tblk, 128], dt.bfloat16, tag="oh",
                                 name="oh", bufs=4)
                    a = t_base + toff
                    nc.vector.tensor_tensor(
                        out=oh[:, 0:Tb, :],
                        in0=dst_sb[:, a:a + Tb, None]
                            .to_broadcast((128, Tb, 128)),
                        in1=iota_sb[:, None, :].to_broadcast((128, Tb, 128)),
                        op=Alu.is_equal)
                    for t in range(Tb):
                        if dbg_agg == "onehot":
                            continue
                        remaining[wi] -= 1
                        nc.tensor.matmul(out=pags[wi][:],
                                         lhsT=gath[:, toff + t, :96],
                                         rhs=oh[:, t, :],
                                         start=not started[wi],
                                         stop=remaining[wi] == 0)
                        started[wi] = True

                for wi in range(nw):
                    if dbg_agg in ("onehot", "mm"):
                        break
                    w = w0 + wi
                    sb1 = ep.tile([96, 128], dt.float32, tag="ep1", name="ep1")
                    nc.vector.tensor_copy(out=sb1[:], in_=pags[wi][:])
                    pt2 = ps_tr.tile([128, 128], dt.float32, tag="ptr", name="ptr")
                    nc.tensor.transpose(out=pt2[:, :96], in_=sb1[:],
                                        identity=id96_sb[:])
                    sb2 = ep.tile([128, 96], dt.float32, tag="ep2", name="ep2")
                    nc.scalar.activation(out=sb2[:], in_=pt2[:, :96],
                                         func=Act.Copy,
                                         scale=dinv_sbT[:, w:w + 1])
                    sb3 = ep.tile([128, 96], dt.bfloat16, tag="ep3", name="ep3")
                    if l < 2:
                        tmp = ep.tile([128, 96], dt.float32, tag="ep3f", name="ep3f")
                        nc.vector.tensor_add(out=tmp[:], in0=sb2[:],
                                             in1=bias[:, :96])
                        nc.vector.tensor_scalar_max(out=sb3[:], in0=tmp[:],
                                                    scalar1=0.0)
                    else:
                        nc.vector.tensor_add(out=sb3[:], in0=sb2[:],
                                             in1=bias[:, :96])
                    pt3 = ps_tr.tile([128, 128], dt.bfloat16, tag="ptr", name="ptr")
                    nc.tensor.transpose(out=pt3[:96, :], in_=sb3[:],
                                        identity=id128b_sb[:])
                    nc.vector.tensor_copy(
                        out=x_dst[:96, w * 128:(w + 1) * 128],
                        in_=pt3[:96, :])
                t_base += T

            nc.gpsimd.memset(x_dst[:96, g.NL:g.NLP], PAD_VAL)

            if dbg_stage == "agg":
                break
            # ---- pooling ----
            nc.vector.tensor_copy(out=scanin[:, 1:g.NL + 1],
                                  in_=x_dst[:96, 0:g.NL])
            nc.vector.tensor_tensor_scan(
                out=scano[:], data0=scanin[:], data1=scanin[:],
                initial=0.0, op0=Alu.add, op1=Alu.bypass)
            cum_l = ep.tile([96, g.G], dt.float32, tag="cuml", name="cuml", bufs=1)
            cum_p = ep.tile([96, g.G], dt.float32, tag="cump", name="cump", bufs=1)
            nc.gpsimd.ap_gather(cum_l[:], scano[:], gap_last_sb[:],
                                channels=96, num_elems=g.NLP, d=1,
                                num_idxs=g.G)
            nc.gpsimd.ap_gather(cum_p[:], scano[:], gap_prev_sb[:],
                                channels=96, num_elems=g.NLP, d=1,
                                num_idxs=g.G)
            gaps = ep.tile([96, g.G], dt.float32, tag="gaps", name="gaps", bufs=1)
            nc.vector.tensor_tensor(out=gaps[:], in0=cum_l[:], in1=cum_p[:],
                                    op=Alu.subtract)
            nc.sync.dma_start(out=gap_in[l].ap(), in_=gaps[:])
            nc.gpsimd.collective_compute(
                "AllReduce", Alu.add, replica_groups=rg,
                ins=[gap_in[l].ap()], outs=[gap_out[l].ap()])
            nc.sync.dma_start(out=gapar_sb[l][:], in_=gap_out[l].ap())

            nc.vector.tensor_tensor_scan(
                out=scano[:], data0=maskneg_sb[:], data1=x_dst[:96, :],
                initial=0.0, op0=Alu.add, op1=Alu.max)
            nc.gpsimd.ap_gather(gmp_all[:, l * g.G:(l + 1) * g.G],
                                scano[:], maxcol_sb[:],
                                channels=96, num_elems=g.NLP, d=1,
                                num_idxs=g.G)

            nc.vector.tensor_mul(out=mg_sb[l][:], in0=gapar_sb[l][:],
                                 in1=invc_bc[:])

        debug_cut = dbg_stage != "full" or dbg_layers < 3
        if debug_cut:
            nc.gpsimd.dma_start(out=out_d.ap(), in_=xbuf[0][:O, :g.G])
        # ---- final readout MLP (f32) ----
        if not debug_cut:
            nc.sync.dma_start(out=gmp_in.ap(), in_=gmp_all[:])
            nc.gpsimd.collective_compute(
                "AllReduce", Alu.max, replica_groups=rg,
                ins=[gmp_in.ap()], outs=[gmp_out.ap()])
            gmpar = pp.tile([96, 3 * g.G], dt.float32, tag="gmpar", name="gmpar")
            nc.sync.dma_start(out=gmpar[:], in_=gmp_out.ap())

            hTa = pp.tile([96, g.G], dt.float32, tag="hTa", name="hTa")
            hTb = pp.tile([96, g.G], dt.float32, tag="hTb", name="hTb")
            nc.vector.tensor_add(out=hTa[:], in0=gmpar[:, 0:g.G],
                                 in1=gmpar[:, g.G:2 * g.G])
            nc.vector.tensor_add(out=hTa[:], in0=hTa[:],
                                 in1=gmpar[:, 2 * g.G:3 * g.G])
            nc.vector.tensor_add(out=hTb[:], in0=mg_sb[0][:], in1=mg_sb[1][:])
            nc.vector.tensor_add(out=hTb[:], in0=hTb[:], in1=mg_sb[2][:])

            ps1 = ps_conv.tile([96, g.G], dt.float32, tag="psc", name="psc")
            nc.tensor.matmul(out=ps1[:], lhsT=Wl1a_sb[:], rhs=hTa[:],
                             start=True, stop=False)
            nc.tensor.matmul(out=ps1[:], lhsT=Wl1b_sb[:], rhs=hTb[:],
                             start=False, stop=True)
            o1 = pp.tile([96, g.G], dt.float32, tag="o1", name="o1")
            nc.scalar.activation(out=o1[:], in_=ps1[:], func=Act.Relu,
                                 bias=bl1_sb[:])
            ps2 = ps_conv.tile([96, g.G], dt.float32, tag="psc", name="psc")
            nc.tensor.matmul(out=ps2[:H2, :], lhsT=Wl2_sb[:], rhs=o1[:],
                             start=True, stop=True)
            o2 = pp.tile([H2, g.G], dt.float32, tag="o2", name="o2")
            nc.scalar.activation(out=o2[:], in_=ps2[:H2, :], func=Act.Relu,
                                 bias=bl2_sb[:])
            ps3 = ps_conv.tile([96, g.G], dt.float32, tag="psc", name="psc")
            nc.tensor.matmul(out=ps3[:O, :], lhsT=Wl3_sb[:], rhs=o2[:],
                             start=True, stop=True)
            o3 = pp.tile([O, g.G], dt.float32, tag="o3", name="o3")
            nc.scalar.activation(out=o3[:], in_=ps3[:O, :], func=Act.Identity,
                                 bias=bl3_sb[:])
            nc.sync.dma_start(out=out_d.ap(), in_=o3[:])

        stk.close()

    nc.compile()
    return nc


_CACHE = {}


def _get_program(geo, meta, n_cores):
    key = (repr(sorted(geo.__dict__.items(), key=str)),
           repr(meta["chunk_plan"]), n_cores)
    if key not in _CACHE:
        _CACHE[key] = build_program(geo, meta, n_cores)
    return _CACHE[key]


def kernel(**inputs):
    from concourse.bass_utils import run_bass_kernel_spmd

    geo = Geo(CFG)
    inputs = {k: np.asarray(v) for k, v in inputs.items()}
    per_core, meta = prep(geo, **inputs)
    nc = _get_program(geo, meta, geo.C)
    res = run_bass_kernel_spmd(nc, per_core, core_ids=list(range(geo.C)))
    out = np.asarray(res.results[0]["out"], f32)   # [OUT, G]
    return np.ascontiguousarray(out.T)             # [G, OUT] float32



# revision 24
# speedup vs baseline: 1.0233x; 1.0233x over previous
"""Trainium2 Bass kernel for nn_BenchGNN_29300266893894 (3-layer GCN with
global-feature concat + global mean/max pooling readout + MLP head).

Self-contained: host-side sharding/packing prep + SPMD Bass/Tile program on
8 NeuronCores via run_bass_kernel_spmd.

Sharding: nodes are split into 8 contiguous shards of 6250 (padded to 6272 =
49*128). Edges are partitioned by dst owner and sorted by dst window; weight
matrices are replicated. Per layer, each core computes h = x_cat @ W for its
nodes, builds a bf16 "gather table" of h' = h*dinv rows (padded to 256 B),
AllGathers it, then aggregates its incident edges with hardware dma_gather +
one-hot matmuls accumulating per-128-dst-window PSUM tiles. Graph pooling is
done with DVE segmented scans + ap_gather boundary extraction and AllReduce.
"""
import sys
import numpy as np
import ml_dtypes

sys.path.insert(0, "/opt/trn_rl_repo")

bf16 = ml_dtypes.bfloat16
f32 = np.float32

# ---------------- problem geometry (hardcoded) ----------------
CFG = dict(N=50000, E=800000, G=512, F_IN=128, HID=96, OUT=10, C=8)

WIN = 128
HALF = 32768
CHUNK_WINDOWS = 4
CONV_CHUNK = 512
PAD_SLOT = 999.0
PAD_VAL = -60000.0


class Geo:
    def __init__(self, cfg):
        self.__dict__.update(cfg)
        assert self.N % self.C == 0
        self.NL = self.N // self.C
        self.NWIN = -(-self.NL // WIN)
        self.NLP = self.NWIN * WIN
        # pooling pad-column tricks need at least one pad column per core
        assert self.NLP > self.NL
        self.NP = self.NLP * self.C
        self.conv_chunks = [(o, min(CONV_CHUNK, self.NLP - o))
                            for o in range(0, self.NLP, CONV_CHUNK)]


def _wrap16(idx, channels):
    idx = np.asarray(idx, np.int16)
    assert len(idx) % 16 == 0
    idx16 = idx.reshape(-1, 16).T
    return np.ascontiguousarray(np.tile(idx16, (channels // 16, 1)))


def prep(geo, x, edge_index, batch, W1, b1, W2, b2, W3, b3,
         Wl1, bl1, Wl2, bl2, Wl3, bl3):
    g = geo
    x = np.asarray(x, f32)
    src = np.asarray(edge_index[0], np.int64)
    dst = np.asarray(edge_index[1], np.int64)
    batch = np.asarray(batch, np.int64)

    def gid_of(n):
        return (n // g.NL) * g.NLP + (n % g.NL)

    deg = np.bincount(dst, minlength=g.N).astype(f32) + 1.0
    dinv = (1.0 / np.sqrt(deg)).astype(f32)
    counts = np.bincount(batch, minlength=g.G).astype(f32)
    invc = (1.0 / np.maximum(counts, 1.0)).astype(f32)

    sums1 = np.zeros((g.G, g.F_IN), f32)
    np.add.at(sums1, batch, x)
    mean1 = (sums1 * invc[:, None]).astype(f32)          # [G, F_IN]
    U1rows = (mean1 @ np.asarray(W1[g.F_IN:], f32))      # [G, HID]

    src_g = gid_of(src)

    core_win = []
    for k in range(g.C):
        lo_n, hi_n = k * g.NL, (k + 1) * g.NL
        sel = (dst >= lo_n) & (dst < hi_n)
        es, ed = src_g[sel], (dst[sel] - lo_n)
        order = np.argsort(ed, kind="stable")
        es, ed = es[order], ed[order]
        wstart = np.searchsorted(ed, np.arange(0, g.NLP + 1, WIN))
        wins = []
        for w in range(g.NWIN):
            a, b = wstart[w], wstart[w + 1]
            ws, wd = es[a:b], ed[a:b] - w * WIN
            lo_m = ws < HALF
            wins.append((ws[lo_m], wd[lo_m], ws[~lo_m] - HALF, wd[~lo_m]))
        core_win.append(wins)

    T_lo = [max(0, max(-(-len(core_win[k][w][0]) // 128) for k in range(g.C)))
            for w in range(g.NWIN)]
    T_hi = [max(0, max(-(-len(core_win[k][w][2]) // 128) for k in range(g.C)))
            for w in range(g.NWIN)]

    # chunk plan: groups of CHUNK_WINDOWS windows; per chunk pack
    # [lo tiles of each window][hi tiles of each window]
    chunk_plan = []   # (w0, nwin, [T_lo..], [T_hi..])
    for w0 in range(0, g.NWIN, CHUNK_WINDOWS):
        nw = min(CHUNK_WINDOWS, g.NWIN - w0)
        chunk_plan.append((w0, nw, T_lo[w0:w0 + nw], T_hi[w0:w0 + nw]))

    per_core = []
    for k in range(g.C):
        lo_n, hi_n = k * g.NL, (k + 1) * g.NL
        idx_list, slot_list = [], []
        for (w0, nw, tls, ths) in chunk_plan:
            for half in (0, 2):   # lo block then hi block
                Ts = tls if half == 0 else ths
                for wi in range(nw):
                    s_arr, d_arr = (core_win[k][w0 + wi][half],
                                    core_win[k][w0 + wi][half + 1])
                    T = Ts[wi]
                    pad = T * 128 - len(s_arr)
                    idx_list.append(np.concatenate(
                        [s_arr, np.zeros(pad, np.int64)]))
                    slot_list.append(np.concatenate(
                        [d_arr, np.full(pad, PAD_SLOT)]))
        idx_all = np.concatenate(idx_list)
        slot_all = np.concatenate(slot_list).astype(f32)
        idx_sb = _wrap16(idx_all, 128)
        dst_sb = np.ascontiguousarray(
            slot_all.reshape(-1, 128).T.astype(bf16))

        bl = batch[lo_n:hi_n]
        # one-hot graph-block matrices for the conv global-feature term:
        # ohconv[p, c, b, j] = 1{batch_local[cs_c + j] == 128*b + p}
        nch = len(g.conv_chunks)
        ohconv = np.zeros((128, nch, 4, CONV_CHUNK), bf16)
        for c, (cs, cw) in enumerate(g.conv_chunks):
            seg = bl[cs:min(cs + cw, g.NL)]
            for j, gid in enumerate(seg):
                ohconv[gid % 128, c, gid // 128, j] = 1
        bcol = np.searchsorted(bl, np.arange(g.G), side="right") - 1
        gap_last = _wrap16(np.where(bcol < 0, 0, bcol + 1), 96)
        present = np.zeros(g.G, bool)
        present[np.unique(bl)] = True
        maxcol = _wrap16(np.where(present, bcol, g.NLP - 1), 96)

        gstart = np.searchsorted(bl, np.arange(g.G), side="left")
        maskneg = np.zeros(g.NLP, f32)
        maskneg[np.unique(gstart[present])] = -1e30
        maskneg[g.NL] = -1e30
        maskneg_row = maskneg.reshape(1, g.NLP).astype(bf16)

        bidx = np.zeros(g.NLP, np.int64)
        bidx[:g.NL] = bl
        batch_idx = _wrap16(bidx, 128)

        xT = np.zeros((g.F_IN, g.NLP), f32)
        xT[:, :g.NL] = x[lo_n:hi_n].T
        dinv_loc = np.zeros(g.NLP, f32)
        dinv_loc[:g.NL] = dinv[lo_n:hi_n]
        dinv_sb = np.ascontiguousarray(dinv_loc.reshape(g.NWIN, WIN).T)

        H, H2, O = g.HID, g.HID // 2, g.OUT
        inp = {
            "xT": xT.astype(bf16),
            "U1rows": np.ascontiguousarray(
                U1rows.reshape(4, 128, H).transpose(1, 0, 2)).astype(bf16),
            "ohconv": np.ascontiguousarray(
                ohconv.reshape(128, 4 * nch, CONV_CHUNK)),
            "gap_last": gap_last, "maxcol": maxcol,
            "maskneg_row": maskneg_row,
            "eidx": idx_sb,
            "dstslot": dst_sb,
            "dinv_sb": dinv_sb,
            "iota128": np.tile(np.arange(WIN, dtype=f32), (128, 1)).astype(bf16),
            "id96": np.eye(96, dtype=f32),
            "id128b": np.eye(128, dtype=bf16),
            "invc_row": invc.reshape(1, g.G),
            "W1a": np.asarray(W1[:g.F_IN], bf16),
            "W1b": np.asarray(W1[g.F_IN:], bf16),
            "W2a": np.asarray(W2[:H], bf16), "W2b": np.asarray(W2[H:], bf16),
            "W3a": np.asarray(W3[:H], bf16), "W3b": np.asarray(W3[H:], bf16),
            "b1_row": np.asarray(b1, f32).reshape(1, H),
            "b2_row": np.asarray(b2, f32).reshape(1, H),
            "b3_row": np.asarray(b3, f32).reshape(1, H),
            "Wl1a": np.asarray(Wl1[:H], f32), "Wl1b": np.asarray(Wl1[H:], f32),
            "Wl2": np.asarray(Wl2, f32), "Wl3": np.asarray(Wl3, f32),
            "bl1": np.asarray(bl1, f32).reshape(H, 1),
            "bl2": np.asarray(bl2, f32).reshape(H2, 1),
            "bl3": np.asarray(bl3, f32).reshape(O, 1),
        }
        per_core.append(inp)

    meta = {"chunk_plan": chunk_plan,
            "Ttot": sum(T_lo) + sum(T_hi)}
    return per_core, meta


# ---------------- device program ----------------


def build_program(geo, meta, n_cores):
    import os
    dbg_layers = int(os.environ.get("KN_LAYERS", "3"))
    dbg_stage = os.environ.get("KN_STAGE", "full")
    dbg_agg = os.environ.get("KN_AGG", "ep")
    import concourse.bacc as bacc
    import concourse.bass as bass
    import concourse.mybir as mybir
    import concourse.tile as tile

    g = geo
    H, H2, O = g.HID, g.HID // 2, g.OUT
    dt = mybir.dt
    Alu = mybir.AluOpType
    Act = mybir.ActivationFunctionType
    chunk_plan = meta["chunk_plan"]
    Ttot = meta["Ttot"]
    Tmax = max(sum(tls) + sum(ths) for (_, _, tls, ths) in chunk_plan)
    Tblk = max(max(max(tls), max(ths)) for (_, _, tls, ths) in chunk_plan)

    nc = bacc.Bacc("TRN2", target_bir_lowering=False, debug=False,
                   num_devices=n_cores)
    rg = [list(range(n_cores))]

    def din(name, shape, dtype):
        return nc.dram_tensor(name, list(shape), dtype, kind="ExternalInput")

    xT_d = din("xT", [g.F_IN, g.NLP], dt.bfloat16)
    U1rows_d = din("U1rows", [128, 4, H], dt.bfloat16)
    ohconv_d = din("ohconv", [128, 4 * len(g.conv_chunks), CONV_CHUNK],
                   dt.bfloat16)
    gap_last_d = din("gap_last", [96, g.G // 16], dt.int16)
    maxcol_d = din("maxcol", [96, g.G // 16], dt.int16)
    maskneg_d = din("maskneg_row", [1, g.NLP], dt.bfloat16)
    eidx_d = din("eidx", [128, Ttot * 8], dt.int16)
    dstslot_d = din("dstslot", [128, Ttot], dt.bfloat16)
    dinv_d = din("dinv_sb", [128, g.NWIN], dt.float32)
    iota_d = din("iota128", [128, 128], dt.bfloat16)
    id96_d = din("id96", [96, 96], dt.float32)
    id128b_d = din("id128b", [128, 128], dt.bfloat16)
    invc_d = din("invc_row", [1, g.G], dt.float32)
    W_d = {n: din(n, [g.F_IN if n[1] == "1" else H, H], dt.bfloat16)
           for n in ("W1a", "W1b", "W2a", "W2b", "W3a", "W3b")}
    b_d = {n: din(n, [1, H], dt.float32)
           for n in ("b1_row", "b2_row", "b3_row")}
    Wl1a_d = din("Wl1a", [H, H], dt.float32)
    Wl1b_d = din("Wl1b", [H, H], dt.float32)
    Wl2_d = din("Wl2", [H, H2], dt.float32)
    Wl3_d = din("Wl3", [H2, O], dt.float32)
    bl1_d = din("bl1", [H, 1], dt.float32)
    bl2_d = din("bl2", [H2, 1], dt.float32)
    bl3_d = din("bl3", [O, 1], dt.float32)

    out_d = nc.dram_tensor("out", [O, g.G], dt.float32, kind="ExternalOutput")

    # internal DRAM
    tshard = nc.dram_tensor("tshard", [g.NLP, 128], dt.bfloat16,
                            kind="Internal")
    table = nc.dram_tensor("table", [g.NP, 128], dt.bfloat16,
                           kind="Internal", addr_space="Shared")
    gap_in = [nc.dram_tensor(f"gap_in{i}", [96, g.G], dt.float32,
                             kind="Internal") for i in range(3)]
    gap_out = [nc.dram_tensor(f"gap_out{i}", [96, g.G], dt.float32,
                              kind="Internal", addr_space="Shared")
               for i in range(3)]
    gmp_in = [nc.dram_tensor(f"gmp_in{i}", [96, g.G], dt.float32,
                             kind="Internal") for i in range(3)]
    gmp_out = [nc.dram_tensor(f"gmp_out{i}", [96, g.G], dt.float32,
                              kind="Internal", addr_space="Shared")
               for i in range(3)]

    with tile.TileContext(nc) as tc:
        import contextlib
        stk = contextlib.ExitStack()
        pp = stk.enter_context(tc.tile_pool(name="persist", bufs=1))
        wk = stk.enter_context(tc.tile_pool(name="work", bufs=2))
        ep = stk.enter_context(tc.tile_pool(name="epil", bufs=2))
        ps_conv = stk.enter_context(
            tc.tile_pool(name="ps_conv", bufs=2, space="PSUM"))
        ps_tr = stk.enter_context(
            tc.tile_pool(name="ps_tr", bufs=2, space="PSUM"))
        ps_agg = stk.enter_context(
            tc.tile_pool(name="ps_agg", bufs=4, space="PSUM"))

        def load(pool, dram, shape, dtype, tag, bcast=None):
            t = pool.tile(shape, dtype, tag=tag, name=tag)
            src = dram.ap() if bcast is None else dram.ap().to_broadcast(bcast)
            nc.sync.dma_start(out=t[:], in_=src)
            return t

        # constants
        iota_sb = load(pp, iota_d, [128, 128], dt.bfloat16, "iota")
        id96_sb = load(pp, id96_d, [96, 96], dt.float32, "id96")
        id128b_sb = load(pp, id128b_d, [128, 128], dt.bfloat16, "id128b")
        dinv_sbT = load(pp, dinv_d, [128, g.NWIN], dt.float32, "dinv")
        invc_bc = load(pp, invc_d, [96, g.G], dt.float32, "invc",
                       bcast=(96, g.G))
        maskneg_sb = load(pp, maskneg_d, [96, g.NLP], dt.bfloat16, "maskn",
                          bcast=(96, g.NLP))
        eidx_sb = load(pp, eidx_d, [128, Ttot * 8], dt.int16, "eidx")
        dst_sb = load(pp, dstslot_d, [128, Ttot], dt.bfloat16, "dstslot")
        gap_last_sb = load(pp, gap_last_d, [96, g.G // 16], dt.int16, "glast")
        maxcol_sb = load(pp, maxcol_d, [96, g.G // 16], dt.int16, "maxcol")
        Urows = load(pp, U1rows_d, [128, 4, H], dt.bfloat16, "Urows")
        tabkeep = pp.tile([128, g.NWIN, 96], dt.bfloat16, tag="tabkeep",
                          name="tabkeep")
        W_sb = {n: load(pp, W_d[n], list(W_d[n].shape), dt.bfloat16, n)
                for n in W_d}
        b_bc = {n: load(pp, b_d[n], [128, H], dt.float32, n, bcast=(128, H))
                for n in b_d}
        Wl1a_sb = load(pp, Wl1a_d, [H, H], dt.float32, "Wl1a")
        Wl1b_sb = load(pp, Wl1b_d, [H, H], dt.float32, "Wl1b")
        Wl2_sb = load(pp, Wl2_d, [H, H2], dt.float32, "Wl2")
        Wl3_sb = load(pp, Wl3_d, [H2, O], dt.float32, "Wl3")
        bl1_sb = load(pp, bl1_d, [H, 1], dt.float32, "bl1")
        bl2_sb = load(pp, bl2_d, [H2, 1], dt.float32, "bl2")
        bl3_sb = load(pp, bl3_d, [O, 1], dt.float32, "bl3")

        xbuf = [pp.tile([128, g.NLP], dt.bfloat16, tag=f"xbuf{i}", name=f"xbuf{i}")
                for i in range(2)]
        nc.sync.dma_start(out=xbuf[0][:g.F_IN, :], in_=xT_d.ap())

        scano = pp.tile([96, g.NLP], dt.float32, tag="scano", name="scano")
        gmp_all = pp.tile([96, 3 * g.G], dt.float32, tag="gmp_all", name="gmp_all")
        mg_sb = [pp.tile([96, g.G], dt.float32, tag=f"mg{i}", name=f"mg{i}")
                 for i in range(3)]
        gapar_sb = [pp.tile([96, g.G], dt.float32, tag=f"gapar{i}", name=f"gapar{i}")
                    for i in range(3)]

        zinit = wk.tile([128, Tmax, 128], dt.bfloat16, tag="gath", name="zinit")
        nc.gpsimd.memset(zinit[:, :g.NWIN, :], 0.0)
        nc.sync.dma_start(
            out=tshard.ap().rearrange("(w p) c -> p w c", p=128),
            in_=zinit[:, :g.NWIN, :])

        layer_W = [("W1a", "W1b", "b1_row"), ("W2a", "W2b", "b2_row"),
                   ("W3a", "W3b", "b3_row")]

        for l in range(dbg_layers):
            F = g.F_IN if l == 0 else H
            Wa = W_sb[layer_W[l][0]]
            Wb = W_sb[layer_W[l][1]]
            bias = b_bc[layer_W[l][2]]
            x_src, x_dst = xbuf[l % 2], xbuf[(l + 1) % 2]

            # ---- conv + gather-table build ----
            for ci, (cs, cw) in enumerate(g.conv_chunks):
                ohc = wk.tile([128, 4, CONV_CHUNK], dt.bfloat16, tag="ohc",
                              name="ohc")
                nc.sync.dma_start(out=ohc[:],
                                  in_=ohconv_d.ap()[:, 4 * ci:4 * ci + 4, :])
                psc = ps_conv.tile([96, CONV_CHUNK], dt.float32, tag="psc", name="psc")
                nc.tensor.matmul(out=psc[:, :cw], lhsT=Wa[:],
                                 rhs=x_src[:F, cs:cs + cw],
                                 start=True, stop=False)
                for b in range(4):
                    nc.tensor.matmul(out=psc[:, :cw], lhsT=Urows[:, b, :],
                                     rhs=ohc[:, b, :cw], start=False,
                                     stop=b == 3)
                hT = wk.tile([96, CONV_CHUNK], dt.float32, tag="hT", name="hT")
                nc.vector.tensor_copy(out=hT[:, :cw], in_=psc[:, :cw])
                for wo in range(0, cw, 128):
                    w = (cs + wo) // 128
                    pt = ps_tr.tile([128, 128], dt.float32, tag="ptr", name="ptr")
                    nc.tensor.transpose(out=pt[:, :96],
                                        in_=hT[:, wo:wo + 128],
                                        identity=id96_sb[:])
                    nc.scalar.activation(out=tabkeep[:, w, :], in_=pt[:, :96],
                                         func=Act.Copy,
                                         scale=dinv_sbT[:, w:w + 1])
                    nc.sync.dma_start(
                        out=tshard.ap()[w * 128:(w + 1) * 128, :96],
                        in_=tabkeep[:, w, :])

            if dbg_stage == "conv":
                break
            # ---- all-gather the table ----
            nc.gpsimd.collective_compute(
                "AllGather", Alu.bypass, replica_groups=rg,
                ins=[tshard.ap()], outs=[table.ap()])

            if dbg_stage == "ag":
                break
            # ---- edge aggregation ----
            t_base = 0
            for (w0, nw, tls, ths) in chunk_plan:
                Tlo, Thi = sum(tls), sum(ths)
                T = Tlo + Thi
                gath = wk.tile([128, Tmax, 128], dt.bfloat16, tag="gath", name="gath")
                # dma_gather is limited to 1024 indices (8 tiles) per call
                for (goff, tstart, cnt, hi) in (
                        (0, t_base, Tlo, False),
                        (Tlo, t_base + Tlo, Thi, True)):
                    if cnt == 0:
                        continue
                    tbl_ap = table.ap()[HALF:, :] if hi else table.ap()
                    done = 0
                    while done < cnt:
                        nt = min(8, cnt - done)
                        nc.gpsimd.dma_gather(
                            gath[:, goff + done:goff + done + nt, :],
                            tbl_ap,
                            eidx_sb[:, 8 * (tstart + done):
                                    8 * (tstart + done + nt)],
                            num_idxs=nt * 128, num_idxs_reg=nt * 128,
                            elem_size=128)
                        done += nt
                if dbg_agg == "gather":
                    t_base += T
                    continue
                # blocks of tiles: (window-in-chunk, tile offset, count)
                blocks = [(wi, sum(tls[:wi]), tls[wi]) for wi in range(nw)
                          if tls[wi] > 0] \
                    + [(wi, Tlo + sum(ths[:wi]), ths[wi]) for wi in range(nw)
                       if ths[wi] > 0]
                pags = [ps_agg.tile([96, 128], dt.float32, tag="pag", name="pag")
                        for _ in range(nw)]
                remaining = [tls[wi] + ths[wi] for wi in range(nw)]
                started = [True] * nw
                for wi in range(nw):
                    # self-loop term: psum += table rows of this window
                    nc.tensor.matmul(out=pags[wi][:],
                                     lhsT=tabkeep[:, w0 + wi, :],
                                     rhs=id128b_sb[:],
                                     start=True, stop=remaining[wi] == 0)
                for (wi, toff, Tb) in blocks:
                    oh = wk.tile([128, Tblk, 128], dt.bfloat16, tag="oh",
                                 name="oh", bufs=4)
                    a = t_base + toff
                    nc.vector.tensor_tensor(
                        out=oh[:, 0:Tb, :],
                        in0=dst_sb[:, a:a + Tb, None]
                            .to_broadcast((128, Tb, 128)),
                        in1=iota_sb[:, None, :].to_broadcast((128, Tb, 128)),
                        op=Alu.is_equal)
                    for t in range(Tb):
                        if dbg_agg == "onehot":
                            continue
                        remaining[wi] -= 1
                        nc.tensor.matmul(out=pags[wi][:],
                                         lhsT=gath[:, toff + t, :96],
                                         rhs=oh[:, t, :],
                                         start=not started[wi],
                                         stop=remaining[wi] == 0)
                        started[wi] = True

                for wi in range(nw):
                    if dbg_agg in ("onehot", "mm"):
                        break
                    w = w0 + wi
                    sb1 = ep.tile([96, 128], dt.float32, tag="ep1", name="ep1")
                    nc.vector.tensor_copy(out=sb1[:], in_=pags[wi][:])
                    pt2 = ps_tr.tile([128, 128], dt.float32, tag="ptr", name="ptr")
                    nc.tensor.transpose(out=pt2[:, :96], in_=sb1[:],
                                        identity=id96_sb[:])
                    sb2 = ep.tile([128, 96], dt.float32, tag="ep2", name="ep2")
                    nc.scalar.activation(out=sb2[:], in_=pt2[:, :96],
                                         func=Act.Copy,
                                         scale=dinv_sbT[:, w:w + 1])
                    sb3 = ep.tile([128, 96], dt.bfloat16, tag="ep3", name="ep3")
                    if l < 2:
                        tmp = ep.tile([128, 96], dt.float32, tag="ep3f", name="ep3f")
                        nc.vector.tensor_add(out=tmp[:], in0=sb2[:],
                                             in1=bias[:, :96])
                        nc.vector.tensor_scalar_max(out=sb3[:], in0=tmp[:],
                                                    scalar1=0.0)
                    else:
                        nc.vector.tensor_add(out=sb3[:], in0=sb2[:],
                                             in1=bias[:, :96])
                    pt3 = ps_tr.tile([128, 128], dt.bfloat16, tag="ptr", name="ptr")
                    nc.tensor.transpose(out=pt3[:96, :], in_=sb3[:],
                                        identity=id128b_sb[:])
                    nc.vector.tensor_copy(
                        out=x_dst[:96, w * 128:(w + 1) * 128],
                        in_=pt3[:96, :])
                t_base += T

            nc.gpsimd.memset(x_dst[:96, g.NL:g.NLP], PAD_VAL)

            if dbg_stage == "agg":
                break
            # ---- pooling ----
            nc.vector.memset(scano[:, 0:1], 0.0)
            nc.vector.tensor_tensor_scan(
                out=scano[:, 1:g.NL + 1], data0=x_dst[:96, 0:g.NL],
                data1=x_dst[:96, 0:g.NL],
                initial=0.0, op0=Alu.add, op1=Alu.bypass)
            cum_l = ep.tile([96, g.G + 1], dt.float32, tag="cuml", name="cuml",
                            bufs=1)
            nc.vector.memset(cum_l[:, 0:1], 0.0)
            nc.gpsimd.ap_gather(cum_l[:, 1:g.G + 1], scano[:], gap_last_sb[:],
                                channels=96, num_elems=g.NLP, d=1,
                                num_idxs=g.G)
            gaps = ep.tile([96, g.G], dt.float32, tag="gaps", name="gaps", bufs=1)
            nc.vector.tensor_tensor(out=gaps[:], in0=cum_l[:, 1:g.G + 1],
                                    in1=cum_l[:, 0:g.G],
                                    op=Alu.subtract)
            nc.sync.dma_start(out=gap_in[l].ap(), in_=gaps[:])
            nc.gpsimd.collective_compute(
                "AllReduce", Alu.add, replica_groups=rg,
                ins=[gap_in[l].ap()], outs=[gap_out[l].ap()])
            nc.sync.dma_start(out=gapar_sb[l][:], in_=gap_out[l].ap())

            nc.vector.tensor_tensor_scan(
                out=scano[:], data0=maskneg_sb[:], data1=x_dst[:96, :],
                initial=0.0, op0=Alu.add, op1=Alu.max)
            nc.gpsimd.ap_gather(gmp_all[:, l * g.G:(l + 1) * g.G],
                                scano[:], maxcol_sb[:],
                                channels=96, num_elems=g.NLP, d=1,
                                num_idxs=g.G)
            # per-layer gmp AllReduce: layers 0-1 overlap the next layer's
            # edge gathers; only layer 2's is exposed
            nc.sync.dma_start(out=gmp_in[l].ap(),
                              in_=gmp_all[:, l * g.G:(l + 1) * g.G])
            nc.gpsimd.collective_compute(
                "AllReduce", Alu.max, replica_groups=rg,
                ins=[gmp_in[l].ap()], outs=[gmp_out[l].ap()])

            nc.vector.tensor_mul(out=mg_sb[l][:], in0=gapar_sb[l][:],
                                 in1=invc_bc[:])

            if l < 2:
                # build Urows for next layer: U = Wb_{l+1}^T @ mg,
                # transposed into [128 graph, 4 block, 96] for the conv
                Wbn = W_sb[layer_W[l + 1][1]]
                mgb = ep.tile([96, g.G], dt.bfloat16, tag="mgb", name="mgb",
                              bufs=1)
                nc.scalar.copy(out=mgb[:], in_=mg_sb[l][:])
                psU = ps_conv.tile([96, g.G], dt.float32, tag="psc", name="psc")
                nc.tensor.matmul(out=psU[:], lhsT=Wbn[:], rhs=mgb[:],
                                 start=True, stop=True)
                Usb = ep.tile([96, g.G], dt.bfloat16, tag="Usb", name="Usb",
                              bufs=1)
                nc.vector.tensor_copy(out=Usb[:], in_=psU[:])
                for b in range(4):
                    ptU = ps_tr.tile([128, 128], dt.bfloat16, tag="ptr",
                                     name="ptr")
                    nc.tensor.transpose(out=ptU[:, :96],
                                        in_=Usb[:, b * 128:(b + 1) * 128],
                                        identity=id128b_sb[:96, :96])
                    nc.scalar.copy(out=Urows[:, b, :], in_=ptU[:, :96])

        debug_cut = dbg_stage != "full" or dbg_layers < 3
        if debug_cut:
            nc.gpsimd.dma_start(out=out_d.ap(), in_=xbuf[0][:O, :g.G])
        # ---- final readout MLP (f32) ----
        if not debug_cut:
            gmpar = pp.tile([96, 3 * g.G], dt.float32, tag="gmpar", name="gmpar")
            for i in range(3):
                nc.sync.dma_start(out=gmpar[:, i * g.G:(i + 1) * g.G],
                                  in_=gmp_out[i].ap())

            hTa = pp.tile([96, g.G], dt.float32, tag="hTa", name="hTa")
            hTb = pp.tile([96, g.G], dt.float32, tag="hTb", name="hTb")
            nc.vector.tensor_add(out=hTa[:], in0=gmpar[:, 0:g.G],
                                 in1=gmpar[:, g.G:2 * g.G])
            nc.vector.tensor_add(out=hTa[:], in0=hTa[:],
                                 in1=gmpar[:, 2 * g.G:3 * g.G])
            nc.vector.tensor_add(out=hTb[:], in0=mg_sb[0][:], in1=mg_sb[1][:])
            nc.vector.tensor_add(out=hTb[:], in0=hTb[:], in1=mg_sb[2][:])

            ps1 = ps_conv.tile([96, g.G], dt.float32, tag="psc", name="psc")
            nc.tensor.matmul(out=ps1[:], lhsT=Wl1a_sb[:], rhs=hTa[:],
                             start=True, stop=False)
            nc.tensor.matmul(out=ps1[:], lhsT=Wl1b_sb[:], rhs=hTb[:],
                             start=False, stop=True)
            o1 = pp.tile([96, g.G], dt.float32, tag="o1", name="o1")
            nc.scalar.activation(out=o1[:], in_=ps1[:], func=Act.Relu,
                                 bias=bl1_sb[:])
            ps2 = ps_conv.tile([96, g.G], dt.float32, tag="psc", name="psc")
            nc.tensor.matmul(out=ps2[:H2, :], lhsT=Wl2_sb[:], rhs=o1[:],
                             start=True, stop=True)
            o2 = pp.tile([H2, g.G], dt.float32, tag="o2", name="o2")
            nc.scalar.activation(out=o2[:], in_=ps2[:H2, :], func=Act.Relu,
                                 bias=bl2_sb[:])
            ps3 = ps_conv.tile([96, g.G], dt.float32, tag="psc", name="psc")
            nc.tensor.matmul(out=ps3[:O, :], lhsT=Wl3_sb[:], rhs=o2[:],
                             start=True, stop=True)
            o3 = pp.tile([O, g.G], dt.float32, tag="o3", name="o3")
            nc.scalar.activation(out=o3[:], in_=ps3[:O, :], func=Act.Identity,
                                 bias=bl3_sb[:])
            nc.sync.dma_start(out=out_d.ap(), in_=o3[:])

        stk.close()

    nc.compile()
    return nc


_CACHE = {}


def _get_program(geo, meta, n_cores):
    key = (repr(sorted(geo.__dict__.items(), key=str)),
           repr(meta["chunk_plan"]), n_cores)
    if key not in _CACHE:
        _CACHE[key] = build_program(geo, meta, n_cores)
    return _CACHE[key]


def kernel(**inputs):
    from concourse.bass_utils import run_bass_kernel_spmd

    geo = Geo(CFG)
    inputs = {k: np.asarray(v) for k, v in inputs.items()}
    per_core, meta = prep(geo, **inputs)
    nc = _get_program(geo, meta, geo.C)
    res = run_bass_kernel_spmd(nc, per_core, core_ids=list(range(geo.C)))
    out = np.asarray(res.results[0]["out"], f32)   # [OUT, G]
    return np.ascontiguousarray(out.T)             # [G, OUT] float32



# revision 25
# speedup vs baseline: 1.2103x; 1.1827x over previous
"""Trainium2 Bass kernel for nn_BenchGNN_29300266893894 (3-layer GCN with
global-feature concat + global mean/max pooling readout + MLP head).

Self-contained: host-side sharding/packing prep + SPMD Bass/Tile program on
8 NeuronCores via run_bass_kernel_spmd.

Sharding: nodes are split into 8 contiguous shards of 6250 (padded to 6272 =
49*128). Edges are partitioned by dst owner and sorted by dst window; weight
matrices are replicated. Per layer, each core computes h = x_cat @ W for its
nodes, builds a bf16 "gather table" of h' = h*dinv rows (padded to 256 B),
AllGathers it, then aggregates its incident edges with hardware dma_gather +
one-hot matmuls accumulating per-128-dst-window PSUM tiles. Graph pooling is
done with DVE segmented scans + ap_gather boundary extraction and AllReduce.
"""
import sys
import numpy as np
import ml_dtypes

sys.path.insert(0, "/opt/trn_rl_repo")

bf16 = ml_dtypes.bfloat16
f32 = np.float32

# ---------------- problem geometry (hardcoded) ----------------
CFG = dict(N=50000, E=800000, G=512, F_IN=128, HID=96, OUT=10, C=8)

WIN = 128
HALF = 32768
CHUNK_WINDOWS = 4
CONV_CHUNK = 512
PAD_SLOT = 999.0
PAD_VAL = -60000.0


class Geo:
    def __init__(self, cfg):
        self.__dict__.update(cfg)
        assert self.N % self.C == 0
        self.NL = self.N // self.C
        self.NWIN = -(-self.NL // WIN)
        self.NLP = self.NWIN * WIN
        # pooling pad-column tricks need at least one pad column per core
        assert self.NLP > self.NL
        self.NP = self.NLP * self.C
        self.conv_chunks = [(o, min(CONV_CHUNK, self.NLP - o))
                            for o in range(0, self.NLP, CONV_CHUNK)]


def _wrap16(idx, channels):
    idx = np.asarray(idx, np.int16)
    assert len(idx) % 16 == 0
    idx16 = idx.reshape(-1, 16).T
    return np.ascontiguousarray(np.tile(idx16, (channels // 16, 1)))


def prep(geo, x, edge_index, batch, W1, b1, W2, b2, W3, b3,
         Wl1, bl1, Wl2, bl2, Wl3, bl3):
    g = geo
    x = np.asarray(x, f32)
    src = np.asarray(edge_index[0], np.int64)
    dst = np.asarray(edge_index[1], np.int64)
    batch = np.asarray(batch, np.int64)

    def gid_of(n):
        return (n // g.NL) * g.NLP + (n % g.NL)

    deg = np.bincount(dst, minlength=g.N).astype(f32) + 1.0
    dinv = (1.0 / np.sqrt(deg)).astype(f32)
    counts = np.bincount(batch, minlength=g.G).astype(f32)
    invc = (1.0 / np.maximum(counts, 1.0)).astype(f32)

    sums1 = np.zeros((g.G, g.F_IN), f32)
    np.add.at(sums1, batch, x)
    mean1 = (sums1 * invc[:, None]).astype(f32)          # [G, F_IN]
    U1rows = (mean1 @ np.asarray(W1[g.F_IN:], f32))      # [G, HID]

    src_g = gid_of(src)

    core_win = []
    for k in range(g.C):
        lo_n, hi_n = k * g.NL, (k + 1) * g.NL
        sel = (dst >= lo_n) & (dst < hi_n)
        es, ed = src_g[sel], (dst[sel] - lo_n)
        order = np.argsort(ed, kind="stable")
        es, ed = es[order], ed[order]
        wstart = np.searchsorted(ed, np.arange(0, g.NLP + 1, WIN))
        wins = []
        for w in range(g.NWIN):
            a, b = wstart[w], wstart[w + 1]
            ws, wd = es[a:b], ed[a:b] - w * WIN
            lo_m = ws < HALF
            wins.append((ws[lo_m], wd[lo_m], ws[~lo_m] - HALF, wd[~lo_m]))
        core_win.append(wins)

    # chunk plan: per chunk of CHUNK_WINDOWS windows, edges of all windows are
    # packed contiguously (chunk-local slot ids); tile counts are chunk-level
    # cross-core maxima, and each window gets a [t0, t1] tile range (union
    # over cores) whose tiles feed its psum via masked one-hot matmuls.
    def _ranges(lens_kw, nw):
        out = []
        for wi in range(nw):
            t0s, t1s = [], []
            for k in range(g.C):
                s = sum(lens_kw[k][:wi])
                L = lens_kw[k][wi]
                if L:
                    t0s.append(s // 128)
                    t1s.append((s + L - 1) // 128)
            out.append((min(t0s), max(t1s)) if t0s else None)
        return out

    chunk_plan = []   # (w0, nw, TC_lo, TC_hi, rng_lo, rng_hi)
    for w0 in range(0, g.NWIN, CHUNK_WINDOWS):
        nw = min(CHUNK_WINDOWS, g.NWIN - w0)
        lens_lo = [[len(core_win[k][w0 + wi][0]) for wi in range(nw)]
                   for k in range(g.C)]
        lens_hi = [[len(core_win[k][w0 + wi][2]) for wi in range(nw)]
                   for k in range(g.C)]
        TC_lo = -(-max(sum(r) for r in lens_lo) // 128)
        TC_hi = -(-max(sum(r) for r in lens_hi) // 128)
        chunk_plan.append((w0, nw, TC_lo, TC_hi,
                           _ranges(lens_lo, nw), _ranges(lens_hi, nw)))

    per_core = []
    for k in range(g.C):
        lo_n, hi_n = k * g.NL, (k + 1) * g.NL
        idx_list, slot_list = [], []
        for (w0, nw, tls, ths) in chunk_plan:
            for half in (0, 2):   # lo block then hi block
                Ts = tls if half == 0 else ths
                for wi in range(nw):
                    s_arr, d_arr = (core_win[k][w0 + wi][half],
                                    core_win[k][w0 + wi][half + 1])
                    T = Ts[wi]
                    pad = T * 128 - len(s_arr)
                    idx_list.append(np.concatenate(
                        [s_arr, np.zeros(pad, np.int64)]))
                    slot_list.append(np.concatenate(
                        [d_arr, np.full(pad, PAD_SLOT)]))
        idx_all = np.concatenate(idx_list)
        slot_all = np.concatenate(slot_list).astype(f32)
        idx_sb = _wrap16(idx_all, 128)
        dst_sb = np.ascontiguousarray(
            slot_all.reshape(-1, 128).T.astype(bf16))

        bl = batch[lo_n:hi_n]
        # one-hot graph-block matrices for the conv global-feature term:
        # ohconv[p, c, b, j] = 1{batch_local[cs_c + j] == 128*b + p}
        nch = len(g.conv_chunks)
        ohconv = np.zeros((128, nch, 4, CONV_CHUNK), bf16)
        for c, (cs, cw) in enumerate(g.conv_chunks):
            seg = bl[cs:min(cs + cw, g.NL)]
            for j, gid in enumerate(seg):
                ohconv[gid % 128, c, gid // 128, j] = 1
        bcol = np.searchsorted(bl, np.arange(g.G), side="right") - 1
        gap_last = _wrap16(np.where(bcol < 0, 0, bcol + 1), 96)
        present = np.zeros(g.G, bool)
        present[np.unique(bl)] = True
        maxcol = _wrap16(np.where(present, bcol, g.NLP - 1), 96)

        gstart = np.searchsorted(bl, np.arange(g.G), side="left")
        maskneg = np.zeros(g.NLP, f32)
        maskneg[np.unique(gstart[present])] = -1e30
        maskneg[g.NL] = -1e30
        maskneg_row = maskneg.reshape(1, g.NLP).astype(bf16)

        bidx = np.zeros(g.NLP, np.int64)
        bidx[:g.NL] = bl
        batch_idx = _wrap16(bidx, 128)

        xT = np.zeros((g.F_IN, g.NLP), f32)
        xT[:, :g.NL] = x[lo_n:hi_n].T
        dinv_loc = np.zeros(g.NLP, f32)
        dinv_loc[:g.NL] = dinv[lo_n:hi_n]
        dinv_sb = np.ascontiguousarray(dinv_loc.reshape(g.NWIN, WIN).T)

        H, H2, O = g.HID, g.HID // 2, g.OUT
        inp = {
            "xT": xT.astype(bf16),
            "U1rows": np.ascontiguousarray(
                U1rows.reshape(4, 128, H).transpose(1, 0, 2)).astype(bf16),
            "ohconv": np.ascontiguousarray(
                ohconv.reshape(128, 4 * nch, CONV_CHUNK)),
            "gap_last": gap_last, "maxcol": maxcol,
            "maskneg_row": maskneg_row,
            "eidx": idx_sb,
            "dstslot": dst_sb,
            "dinv_sb": dinv_sb,
            "iota128": np.tile(np.arange(WIN, dtype=f32), (128, 1)).astype(bf16),
            "id96": np.eye(96, dtype=f32),
            "id128b": np.eye(128, dtype=bf16),
            "invc_row": invc.reshape(1, g.G),
            "W1a": np.asarray(W1[:g.F_IN], bf16),
            "W1b": np.asarray(W1[g.F_IN:], bf16),
            "W2a": np.asarray(W2[:H], bf16), "W2b": np.asarray(W2[H:], bf16),
            "W3a": np.asarray(W3[:H], bf16), "W3b": np.asarray(W3[H:], bf16),
            "b1_row": np.asarray(b1, f32).reshape(1, H),
            "b2_row": np.asarray(b2, f32).reshape(1, H),
            "b3_row": np.asarray(b3, f32).reshape(1, H),
            "Wl1a": np.asarray(Wl1[:H], f32), "Wl1b": np.asarray(Wl1[H:], f32),
            "Wl2": np.asarray(Wl2, f32), "Wl3": np.asarray(Wl3, f32),
            "bl1": np.asarray(bl1, f32).reshape(H, 1),
            "bl2": np.asarray(bl2, f32).reshape(H2, 1),
            "bl3": np.asarray(bl3, f32).reshape(O, 1),
        }
        per_core.append(inp)

    meta = {"chunk_plan": chunk_plan,
            "Ttot": sum(T_lo) + sum(T_hi)}
    return per_core, meta


# ---------------- device program ----------------


def build_program(geo, meta, n_cores):
    import os
    dbg_layers = int(os.environ.get("KN_LAYERS", "3"))
    dbg_stage = os.environ.get("KN_STAGE", "full")
    dbg_agg = os.environ.get("KN_AGG", "ep")
    import concourse.bacc as bacc
    import concourse.bass as bass
    import concourse.mybir as mybir
    import concourse.tile as tile

    g = geo
    H, H2, O = g.HID, g.HID // 2, g.OUT
    dt = mybir.dt
    Alu = mybir.AluOpType
    Act = mybir.ActivationFunctionType
    chunk_plan = meta["chunk_plan"]
    Ttot = meta["Ttot"]
    Tmax = max(sum(tls) + sum(ths) for (_, _, tls, ths) in chunk_plan)
    Tblk = max(max(max(tls), max(ths)) for (_, _, tls, ths) in chunk_plan)

    nc = bacc.Bacc("TRN2", target_bir_lowering=False, debug=False,
                   num_devices=n_cores)
    rg = [list(range(n_cores))]

    def din(name, shape, dtype):
        return nc.dram_tensor(name, list(shape), dtype, kind="ExternalInput")

    xT_d = din("xT", [g.F_IN, g.NLP], dt.bfloat16)
    U1rows_d = din("U1rows", [128, 4, H], dt.bfloat16)
    ohconv_d = din("ohconv", [128, 4 * len(g.conv_chunks), CONV_CHUNK],
                   dt.bfloat16)
    gap_last_d = din("gap_last", [96, g.G // 16], dt.int16)
    maxcol_d = din("maxcol", [96, g.G // 16], dt.int16)
    maskneg_d = din("maskneg_row", [1, g.NLP], dt.bfloat16)
    eidx_d = din("eidx", [128, Ttot * 8], dt.int16)
    dstslot_d = din("dstslot", [128, Ttot], dt.bfloat16)
    dinv_d = din("dinv_sb", [128, g.NWIN], dt.float32)
    iota_d = din("iota128", [128, 128], dt.bfloat16)
    id96_d = din("id96", [96, 96], dt.float32)
    id128b_d = din("id128b", [128, 128], dt.bfloat16)
    invc_d = din("invc_row", [1, g.G], dt.float32)
    W_d = {n: din(n, [g.F_IN if n[1] == "1" else H, H], dt.bfloat16)
           for n in ("W1a", "W1b", "W2a", "W2b", "W3a", "W3b")}
    b_d = {n: din(n, [1, H], dt.float32)
           for n in ("b1_row", "b2_row", "b3_row")}
    Wl1a_d = din("Wl1a", [H, H], dt.float32)
    Wl1b_d = din("Wl1b", [H, H], dt.float32)
    Wl2_d = din("Wl2", [H, H2], dt.float32)
    Wl3_d = din("Wl3", [H2, O], dt.float32)
    bl1_d = din("bl1", [H, 1], dt.float32)
    bl2_d = din("bl2", [H2, 1], dt.float32)
    bl3_d = din("bl3", [O, 1], dt.float32)

    out_d = nc.dram_tensor("out", [O, g.G], dt.float32, kind="ExternalOutput")

    # internal DRAM
    tshard = nc.dram_tensor("tshard", [g.NLP, 128], dt.bfloat16,
                            kind="Internal")
    table = nc.dram_tensor("table", [g.NP, 128], dt.bfloat16,
                           kind="Internal", addr_space="Shared")
    gap_in = [nc.dram_tensor(f"gap_in{i}", [96, g.G], dt.float32,
                             kind="Internal") for i in range(3)]
    gap_out = [nc.dram_tensor(f"gap_out{i}", [96, g.G], dt.float32,
                              kind="Internal", addr_space="Shared")
               for i in range(3)]
    gmp_in = [nc.dram_tensor(f"gmp_in{i}", [96, g.G], dt.float32,
                             kind="Internal") for i in range(3)]
    gmp_out = [nc.dram_tensor(f"gmp_out{i}", [96, g.G], dt.float32,
                              kind="Internal", addr_space="Shared")
               for i in range(3)]

    with tile.TileContext(nc) as tc:
        import contextlib
        stk = contextlib.ExitStack()
        pp = stk.enter_context(tc.tile_pool(name="persist", bufs=1))
        wk = stk.enter_context(tc.tile_pool(name="work", bufs=2))
        ep = stk.enter_context(tc.tile_pool(name="epil", bufs=2))
        ps_conv = stk.enter_context(
            tc.tile_pool(name="ps_conv", bufs=2, space="PSUM"))
        ps_tr = stk.enter_context(
            tc.tile_pool(name="ps_tr", bufs=2, space="PSUM"))
        ps_agg = stk.enter_context(
            tc.tile_pool(name="ps_agg", bufs=4, space="PSUM"))

        def load(pool, dram, shape, dtype, tag, bcast=None):
            t = pool.tile(shape, dtype, tag=tag, name=tag)
            src = dram.ap() if bcast is None else dram.ap().to_broadcast(bcast)
            nc.sync.dma_start(out=t[:], in_=src)
            return t

        # constants
        iota_sb = load(pp, iota_d, [128, 128], dt.bfloat16, "iota")
        id96_sb = load(pp, id96_d, [96, 96], dt.float32, "id96")
        id128b_sb = load(pp, id128b_d, [128, 128], dt.bfloat16, "id128b")
        dinv_sbT = load(pp, dinv_d, [128, g.NWIN], dt.float32, "dinv")
        invc_bc = load(pp, invc_d, [96, g.G], dt.float32, "invc",
                       bcast=(96, g.G))
        maskneg_sb = load(pp, maskneg_d, [96, g.NLP], dt.bfloat16, "maskn",
                          bcast=(96, g.NLP))
        eidx_sb = load(pp, eidx_d, [128, Ttot * 8], dt.int16, "eidx")
        dst_sb = load(pp, dstslot_d, [128, Ttot], dt.bfloat16, "dstslot")
        gap_last_sb = load(pp, gap_last_d, [96, g.G // 16], dt.int16, "glast")
        maxcol_sb = load(pp, maxcol_d, [96, g.G // 16], dt.int16, "maxcol")
        Urows = load(pp, U1rows_d, [128, 4, H], dt.bfloat16, "Urows")
        tabkeep = pp.tile([128, g.NWIN, 96], dt.bfloat16, tag="tabkeep",
                          name="tabkeep")
        W_sb = {n: load(pp, W_d[n], list(W_d[n].shape), dt.bfloat16, n)
                for n in W_d}
        b_bc = {n: load(pp, b_d[n], [128, H], dt.float32, n, bcast=(128, H))
                for n in b_d}
        Wl1a_sb = load(pp, Wl1a_d, [H, H], dt.float32, "Wl1a")
        Wl1b_sb = load(pp, Wl1b_d, [H, H], dt.float32, "Wl1b")
        Wl2_sb = load(pp, Wl2_d, [H, H2], dt.float32, "Wl2")
        Wl3_sb = load(pp, Wl3_d, [H2, O], dt.float32, "Wl3")
        bl1_sb = load(pp, bl1_d, [H, 1], dt.float32, "bl1")
        bl2_sb = load(pp, bl2_d, [H2, 1], dt.float32, "bl2")
        bl3_sb = load(pp, bl3_d, [O, 1], dt.float32, "bl3")

        xbuf = [pp.tile([128, g.NLP], dt.bfloat16, tag=f"xbuf{i}", name=f"xbuf{i}")
                for i in range(2)]
        nc.sync.dma_start(out=xbuf[0][:g.F_IN, :], in_=xT_d.ap())

        scano = pp.tile([96, g.NLP], dt.float32, tag="scano", name="scano")
        gmp_all = pp.tile([96, 3 * g.G], dt.float32, tag="gmp_all", name="gmp_all")
        mg_sb = [pp.tile([96, g.G], dt.float32, tag=f"mg{i}", name=f"mg{i}")
                 for i in range(3)]
        gapar_sb = [pp.tile([96, g.G], dt.float32, tag=f"gapar{i}", name=f"gapar{i}")
                    for i in range(3)]

        zinit = wk.tile([128, Tmax, 128], dt.bfloat16, tag="gath", name="zinit")
        nc.gpsimd.memset(zinit[:, :g.NWIN, :], 0.0)
        nc.sync.dma_start(
            out=tshard.ap().rearrange("(w p) c -> p w c", p=128),
            in_=zinit[:, :g.NWIN, :])

        layer_W = [("W1a", "W1b", "b1_row"), ("W2a", "W2b", "b2_row"),
                   ("W3a", "W3b", "b3_row")]

        for l in range(dbg_layers):
            F = g.F_IN if l == 0 else H
            Wa = W_sb[layer_W[l][0]]
            Wb = W_sb[layer_W[l][1]]
            bias = b_bc[layer_W[l][2]]
            x_src, x_dst = xbuf[l % 2], xbuf[(l + 1) % 2]

            # ---- conv + gather-table build ----
            for ci, (cs, cw) in enumerate(g.conv_chunks):
                ohc = wk.tile([128, 4, CONV_CHUNK], dt.bfloat16, tag="ohc",
                              name="ohc")
                nc.sync.dma_start(out=ohc[:],
                                  in_=ohconv_d.ap()[:, 4 * ci:4 * ci + 4, :])
                psc = ps_conv.tile([96, CONV_CHUNK], dt.float32, tag="psc", name="psc")
                nc.tensor.matmul(out=psc[:, :cw], lhsT=Wa[:],
                                 rhs=x_src[:F, cs:cs + cw],
                                 start=True, stop=False)
                for b in range(4):
                    nc.tensor.matmul(out=psc[:, :cw], lhsT=Urows[:, b, :],
                                     rhs=ohc[:, b, :cw], start=False,
                                     stop=b == 3)
                hT = wk.tile([96, CONV_CHUNK], dt.float32, tag="hT", name="hT")
                nc.vector.tensor_copy(out=hT[:, :cw], in_=psc[:, :cw])
                for wo in range(0, cw, 128):
                    w = (cs + wo) // 128
                    pt = ps_tr.tile([128, 128], dt.float32, tag="ptr", name="ptr")
                    nc.tensor.transpose(out=pt[:, :96],
                                        in_=hT[:, wo:wo + 128],
                                        identity=id96_sb[:])
                    nc.scalar.activation(out=tabkeep[:, w, :], in_=pt[:, :96],
                                         func=Act.Copy,
                                         scale=dinv_sbT[:, w:w + 1])
                    nc.sync.dma_start(
                        out=tshard.ap()[w * 128:(w + 1) * 128, :96],
                        in_=tabkeep[:, w, :])

            if dbg_stage == "conv":
                break
            # ---- all-gather the table ----
            nc.gpsimd.collective_compute(
                "AllGather", Alu.bypass, replica_groups=rg,
                ins=[tshard.ap()], outs=[table.ap()])

            if dbg_stage == "ag":
                break
            # ---- edge aggregation ----
            t_base = 0
            for (w0, nw, tls, ths) in chunk_plan:
                Tlo, Thi = sum(tls), sum(ths)
                T = Tlo + Thi
                gath = wk.tile([128, Tmax, 128], dt.bfloat16, tag="gath", name="gath")
                # dma_gather is limited to 1024 indices (8 tiles) per call
                for (goff, tstart, cnt, hi) in (
                        (0, t_base, Tlo, False),
                        (Tlo, t_base + Tlo, Thi, True)):
                    if cnt == 0:
                        continue
                    tbl_ap = table.ap()[HALF:, :] if hi else table.ap()
                    done = 0
                    while done < cnt:
                        nt = min(8, cnt - done)
                        nc.gpsimd.dma_gather(
                            gath[:, goff + done:goff + done + nt, :],
                            tbl_ap,
                            eidx_sb[:, 8 * (tstart + done):
                                    8 * (tstart + done + nt)],
                            num_idxs=nt * 128, num_idxs_reg=nt * 128,
                            elem_size=128)
                        done += nt
                if dbg_agg == "gather":
                    t_base += T
                    continue
                # blocks of tiles: (window-in-chunk, tile offset, count)
                blocks = [(wi, sum(tls[:wi]), tls[wi]) for wi in range(nw)
                          if tls[wi] > 0] \
                    + [(wi, Tlo + sum(ths[:wi]), ths[wi]) for wi in range(nw)
                       if ths[wi] > 0]
                pags = [ps_agg.tile([96, 128], dt.float32, tag="pag", name="pag")
                        for _ in range(nw)]
                remaining = [tls[wi] + ths[wi] for wi in range(nw)]
                started = [True] * nw
                for wi in range(nw):
                    # self-loop term: psum += table rows of this window
                    nc.tensor.matmul(out=pags[wi][:],
                                     lhsT=tabkeep[:, w0 + wi, :],
                                     rhs=id128b_sb[:],
                                     start=True, stop=remaining[wi] == 0)
                for (wi, toff, Tb) in blocks:
                    oh = wk.tile([128, Tblk, 128], dt.bfloat16, tag="oh",
                                 name="oh", bufs=4)
                    a = t_base + toff
                    nc.vector.tensor_tensor(
                        out=oh[:, 0:Tb, :],
                        in0=dst_sb[:, a:a + Tb, None]
                            .to_broadcast((128, Tb, 128)),
                        in1=iota_sb[:, None, :].to_broadcast((128, Tb, 128)),
                        op=Alu.is_equal)
                    for t in range(Tb):
                        if dbg_agg == "onehot":
                            continue
                        remaining[wi] -= 1
                        nc.tensor.matmul(out=pags[wi][:],
                                         lhsT=gath[:, toff + t, :96],
                                         rhs=oh[:, t, :],
                                         start=not started[wi],
                                         stop=remaining[wi] == 0)
                        started[wi] = True

                for wi in range(nw):
                    if dbg_agg in ("onehot", "mm"):
                        break
                    w = w0 + wi
                    sb1 = ep.tile([96, 128], dt.float32, tag="ep1", name="ep1")
                    nc.vector.tensor_copy(out=sb1[:], in_=pags[wi][:])
                    pt2 = ps_tr.tile([128, 128], dt.float32, tag="ptr", name="ptr")
                    nc.tensor.transpose(out=pt2[:, :96], in_=sb1[:],
                                        identity=id96_sb[:])
                    sb2 = ep.tile([128, 96], dt.float32, tag="ep2", name="ep2")
                    nc.scalar.activation(out=sb2[:], in_=pt2[:, :96],
                                         func=Act.Copy,
                                         scale=dinv_sbT[:, w:w + 1])
                    sb3 = ep.tile([128, 96], dt.bfloat16, tag="ep3", name="ep3")
                    if l < 2:
                        tmp = ep.tile([128, 96], dt.float32, tag="ep3f", name="ep3f")
                        nc.vector.tensor_add(out=tmp[:], in0=sb2[:],
                                             in1=bias[:, :96])
                        nc.vector.tensor_scalar_max(out=sb3[:], in0=tmp[:],
                                                    scalar1=0.0)
                    else:
                        nc.vector.tensor_add(out=sb3[:], in0=sb2[:],
                                             in1=bias[:, :96])
                    pt3 = ps_tr.tile([128, 128], dt.bfloat16, tag="ptr", name="ptr")
                    nc.tensor.transpose(out=pt3[:96, :], in_=sb3[:],
                                        identity=id128b_sb[:])
                    nc.vector.tensor_copy(
                        out=x_dst[:96, w * 128:(w + 1) * 128],
                        in_=pt3[:96, :])
                t_base += T

            nc.gpsimd.memset(x_dst[:96, g.NL:g.NLP], PAD_VAL)

            if dbg_stage == "agg":
                break
            # ---- pooling ----
            nc.vector.memset(scano[:, 0:1], 0.0)
            nc.vector.tensor_tensor_scan(
                out=scano[:, 1:g.NL + 1], data0=x_dst[:96, 0:g.NL],
                data1=x_dst[:96, 0:g.NL],
                initial=0.0, op0=Alu.add, op1=Alu.bypass)
            cum_l = ep.tile([96, g.G + 1], dt.float32, tag="cuml", name="cuml",
                            bufs=1)
            nc.vector.memset(cum_l[:, 0:1], 0.0)
            nc.gpsimd.ap_gather(cum_l[:, 1:g.G + 1], scano[:], gap_last_sb[:],
                                channels=96, num_elems=g.NLP, d=1,
                                num_idxs=g.G)
            gaps = ep.tile([96, g.G], dt.float32, tag="gaps", name="gaps", bufs=1)
            nc.vector.tensor_tensor(out=gaps[:], in0=cum_l[:, 1:g.G + 1],
                                    in1=cum_l[:, 0:g.G],
                                    op=Alu.subtract)
            nc.sync.dma_start(out=gap_in[l].ap(), in_=gaps[:])
            nc.gpsimd.collective_compute(
                "AllReduce", Alu.add, replica_groups=rg,
                ins=[gap_in[l].ap()], outs=[gap_out[l].ap()])
            nc.sync.dma_start(out=gapar_sb[l][:], in_=gap_out[l].ap())

            nc.vector.tensor_tensor_scan(
                out=scano[:], data0=maskneg_sb[:], data1=x_dst[:96, :],
                initial=0.0, op0=Alu.add, op1=Alu.max)
            nc.gpsimd.ap_gather(gmp_all[:, l * g.G:(l + 1) * g.G],
                                scano[:], maxcol_sb[:],
                                channels=96, num_elems=g.NLP, d=1,
                                num_idxs=g.G)
            # per-layer gmp AllReduce: layers 0-1 overlap the next layer's
            # edge gathers; only layer 2's is exposed
            nc.sync.dma_start(out=gmp_in[l].ap(),
                              in_=gmp_all[:, l * g.G:(l + 1) * g.G])
            nc.gpsimd.collective_compute(
                "AllReduce", Alu.max, replica_groups=rg,
                ins=[gmp_in[l].ap()], outs=[gmp_out[l].ap()])

            nc.vector.tensor_mul(out=mg_sb[l][:], in0=gapar_sb[l][:],
                                 in1=invc_bc[:])

            if l < 2:
                # build Urows for next layer: U = Wb_{l+1}^T @ mg,
                # transposed into [128 graph, 4 block, 96] for the conv
                Wbn = W_sb[layer_W[l + 1][1]]
                mgb = ep.tile([96, g.G], dt.bfloat16, tag="mgb", name="mgb",
                              bufs=1)
                nc.scalar.copy(out=mgb[:], in_=mg_sb[l][:])
                psU = ps_conv.tile([96, g.G], dt.float32, tag="psc", name="psc")
                nc.tensor.matmul(out=psU[:], lhsT=Wbn[:], rhs=mgb[:],
                                 start=True, stop=True)
                Usb = ep.tile([96, g.G], dt.bfloat16, tag="Usb", name="Usb",
                              bufs=1)
                nc.vector.tensor_copy(out=Usb[:], in_=psU[:])
                for b in range(4):
                    ptU = ps_tr.tile([128, 128], dt.bfloat16, tag="ptr",
                                     name="ptr")
                    nc.tensor.transpose(out=ptU[:, :96],
                                        in_=Usb[:, b * 128:(b + 1) * 128],
                                        identity=id128b_sb[:96, :96])
                    nc.scalar.copy(out=Urows[:, b, :], in_=ptU[:, :96])

        debug_cut = dbg_stage != "full" or dbg_layers < 3
        if debug_cut:
            nc.gpsimd.dma_start(out=out_d.ap(), in_=xbuf[0][:O, :g.G])
        # ---- final readout MLP (f32) ----
        if not debug_cut:
            gmpar = pp.tile([96, 3 * g.G], dt.float32, tag="gmpar", name="gmpar")
            for i in range(3):
                nc.sync.dma_start(out=gmpar[:, i * g.G:(i + 1) * g.G],
                                  in_=gmp_out[i].ap())

            hTa = pp.tile([96, g.G], dt.float32, tag="hTa", name="hTa")
            hTb = pp.tile([96, g.G], dt.float32, tag="hTb", name="hTb")
            nc.vector.tensor_add(out=hTa[:], in0=gmpar[:, 0:g.G],
                                 in1=gmpar[:, g.G:2 * g.G])
            nc.vector.tensor_add(out=hTa[:], in0=hTa[:],
                                 in1=gmpar[:, 2 * g.G:3 * g.G])
            nc.vector.tensor_add(out=hTb[:], in0=mg_sb[0][:], in1=mg_sb[1][:])
            nc.vector.tensor_add(out=hTb[:], in0=hTb[:], in1=mg_sb[2][:])

            ps1 = ps_conv.tile([96, g.G], dt.float32, tag="psc", name="psc")
            nc.tensor.matmul(out=ps1[:], lhsT=Wl1a_sb[:], rhs=hTa[:],
                             start=True, stop=False)
            nc.tensor.matmul(out=ps1[:], lhsT=Wl1b_sb[:], rhs=hTb[:],
                             start=False, stop=True)
            o1 = pp.tile([96, g.G], dt.float32, tag="o1", name="o1")
            nc.scalar.activation(out=o1[:], in_=ps1[:], func=Act.Relu,
                                 bias=bl1_sb[:])
            ps2 = ps_conv.tile([96, g.G], dt.float32, tag="psc", name="psc")
            nc.tensor.matmul(out=ps2[:H2, :], lhsT=Wl2_sb[:], rhs=o1[:],
                             start=True, stop=True)
            o2 = pp.tile([H2, g.G], dt.float32, tag="o2", name="o2")
            nc.scalar.activation(out=o2[:], in_=ps2[:H2, :], func=Act.Relu,
                                 bias=bl2_sb[:])
            ps3 = ps_conv.tile([96, g.G], dt.float32, tag="psc", name="psc")
            nc.tensor.matmul(out=ps3[:O, :], lhsT=Wl3_sb[:], rhs=o2[:],
                             start=True, stop=True)
            o3 = pp.tile([O, g.G], dt.float32, tag="o3", name="o3")
            nc.scalar.activation(out=o3[:], in_=ps3[:O, :], func=Act.Identity,
                                 bias=bl3_sb[:])
            nc.sync.dma_start(out=out_d.ap(), in_=o3[:])

        stk.close()

    nc.compile()
    return nc


_CACHE = {}


def _get_program(geo, meta, n_cores):
    key = (repr(sorted(geo.__dict__.items(), key=str)),
           repr(meta["chunk_plan"]), n_cores)
    if key not in _CACHE:
        _CACHE[key] = build_program(geo, meta, n_cores)
    return _CACHE[key]


def kernel(**inputs):
    from concourse.bass_utils import run_bass_kernel_spmd

    geo = Geo(CFG)
    inputs = {k: np.asarray(v) for k, v in inputs.items()}
    per_core, meta = prep(geo, **inputs)
    nc = _get_program(geo, meta, geo.C)
    res = run_bass_kernel_spmd(nc, per_core, core_ids=list(range(geo.C)))
    out = np.asarray(res.results[0]["out"], f32)   # [OUT, G]
    return np.ascontiguousarray(out.T)             # [G, OUT] float32



# revision 35
# speedup vs baseline: 1.2534x; 1.0356x over previous
"""Trainium2 Bass kernel for nn_BenchGNN_29300266893894 (3-layer GCN with
global-feature concat + global mean/max pooling readout + MLP head).

Self-contained: host-side sharding/packing prep + SPMD Bass/Tile program on
8 NeuronCores via run_bass_kernel_spmd.

Sharding: nodes are split into 8 contiguous shards of 6250 (padded to 6272 =
49*128). Edges are partitioned by dst owner and sorted by dst window; weight
matrices are replicated. Per layer, each core computes h = x_cat @ W for its
nodes, builds a bf16 "gather table" of h' = h*dinv rows (padded to 256 B),
AllGathers it, then aggregates its incident edges with hardware dma_gather +
one-hot matmuls accumulating per-128-dst-window PSUM tiles. Graph pooling is
done with DVE segmented scans + ap_gather boundary extraction and AllReduce.
"""
import sys
import numpy as np
import ml_dtypes

sys.path.insert(0, "/opt/trn_rl_repo")

bf16 = ml_dtypes.bfloat16
f32 = np.float32

# ---------------- problem geometry (hardcoded) ----------------
CFG = dict(N=50000, E=800000, G=512, F_IN=128, HID=96, OUT=10, C=8)

WIN = 128
HALF = 32768
CHUNK_WINDOWS = 4
CONV_CHUNK = 512
PAD_SLOT = 999.0
PAD_VAL = -60000.0


class Geo:
    def __init__(self, cfg):
        self.__dict__.update(cfg)
        assert self.N % self.C == 0
        self.NL = self.N // self.C
        self.NWIN = -(-self.NL // WIN)
        self.NLP = self.NWIN * WIN
        # pooling pad-column tricks need at least one pad column per core
        assert self.NLP > self.NL
        self.NP = self.NLP * self.C
        self.conv_chunks = [(o, min(CONV_CHUNK, self.NLP - o))
                            for o in range(0, self.NLP, CONV_CHUNK)]


def _wrap16(idx, channels):
    idx = np.asarray(idx, np.int16)
    assert len(idx) % 16 == 0
    idx16 = idx.reshape(-1, 16).T
    return np.ascontiguousarray(np.tile(idx16, (channels // 16, 1)))


def prep(geo, x, edge_index, batch, W1, b1, W2, b2, W3, b3,
         Wl1, bl1, Wl2, bl2, Wl3, bl3):
    g = geo
    x = np.asarray(x, f32)
    src = np.asarray(edge_index[0], np.int64)
    dst = np.asarray(edge_index[1], np.int64)
    batch = np.asarray(batch, np.int64)

    def gid_of(n):
        return (n // g.NL) * g.NLP + (n % g.NL)

    deg = np.bincount(dst, minlength=g.N).astype(f32) + 1.0
    dinv = (1.0 / np.sqrt(deg)).astype(f32)
    counts = np.bincount(batch, minlength=g.G).astype(f32)
    invc = (1.0 / np.maximum(counts, 1.0)).astype(f32)

    sums1 = np.zeros((g.G, g.F_IN), f32)
    np.add.at(sums1, batch, x)
    mean1 = (sums1 * invc[:, None]).astype(f32)          # [G, F_IN]
    U1rows = (mean1 @ np.asarray(W1[g.F_IN:], f32))      # [G, HID]

    src_g = gid_of(src)

    core_win = []
    for k in range(g.C):
        lo_n, hi_n = k * g.NL, (k + 1) * g.NL
        sel = (dst >= lo_n) & (dst < hi_n)
        es, ed = src_g[sel], (dst[sel] - lo_n)
        order = np.argsort(ed, kind="stable")
        es, ed = es[order], ed[order]
        wstart = np.searchsorted(ed, np.arange(0, g.NLP + 1, WIN))
        wins = []
        for w in range(g.NWIN):
            a, b = wstart[w], wstart[w + 1]
            ws, wd = es[a:b], ed[a:b] - w * WIN
            lo_m = ws < HALF
            wins.append((ws[lo_m], wd[lo_m], ws[~lo_m] - HALF, wd[~lo_m]))
        core_win.append(wins)

    # chunk plan: per chunk of CHUNK_WINDOWS windows, edges of all windows are
    # packed contiguously (chunk-local slot ids); tile counts are chunk-level
    # cross-core maxima, and each window gets a [t0, t1] tile range (union
    # over cores) whose tiles feed its psum via masked one-hot matmuls.
    def _ranges(lens_kw, nw):
        out = []
        for wi in range(nw):
            t0s, t1s = [], []
            for k in range(g.C):
                s = sum(lens_kw[k][:wi])
                L = lens_kw[k][wi]
                if L:
                    t0s.append(s // 128)
                    t1s.append((s + L - 1) // 128)
            out.append((min(t0s), max(t1s)) if t0s else None)
        return out

    chunk_plan = []   # (w0, nw, TC_lo, TC_hi, rng_lo, rng_hi)
    for w0 in range(0, g.NWIN, CHUNK_WINDOWS):
        nw = min(CHUNK_WINDOWS, g.NWIN - w0)
        lens_lo = [[len(core_win[k][w0 + wi][0]) for wi in range(nw)]
                   for k in range(g.C)]
        lens_hi = [[len(core_win[k][w0 + wi][2]) for wi in range(nw)]
                   for k in range(g.C)]
        TC_lo = -(-max(sum(r) for r in lens_lo) // 128)
        TC_hi = -(-max(sum(r) for r in lens_hi) // 128)
        chunk_plan.append((w0, nw, TC_lo, TC_hi,
                           _ranges(lens_lo, nw), _ranges(lens_hi, nw)))

    per_core = []
    for k in range(g.C):
        lo_n, hi_n = k * g.NL, (k + 1) * g.NL
        idx_list, slot_list = [], []
        for (w0, nw, TCl, TCh, _, _) in chunk_plan:
            for half, TC in ((0, TCl), (2, TCh)):
                s_arr = np.concatenate(
                    [core_win[k][w0 + wi][half] for wi in range(nw)]
                    + [np.zeros(0, np.int64)])
                d_arr = np.concatenate(
                    [core_win[k][w0 + wi][half + 1] + 128 * wi
                     for wi in range(nw)] + [np.zeros(0, np.int64)])
                pad = TC * 128 - len(s_arr)
                idx_list.append(np.concatenate(
                    [s_arr, np.zeros(pad, np.int64)]))
                slot_list.append(np.concatenate(
                    [d_arr, np.full(pad, PAD_SLOT)]))
        idx_all = np.concatenate(idx_list)
        slot_all = np.concatenate(slot_list).astype(f32)
        idx_sb = _wrap16(idx_all, 128)
        dst_sb = np.ascontiguousarray(
            slot_all.reshape(-1, 128).T.astype(f32))

        bl = batch[lo_n:hi_n]
        # one-hot graph-block matrices for the conv global-feature term:
        # ohconv[p, c, b, j] = 1{batch_local[cs_c + j] == 128*b + p}
        nch = len(g.conv_chunks)
        ohconv = np.zeros((128, nch, 4, CONV_CHUNK), bf16)
        for c, (cs, cw) in enumerate(g.conv_chunks):
            seg = bl[cs:min(cs + cw, g.NL)]
            for j, gid in enumerate(seg):
                ohconv[gid % 128, c, gid // 128, j] = 1
        bcol = np.searchsorted(bl, np.arange(g.G), side="right") - 1
        gap_last = _wrap16(np.where(bcol < 0, 0, bcol + 1), 96)
        present = np.zeros(g.G, bool)
        present[np.unique(bl)] = True
        maxcol = _wrap16(np.where(present, bcol, g.NLP - 1), 96)

        gstart = np.searchsorted(bl, np.arange(g.G), side="left")
        maskneg = np.zeros(g.NLP, f32)
        maskneg[np.unique(gstart[present])] = -1e30
        maskneg[g.NL] = -1e30
        maskneg_row = maskneg.reshape(1, g.NLP).astype(bf16)

        xT = np.zeros((g.F_IN, g.NLP), f32)
        xT[:, :g.NL] = x[lo_n:hi_n].T
        dinv_loc = np.zeros(g.NLP, f32)
        dinv_loc[:g.NL] = dinv[lo_n:hi_n]
        dinv_sb = np.ascontiguousarray(dinv_loc.reshape(g.NWIN, WIN).T)

        H, H2, O = g.HID, g.HID // 2, g.OUT
        inp = {
            "xT": xT.astype(bf16),
            "U1rows": np.ascontiguousarray(
                U1rows.reshape(4, 128, H).transpose(1, 0, 2)).astype(bf16),
            "ohconv": np.ascontiguousarray(
                ohconv.reshape(128, 4 * nch, CONV_CHUNK)),
            "gap_last": gap_last, "maxcol": maxcol,
            "maskneg_row": maskneg_row,
            "eidx": idx_sb,
            "dstslot": dst_sb,
            "dinv_sb": dinv_sb,
            "iota512": np.tile(np.arange(CHUNK_WINDOWS * WIN, dtype=f32),
                               (128, 1)),
            "id96": np.eye(96, dtype=f32),
            "id128b": np.eye(128, dtype=bf16),
            "invc_row": invc.reshape(1, g.G),
            "W1a": np.asarray(W1[:g.F_IN], bf16),
            "W1b": np.asarray(W1[g.F_IN:], bf16),
            "W2a": np.asarray(W2[:H], bf16), "W2b": np.asarray(W2[H:], bf16),
            "W3a": np.asarray(W3[:H], bf16), "W3b": np.asarray(W3[H:], bf16),
            "b1_row": np.asarray(b1, f32).reshape(1, H),
            "b2_row": np.asarray(b2, f32).reshape(1, H),
            "b3_row": np.asarray(b3, f32).reshape(1, H),
            "Wl1a": np.asarray(Wl1[:H], f32), "Wl1b": np.asarray(Wl1[H:], f32),
            "Wl2": np.asarray(Wl2, f32), "Wl3": np.asarray(Wl3, f32),
            "bl1": np.asarray(bl1, f32).reshape(H, 1),
            "bl2": np.asarray(bl2, f32).reshape(H2, 1),
            "bl3": np.asarray(bl3, f32).reshape(O, 1),
        }
        per_core.append(inp)

    meta = {"chunk_plan": chunk_plan,
            "Ttot": sum(c[2] + c[3] for c in chunk_plan)}
    return per_core, meta


# ---------------- device program ----------------


def build_program(geo, meta, n_cores):
    import os
    dbg_layers = int(os.environ.get("KN_LAYERS", "3"))
    dbg_stage = os.environ.get("KN_STAGE", "full")
    dbg_agg = os.environ.get("KN_AGG", "ep")
    import concourse.bacc as bacc
    import concourse.bass as bass
    import concourse.mybir as mybir
    import concourse.tile as tile

    g = geo
    H, H2, O = g.HID, g.HID // 2, g.OUT
    dt = mybir.dt
    Alu = mybir.AluOpType
    Act = mybir.ActivationFunctionType
    chunk_plan = meta["chunk_plan"]
    Ttot = meta["Ttot"]
    Tmax = max(max(c[2] + c[3] for c in chunk_plan), g.NWIN)
    Tblk = max(r[1] - r[0] + 1
               for (_, _, _, _, rl, rh) in chunk_plan
               for r in rl + rh if r is not None)

    nc = bacc.Bacc("TRN2", target_bir_lowering=False, debug=False,
                   num_devices=n_cores)
    rg = [list(range(n_cores))]

    def din(name, shape, dtype):
        return nc.dram_tensor(name, list(shape), dtype, kind="ExternalInput")

    xT_d = din("xT", [g.F_IN, g.NLP], dt.bfloat16)
    U1rows_d = din("U1rows", [128, 4, H], dt.bfloat16)
    ohconv_d = din("ohconv", [128, 4 * len(g.conv_chunks), CONV_CHUNK],
                   dt.bfloat16)
    gap_last_d = din("gap_last", [96, g.G // 16], dt.int16)
    maxcol_d = din("maxcol", [96, g.G // 16], dt.int16)
    maskneg_d = din("maskneg_row", [1, g.NLP], dt.bfloat16)
    eidx_d = din("eidx", [128, Ttot * 8], dt.int16)
    dstslot_d = din("dstslot", [128, Ttot], dt.float32)
    dinv_d = din("dinv_sb", [128, g.NWIN], dt.float32)
    iota_d = din("iota512", [128, CHUNK_WINDOWS * WIN], dt.float32)
    id96_d = din("id96", [96, 96], dt.float32)
    id128b_d = din("id128b", [128, 128], dt.bfloat16)
    invc_d = din("invc_row", [1, g.G], dt.float32)
    W_d = {n: din(n, [g.F_IN if n[1] == "1" else H, H], dt.bfloat16)
           for n in ("W1a", "W1b", "W2a", "W2b", "W3a", "W3b")}
    b_d = {n: din(n, [1, H], dt.float32)
           for n in ("b1_row", "b2_row", "b3_row")}
    Wl1a_d = din("Wl1a", [H, H], dt.float32)
    Wl1b_d = din("Wl1b", [H, H], dt.float32)
    Wl2_d = din("Wl2", [H, H2], dt.float32)
    Wl3_d = din("Wl3", [H2, O], dt.float32)
    bl1_d = din("bl1", [H, 1], dt.float32)
    bl2_d = din("bl2", [H2, 1], dt.float32)
    bl3_d = din("bl3", [O, 1], dt.float32)

    out_d = nc.dram_tensor("out", [O, g.G], dt.float32, kind="ExternalOutput")

    # internal DRAM
    tshard = nc.dram_tensor("tshard", [g.NLP, 128], dt.bfloat16,
                            kind="Internal")
    table = nc.dram_tensor("table", [g.NP, 128], dt.bfloat16,
                           kind="Internal", addr_space="Shared")
    gap_in = [nc.dram_tensor(f"gap_in{i}", [96, g.G], dt.float32,
                             kind="Internal") for i in range(3)]
    gap_out = [nc.dram_tensor(f"gap_out{i}", [96, g.G], dt.float32,
                              kind="Internal", addr_space="Shared")
               for i in range(3)]
    gmp_in = [nc.dram_tensor(f"gmp_in{i}", [96, g.G], dt.float32,
                             kind="Internal") for i in range(3)]
    gmp_out = [nc.dram_tensor(f"gmp_out{i}", [96, g.G], dt.float32,
                              kind="Internal", addr_space="Shared")
               for i in range(3)]

    with tile.TileContext(nc) as tc:
        import contextlib
        stk = contextlib.ExitStack()
        pp = stk.enter_context(tc.tile_pool(name="persist", bufs=1))
        wk = stk.enter_context(tc.tile_pool(name="work", bufs=2))
        ep = stk.enter_context(tc.tile_pool(name="epil", bufs=2))
        ps_conv = stk.enter_context(
            tc.tile_pool(name="ps_conv", bufs=2, space="PSUM"))
        ps_tr = stk.enter_context(
            tc.tile_pool(name="ps_tr", bufs=2, space="PSUM"))
        ps_agg = stk.enter_context(
            tc.tile_pool(name="ps_agg", bufs=4, space="PSUM"))

        def load(pool, dram, shape, dtype, tag, bcast=None):
            t = pool.tile(shape, dtype, tag=tag, name=tag)
            src = dram.ap() if bcast is None else dram.ap().to_broadcast(bcast)
            nc.sync.dma_start(out=t[:], in_=src)
            return t

        # constants
        iota_sb = load(pp, iota_d, [128, CHUNK_WINDOWS * WIN], dt.float32,
                       "iota")
        id96_sb = load(pp, id96_d, [96, 96], dt.float32, "id96")
        id128b_sb = load(pp, id128b_d, [128, 128], dt.bfloat16, "id128b")
        dinv_sbT = load(pp, dinv_d, [128, g.NWIN], dt.float32, "dinv")
        invc_bc = load(pp, invc_d, [96, g.G], dt.float32, "invc",
                       bcast=(96, g.G))
        maskneg_sb = load(pp, maskneg_d, [96, g.NLP], dt.bfloat16, "maskn",
                          bcast=(96, g.NLP))
        eidx_sb = load(pp, eidx_d, [128, Ttot * 8], dt.int16, "eidx")
        dst_sb = load(pp, dstslot_d, [128, Ttot], dt.float32, "dstslot")
        gap_last_sb = load(pp, gap_last_d, [96, g.G // 16], dt.int16, "glast")
        maxcol_sb = load(pp, maxcol_d, [96, g.G // 16], dt.int16, "maxcol")
        Urows = load(pp, U1rows_d, [128, 4, H], dt.bfloat16, "Urows")
        tabkeep = pp.tile([128, g.NWIN, 96], dt.bfloat16, tag="tabkeep",
                          name="tabkeep")
        W_sb = {n: load(pp, W_d[n], list(W_d[n].shape), dt.bfloat16, n)
                for n in W_d}
        b_bc = {n: load(pp, b_d[n], [128, H], dt.float32, n, bcast=(128, H))
                for n in b_d}
        Wl1a_sb = load(pp, Wl1a_d, [H, H], dt.float32, "Wl1a")
        Wl1b_sb = load(pp, Wl1b_d, [H, H], dt.float32, "Wl1b")
        Wl2_sb = load(pp, Wl2_d, [H, H2], dt.float32, "Wl2")
        Wl3_sb = load(pp, Wl3_d, [H2, O], dt.float32, "Wl3")
        bl1_sb = load(pp, bl1_d, [H, 1], dt.float32, "bl1")
        bl2_sb = load(pp, bl2_d, [H2, 1], dt.float32, "bl2")
        bl3_sb = load(pp, bl3_d, [O, 1], dt.float32, "bl3")

        xbuf = [pp.tile([128, g.NLP], dt.bfloat16, tag=f"xbuf{i}", name=f"xbuf{i}")
                for i in range(2)]
        nc.sync.dma_start(out=xbuf[0][:g.F_IN, :], in_=xT_d.ap())

        scano = pp.tile([96, g.NLP], dt.float32, tag="scano", name="scano")
        gmp_all = pp.tile([96, 3 * g.G], dt.float32, tag="gmp_all", name="gmp_all")
        mg_sb = [pp.tile([96, g.G], dt.float32, tag=f"mg{i}", name=f"mg{i}")
                 for i in range(3)]
        gapar_sb = [pp.tile([96, g.G], dt.float32, tag=f"gapar{i}", name=f"gapar{i}")
                    for i in range(3)]

        zinit = wk.tile([128, Tmax, 128], dt.bfloat16, tag="gath", name="zinit")
        nc.gpsimd.memset(zinit[:, :g.NWIN, :], 0.0)
        nc.sync.dma_start(
            out=tshard.ap().rearrange("(w p) c -> p w c", p=128),
            in_=zinit[:, :g.NWIN, :])

        layer_W = [("W1a", "W1b", "b1_row"), ("W2a", "W2b", "b2_row"),
                   ("W3a", "W3b", "b3_row")]

        for l in range(dbg_layers):
            F = g.F_IN if l == 0 else H
            Wa = W_sb[layer_W[l][0]]
            Wb = W_sb[layer_W[l][1]]
            bias = b_bc[layer_W[l][2]]
            x_src, x_dst = xbuf[l % 2], xbuf[(l + 1) % 2]

            # ---- conv + gather-table build ----
            for ci, (cs, cw) in enumerate(g.conv_chunks):
                ohc = wk.tile([128, 4, CONV_CHUNK], dt.bfloat16, tag="ohc",
                              name="ohc")
                nc.sync.dma_start(out=ohc[:],
                                  in_=ohconv_d.ap()[:, 4 * ci:4 * ci + 4, :])
                psc = ps_conv.tile([96, CONV_CHUNK], dt.float32, tag="psc", name="psc")
                nc.tensor.matmul(out=psc[:, :cw], lhsT=Wa[:],
                                 rhs=x_src[:F, cs:cs + cw],
                                 start=True, stop=False)
                for b in range(4):
                    nc.tensor.matmul(out=psc[:, :cw], lhsT=Urows[:, b, :],
                                     rhs=ohc[:, b, :cw], start=False,
                                     stop=b == 3)
                hT = wk.tile([96, CONV_CHUNK], dt.float32, tag="hT", name="hT")
                nc.vector.tensor_copy(out=hT[:, :cw], in_=psc[:, :cw])
                for wo in range(0, cw, 128):
                    w = (cs + wo) // 128
                    pt = ps_tr.tile([128, 128], dt.float32, tag="ptr", name="ptr")
                    nc.tensor.transpose(out=pt[:, :96],
                                        in_=hT[:, wo:wo + 128],
                                        identity=id96_sb[:])
                    nc.scalar.activation(out=tabkeep[:, w, :], in_=pt[:, :96],
                                         func=Act.Copy,
                                         scale=dinv_sbT[:, w:w + 1])
                    nc.sync.dma_start(
                        out=tshard.ap()[w * 128:(w + 1) * 128, :96],
                        in_=tabkeep[:, w, :])

            if dbg_stage == "conv":
                break
            # ---- all-gather the table ----
            nc.gpsimd.collective_compute(
                "AllGather", Alu.bypass, replica_groups=rg,
                ins=[tshard.ap()], outs=[table.ap()])

            if dbg_stage == "ag":
                break
            # ---- edge aggregation ----
            t_base = 0
            for (w0, nw, Tlo, Thi, rng_l, rng_h) in chunk_plan:
                T = Tlo + Thi
                gath = wk.tile([128, Tmax, 128], dt.bfloat16, tag="gath", name="gath")
                # dma_gather is limited to 1024 indices (8 tiles) per call
                for (goff, tstart, cnt, hi) in (
                        (0, t_base, Tlo, False),
                        (Tlo, t_base + Tlo, Thi, True)):
                    if cnt == 0:
                        continue
                    tbl_ap = table.ap()[HALF:, :] if hi else table.ap()
                    done = 0
                    while done < cnt:
                        nt = min(8, cnt - done)
                        nc.gpsimd.dma_gather(
                            gath[:, goff + done:goff + done + nt, :],
                            tbl_ap,
                            eidx_sb[:, 8 * (tstart + done):
                                    8 * (tstart + done + nt)],
                            num_idxs=nt * 128, num_idxs_reg=nt * 128,
                            elem_size=128)
                        done += nt
                if dbg_agg == "gather":
                    t_base += T
                    continue
                # per window: masked one-hot matmuls over its tile ranges
                blocks = []   # (wi, gath offset of first tile, ntiles)
                for wi in range(nw):
                    if rng_l[wi] is not None:
                        blocks.append((wi, rng_l[wi][0],
                                       rng_l[wi][1] - rng_l[wi][0] + 1))
                    if rng_h[wi] is not None:
                        blocks.append((wi, Tlo + rng_h[wi][0],
                                       rng_h[wi][1] - rng_h[wi][0] + 1))
                pags = [ps_agg.tile([96, 128], dt.float32, tag="pag", name="pag")
                        for _ in range(nw)]
                remaining = [0] * nw
                for (wi, _, Tb) in blocks:
                    remaining[wi] += Tb
                for wi in range(nw):
                    # self-loop term: psum += table rows of this window
                    nc.tensor.matmul(out=pags[wi][:],
                                     lhsT=tabkeep[:, w0 + wi, :],
                                     rhs=id128b_sb[:],
                                     start=True, stop=remaining[wi] == 0)
                for (wi, toff, Tb) in blocks:
                    oh = wk.tile([128, Tblk, 128], dt.bfloat16, tag="oh",
                                 name="oh", bufs=4)
                    a = t_base + toff
                    nc.vector.tensor_tensor(
                        out=oh[:, 0:Tb, :],
                        in0=dst_sb[:, a:a + Tb, None]
                            .to_broadcast((128, Tb, 128)),
                        in1=iota_sb[:, None, wi * 128:(wi + 1) * 128]
                            .to_broadcast((128, Tb, 128)),
                        op=Alu.is_equal)
                    for t in range(Tb):
                        if dbg_agg == "onehot":
                            continue
                        remaining[wi] -= 1
                        nc.tensor.matmul(out=pags[wi][:],
                                         lhsT=gath[:, toff + t, :96],
                                         rhs=oh[:, t, :],
                                         start=False,
                                         stop=remaining[wi] == 0)

                for wi in range(nw):
                    if dbg_agg in ("onehot", "mm"):
                        break
                    w = w0 + wi
                    sb1 = ep.tile([96, 128], dt.float32, tag="ep1", name="ep1")
                    nc.vector.tensor_copy(out=sb1[:], in_=pags[wi][:])
                    pt2 = ps_tr.tile([128, 128], dt.float32, tag="ptr", name="ptr")
                    nc.tensor.transpose(out=pt2[:, :96], in_=sb1[:],
                                        identity=id96_sb[:])
                    sb2 = ep.tile([128, 96], dt.float32, tag="ep2", name="ep2")
                    nc.scalar.activation(out=sb2[:], in_=pt2[:, :96],
                                         func=Act.Copy,
                                         scale=dinv_sbT[:, w:w + 1])
                    sb3 = ep.tile([128, 96], dt.bfloat16, tag="ep3", name="ep3")
                    if l < 2:
                        tmp = ep.tile([128, 96], dt.float32, tag="ep3f", name="ep3f")
                        nc.vector.tensor_add(out=tmp[:], in0=sb2[:],
                                             in1=bias[:, :96])
                        nc.vector.tensor_scalar_max(out=sb3[:], in0=tmp[:],
                                                    scalar1=0.0)
                    else:
                        nc.vector.tensor_add(out=sb3[:], in0=sb2[:],
                                             in1=bias[:, :96])
                    pt3 = ps_tr.tile([128, 128], dt.bfloat16, tag="ptr", name="ptr")
                    nc.tensor.transpose(out=pt3[:96, :], in_=sb3[:],
                                        identity=id128b_sb[:])
                    nc.vector.tensor_copy(
                        out=x_dst[:96, w * 128:(w + 1) * 128],
                        in_=pt3[:96, :])
                t_base += T

            nc.gpsimd.memset(x_dst[:96, g.NL:g.NLP], PAD_VAL)

            if dbg_stage == "agg":
                break
            # ---- pooling ----
            nc.vector.memset(scano[:, 0:1], 0.0)
            nc.vector.tensor_tensor_scan(
                out=scano[:, 1:g.NL + 1], data0=x_dst[:96, 0:g.NL],
                data1=x_dst[:96, 0:g.NL],
                initial=0.0, op0=Alu.add, op1=Alu.bypass)
            cum_l = ep.tile([96, g.G + 1], dt.float32, tag="cuml", name="cuml",
                            bufs=1)
            nc.vector.memset(cum_l[:, 0:1], 0.0)
            nc.gpsimd.ap_gather(cum_l[:, 1:g.G + 1], scano[:], gap_last_sb[:],
                                channels=96, num_elems=g.NLP, d=1,
                                num_idxs=g.G)
            gaps = ep.tile([96, g.G], dt.float32, tag="gaps", name="gaps", bufs=1)
            nc.vector.tensor_tensor(out=gaps[:], in0=cum_l[:, 1:g.G + 1],
                                    in1=cum_l[:, 0:g.G],
                                    op=Alu.subtract)
            nc.sync.dma_start(out=gap_in[l].ap(), in_=gaps[:])
            nc.gpsimd.collective_compute(
                "AllReduce", Alu.add, replica_groups=rg,
                ins=[gap_in[l].ap()], outs=[gap_out[l].ap()])
            nc.sync.dma_start(out=gapar_sb[l][:], in_=gap_out[l].ap())

            nc.vector.tensor_tensor_scan(
                out=scano[:], data0=maskneg_sb[:], data1=x_dst[:96, :],
                initial=0.0, op0=Alu.add, op1=Alu.max)
            nc.gpsimd.ap_gather(gmp_all[:, l * g.G:(l + 1) * g.G],
                                scano[:], maxcol_sb[:],
                                channels=96, num_elems=g.NLP, d=1,
                                num_idxs=g.G)
            # per-layer gmp AllReduce: layers 0-1 overlap the next layer's
            # edge gathers; only layer 2's is exposed
            nc.sync.dma_start(out=gmp_in[l].ap(),
                              in_=gmp_all[:, l * g.G:(l + 1) * g.G])
            nc.gpsimd.collective_compute(
                "AllReduce", Alu.max, replica_groups=rg,
                ins=[gmp_in[l].ap()], outs=[gmp_out[l].ap()])

            nc.vector.tensor_mul(out=mg_sb[l][:], in0=gapar_sb[l][:],
                                 in1=invc_bc[:])

            if l < 2:
                # build Urows for next layer: U = Wb_{l+1}^T @ mg,
                # transposed into [128 graph, 4 block, 96] for the conv
                Wbn = W_sb[layer_W[l + 1][1]]
                mgb = ep.tile([96, g.G], dt.bfloat16, tag="mgb", name="mgb",
                              bufs=1)
                nc.scalar.copy(out=mgb[:], in_=mg_sb[l][:])
                psU = ps_conv.tile([96, g.G], dt.float32, tag="psc", name="psc")
                nc.tensor.matmul(out=psU[:], lhsT=Wbn[:], rhs=mgb[:],
                                 start=True, stop=True)
                Usb = ep.tile([96, g.G], dt.bfloat16, tag="Usb", name="Usb",
                              bufs=1)
                nc.vector.tensor_copy(out=Usb[:], in_=psU[:])
                for b in range(4):
                    ptU = ps_tr.tile([128, 128], dt.bfloat16, tag="ptr",
                                     name="ptr")
                    nc.tensor.transpose(out=ptU[:, :96],
                                        in_=Usb[:, b * 128:(b + 1) * 128],
                                        identity=id128b_sb[:96, :96])
                    nc.scalar.copy(out=Urows[:, b, :], in_=ptU[:, :96])

        debug_cut = dbg_stage != "full" or dbg_layers < 3
        if debug_cut:
            nc.gpsimd.dma_start(out=out_d.ap(), in_=xbuf[0][:O, :g.G])
        # ---- final readout MLP (f32) ----
        if not debug_cut:
            gmpar = pp.tile([96, 3 * g.G], dt.float32, tag="gmpar", name="gmpar")
            for i in range(3):
                nc.sync.dma_start(out=gmpar[:, i * g.G:(i + 1) * g.G],
                                  in_=gmp_out[i].ap())

            hTa = pp.tile([96, g.G], dt.float32, tag="hTa", name="hTa")
            hTb = pp.tile([96, g.G], dt.float32, tag="hTb", name="hTb")
            nc.vector.tensor_add(out=hTa[:], in0=gmpar[:, 0:g.G],
                                 in1=gmpar[:, g.G:2 * g.G])
            nc.vector.tensor_add(out=hTa[:], in0=hTa[:],
                                 in1=gmpar[:, 2 * g.G:3 * g.G])
            nc.vector.tensor_add(out=hTb[:], in0=mg_sb[0][:], in1=mg_sb[1][:])
            nc.vector.tensor_add(out=hTb[:], in0=hTb[:], in1=mg_sb[2][:])

            ps1 = ps_conv.tile([96, g.G], dt.float32, tag="psc", name="psc")
            nc.tensor.matmul(out=ps1[:], lhsT=Wl1a_sb[:], rhs=hTa[:],
                             start=True, stop=False)
            nc.tensor.matmul(out=ps1[:], lhsT=Wl1b_sb[:], rhs=hTb[:],
                             start=False, stop=True)
            o1 = pp.tile([96, g.G], dt.float32, tag="o1", name="o1")
            nc.scalar.activation(out=o1[:], in_=ps1[:], func=Act.Relu,
                                 bias=bl1_sb[:])
            ps2 = ps_conv.tile([96, g.G], dt.float32, tag="psc", name="psc")
            nc.tensor.matmul(out=ps2[:H2, :], lhsT=Wl2_sb[:], rhs=o1[:],
                             start=True, stop=True)
            o2 = pp.tile([H2, g.G], dt.float32, tag="o2", name="o2")
            nc.scalar.activation(out=o2[:], in_=ps2[:H2, :], func=Act.Relu,
                                 bias=bl2_sb[:])
            ps3 = ps_conv.tile([96, g.G], dt.float32, tag="psc", name="psc")
            nc.tensor.matmul(out=ps3[:O, :], lhsT=Wl3_sb[:], rhs=o2[:],
                             start=True, stop=True)
            o3 = pp.tile([O, g.G], dt.float32, tag="o3", name="o3")
            nc.scalar.activation(out=o3[:], in_=ps3[:O, :], func=Act.Identity,
                                 bias=bl3_sb[:])
            nc.sync.dma_start(out=out_d.ap(), in_=o3[:])

        stk.close()

    nc.compile()
    return nc


_CACHE = {}


def _get_program(geo, meta, n_cores):
    key = (repr(sorted(geo.__dict__.items(), key=str)),
           repr(meta["chunk_plan"]), n_cores)
    if key not in _CACHE:
        _CACHE[key] = build_program(geo, meta, n_cores)
    return _CACHE[key]


def kernel(**inputs):
    from concourse.bass_utils import run_bass_kernel_spmd

    geo = Geo(CFG)
    inputs = {k: np.asarray(v) for k, v in inputs.items()}
    per_core, meta = prep(geo, **inputs)
    nc = _get_program(geo, meta, geo.C)
    res = run_bass_kernel_spmd(nc, per_core, core_ids=list(range(geo.C)))
    out = np.asarray(res.results[0]["out"], f32)   # [OUT, G]
    return np.ascontiguousarray(out.T)             # [G, OUT] float32



# revision 41
# speedup vs baseline: 1.2869x; 1.0267x over previous
"""Trainium2 Bass kernel for nn_BenchGNN_29300266893894 (3-layer GCN with
global-feature concat + global mean/max pooling readout + MLP head).

Self-contained: host-side sharding/packing prep + SPMD Bass/Tile program on
8 NeuronCores via run_bass_kernel_spmd.

Sharding: nodes are split into 8 contiguous shards of 6250 (padded to 6272 =
49*128). Edges are partitioned by dst owner and sorted by dst window; weight
matrices are replicated. Per layer, each core computes h = x_cat @ W for its
nodes, builds a bf16 "gather table" of h' = h*dinv rows (padded to 256 B),
AllGathers it, then aggregates its incident edges with hardware dma_gather +
one-hot matmuls accumulating per-128-dst-window PSUM tiles. Graph pooling is
done with DVE segmented scans + ap_gather boundary extraction and AllReduce.
"""
import sys
import numpy as np
import ml_dtypes

sys.path.insert(0, "/opt/trn_rl_repo")

bf16 = ml_dtypes.bfloat16
f32 = np.float32

# ---------------- problem geometry (hardcoded) ----------------
CFG = dict(N=50000, E=800000, G=512, F_IN=128, HID=96, OUT=10, C=8)

WIN = 128
WINA = 25            # windows per core in table half A
COLA = WINA * WIN    # 3200
CHUNK_WINDOWS = 4
CONV_CHUNK = 512
PAD_SLOT = 999.0
PAD_VAL = -60000.0


class Geo:
    def __init__(self, cfg):
        self.__dict__.update(cfg)
        assert self.N % self.C == 0
        self.NL = self.N // self.C
        self.NWIN = -(-self.NL // WIN)
        self.NLP = self.NWIN * WIN
        # pooling pad-column tricks need at least one pad column per core
        assert self.NLP > self.NL
        self.NP = self.NLP * self.C
        self.conv_chunks = [(o, min(CONV_CHUNK, self.NLP - o))
                            for o in range(0, self.NLP, CONV_CHUNK)]


def _wrap16(idx, channels):
    idx = np.asarray(idx, np.int16)
    assert len(idx) % 16 == 0
    idx16 = idx.reshape(-1, 16).T
    return np.ascontiguousarray(np.tile(idx16, (channels // 16, 1)))


def prep(geo, x, edge_index, batch, W1, b1, W2, b2, W3, b3,
         Wl1, bl1, Wl2, bl2, Wl3, bl3):
    g = geo
    x = np.asarray(x, f32)
    src = np.asarray(edge_index[0], np.int64)
    dst = np.asarray(edge_index[1], np.int64)
    batch = np.asarray(batch, np.int64)

    def gid_of(n):
        return (n // g.NL) * g.NLP + (n % g.NL)

    deg = np.bincount(dst, minlength=g.N).astype(f32) + 1.0
    dinv = (1.0 / np.sqrt(deg)).astype(f32)
    counts = np.bincount(batch, minlength=g.G).astype(f32)
    invc = (1.0 / np.maximum(counts, 1.0)).astype(f32)

    sums1 = np.zeros((g.G, g.F_IN), f32)
    np.add.at(sums1, batch, x)
    mean1 = (sums1 * invc[:, None]).astype(f32)          # [G, F_IN]
    U1rows = (mean1 @ np.asarray(W1[g.F_IN:], f32))      # [G, HID]

    src_g = gid_of(src)

    core_win = []
    for k in range(g.C):
        lo_n, hi_n = k * g.NL, (k + 1) * g.NL
        sel = (dst >= lo_n) & (dst < hi_n)
        es, ed = src_g[sel], (dst[sel] - lo_n)
        order = np.argsort(ed, kind="stable")
        es, ed = es[order], ed[order]
        wstart = np.searchsorted(ed, np.arange(0, g.NLP + 1, WIN))
        wins = []
        COLB = g.NLP - COLA
        for w in range(g.NWIN):
            a, b = wstart[w], wstart[w + 1]
            ws, wd = es[a:b], ed[a:b] - w * WIN
            c_, off = ws // g.NLP, ws % g.NLP
            a_m = off < COLA
            wins.append((c_[a_m] * COLA + off[a_m], wd[a_m],
                         c_[~a_m] * COLB + (off[~a_m] - COLA), wd[~a_m]))
        core_win.append(wins)

    # chunk plan: per chunk of CHUNK_WINDOWS windows, edges of all windows are
    # packed contiguously (chunk-local slot ids); tile counts are chunk-level
    # cross-core maxima, and each window gets a [t0, t1] tile range (union
    # over cores) whose tiles feed its psum via masked one-hot matmuls.
    def _ranges(lens_kw, nw):
        out = []
        for wi in range(nw):
            t0s, t1s = [], []
            for k in range(g.C):
                s = sum(lens_kw[k][:wi])
                L = lens_kw[k][wi]
                if L:
                    t0s.append(s // 128)
                    t1s.append((s + L - 1) // 128)
            out.append((min(t0s), max(t1s)) if t0s else None)
        return out

    chunk_plan = []   # (w0, nw, TC_lo, TC_hi, rng_lo, rng_hi)
    for w0 in range(0, g.NWIN, CHUNK_WINDOWS):
        nw = min(CHUNK_WINDOWS, g.NWIN - w0)
        lens_lo = [[len(core_win[k][w0 + wi][0]) for wi in range(nw)]
                   for k in range(g.C)]
        lens_hi = [[len(core_win[k][w0 + wi][2]) for wi in range(nw)]
                   for k in range(g.C)]
        TC_lo = -(-max(sum(r) for r in lens_lo) // 128)
        TC_hi = -(-max(sum(r) for r in lens_hi) // 128)
        chunk_plan.append((w0, nw, TC_lo, TC_hi,
                           _ranges(lens_lo, nw), _ranges(lens_hi, nw)))

    per_core = []
    for k in range(g.C):
        lo_n, hi_n = k * g.NL, (k + 1) * g.NL
        idx_list, slot_list = [], []
        for (w0, nw, TCl, TCh, _, _) in chunk_plan:
            for half, TC in ((0, TCl), (2, TCh)):
                s_arr = np.concatenate(
                    [core_win[k][w0 + wi][half] for wi in range(nw)]
                    + [np.zeros(0, np.int64)])
                d_arr = np.concatenate(
                    [core_win[k][w0 + wi][half + 1] + 128 * wi
                     for wi in range(nw)] + [np.zeros(0, np.int64)])
                pad = TC * 128 - len(s_arr)
                idx_list.append(np.concatenate(
                    [s_arr, np.zeros(pad, np.int64)]))
                slot_list.append(np.concatenate(
                    [d_arr, np.full(pad, PAD_SLOT)]))
        idx_all = np.concatenate(idx_list)
        slot_all = np.concatenate(slot_list).astype(f32)
        idx_sb = _wrap16(idx_all, 128)
        dst_sb = np.ascontiguousarray(
            slot_all.reshape(-1, 128).T.astype(f32))

        bl = batch[lo_n:hi_n]
        # one-hot graph-block matrices for the conv global-feature term:
        # ohconv[p, c, b, j] = 1{batch_local[cs_c + j] == 128*b + p}
        nch = len(g.conv_chunks)
        ohconv = np.zeros((128, nch, 4, CONV_CHUNK), bf16)
        for c, (cs, cw) in enumerate(g.conv_chunks):
            seg = bl[cs:min(cs + cw, g.NL)]
            for j, gid in enumerate(seg):
                ohconv[gid % 128, c, gid // 128, j] = 1
        bcol = np.searchsorted(bl, np.arange(g.G), side="right") - 1
        gap_last = _wrap16(np.where(bcol < 0, 0, bcol + 1), 96)
        present = np.zeros(g.G, bool)
        present[np.unique(bl)] = True
        maxcol = _wrap16(np.where(present, bcol, g.NLP - 1), 96)

        gstart = np.searchsorted(bl, np.arange(g.G), side="left")
        maskneg = np.zeros(g.NLP, f32)
        maskneg[np.unique(gstart[present])] = -1e30
        maskneg[g.NL] = -1e30
        maskneg_row = maskneg.reshape(1, g.NLP).astype(bf16)

        xT = np.zeros((g.F_IN, g.NLP), f32)
        xT[:, :g.NL] = x[lo_n:hi_n].T
        dinv_loc = np.zeros(g.NLP, f32)
        dinv_loc[:g.NL] = dinv[lo_n:hi_n]
        dinv_sb = np.ascontiguousarray(dinv_loc.reshape(g.NWIN, WIN).T)

        H, H2, O = g.HID, g.HID // 2, g.OUT
        inp = {
            "xT": xT.astype(bf16),
            "U1rows": np.ascontiguousarray(
                U1rows.reshape(4, 128, H).transpose(1, 0, 2)).astype(bf16),
            "ohconv": np.ascontiguousarray(
                ohconv.reshape(128, 4 * nch, CONV_CHUNK)),
            "gap_last": gap_last, "maxcol": maxcol,
            "maskneg_row": maskneg_row,
            "eidx": idx_sb,
            "dstslot": dst_sb,
            "dinv_sb": dinv_sb,
            "iota512": np.tile(np.arange(CHUNK_WINDOWS * WIN, dtype=f32),
                               (128, 1)),
            "id96": np.eye(96, dtype=f32),
            "id128b": np.eye(128, dtype=bf16),
            "invc_row": invc.reshape(1, g.G),
            "W1a": np.asarray(W1[:g.F_IN], bf16),
            "W1b": np.asarray(W1[g.F_IN:], bf16),
            "W2a": np.asarray(W2[:H], bf16), "W2b": np.asarray(W2[H:], bf16),
            "W3a": np.asarray(W3[:H], bf16), "W3b": np.asarray(W3[H:], bf16),
            "b1_row": np.asarray(b1, f32).reshape(1, H),
            "b2_row": np.asarray(b2, f32).reshape(1, H),
            "b3_row": np.asarray(b3, f32).reshape(1, H),
            "Wl1a": np.asarray(Wl1[:H], f32), "Wl1b": np.asarray(Wl1[H:], f32),
            "Wl2": np.asarray(Wl2, f32), "Wl3": np.asarray(Wl3, f32),
            "bl1": np.asarray(bl1, f32).reshape(H, 1),
            "bl2": np.asarray(bl2, f32).reshape(H2, 1),
            "bl3": np.asarray(bl3, f32).reshape(O, 1),
        }
        per_core.append(inp)

    meta = {"chunk_plan": chunk_plan,
            "Ttot": sum(c[2] + c[3] for c in chunk_plan)}
    return per_core, meta


# ---------------- device program ----------------


def build_program(geo, meta, n_cores):
    import os
    dbg_layers = int(os.environ.get("KN_LAYERS", "3"))
    dbg_stage = os.environ.get("KN_STAGE", "full")
    dbg_agg = os.environ.get("KN_AGG", "ep")
    import concourse.bacc as bacc
    import concourse.bass as bass
    import concourse.mybir as mybir
    import concourse.tile as tile

    g = geo
    H, H2, O = g.HID, g.HID // 2, g.OUT
    dt = mybir.dt
    Alu = mybir.AluOpType
    Act = mybir.ActivationFunctionType
    chunk_plan = meta["chunk_plan"]
    Ttot = meta["Ttot"]
    Tmax = max(max(c[2] + c[3] for c in chunk_plan), g.NWIN)
    Tblk = max(r[1] - r[0] + 1
               for (_, _, _, _, rl, rh) in chunk_plan
               for r in rl + rh if r is not None)

    nc = bacc.Bacc("TRN2", target_bir_lowering=False, debug=False,
                   num_devices=n_cores)
    rg = [list(range(n_cores))]

    def din(name, shape, dtype):
        return nc.dram_tensor(name, list(shape), dtype, kind="ExternalInput")

    xT_d = din("xT", [g.F_IN, g.NLP], dt.bfloat16)
    U1rows_d = din("U1rows", [128, 4, H], dt.bfloat16)
    ohconv_d = din("ohconv", [128, 4 * len(g.conv_chunks), CONV_CHUNK],
                   dt.bfloat16)
    gap_last_d = din("gap_last", [96, g.G // 16], dt.int16)
    maxcol_d = din("maxcol", [96, g.G // 16], dt.int16)
    maskneg_d = din("maskneg_row", [1, g.NLP], dt.bfloat16)
    eidx_d = din("eidx", [128, Ttot * 8], dt.int16)
    dstslot_d = din("dstslot", [128, Ttot], dt.float32)
    dinv_d = din("dinv_sb", [128, g.NWIN], dt.float32)
    iota_d = din("iota512", [128, CHUNK_WINDOWS * WIN], dt.float32)
    id96_d = din("id96", [96, 96], dt.float32)
    id128b_d = din("id128b", [128, 128], dt.bfloat16)
    invc_d = din("invc_row", [1, g.G], dt.float32)
    W_d = {n: din(n, [g.F_IN if n[1] == "1" else H, H], dt.bfloat16)
           for n in ("W1a", "W1b", "W2a", "W2b", "W3a", "W3b")}
    b_d = {n: din(n, [1, H], dt.float32)
           for n in ("b1_row", "b2_row", "b3_row")}
    Wl1a_d = din("Wl1a", [H, H], dt.float32)
    Wl1b_d = din("Wl1b", [H, H], dt.float32)
    Wl2_d = din("Wl2", [H, H2], dt.float32)
    Wl3_d = din("Wl3", [H2, O], dt.float32)
    bl1_d = din("bl1", [H, 1], dt.float32)
    bl2_d = din("bl2", [H2, 1], dt.float32)
    bl3_d = din("bl3", [O, 1], dt.float32)

    out_d = nc.dram_tensor("out", [O, g.G], dt.float32, kind="ExternalOutput")

    # internal DRAM (table split in two halves so AG-A overlaps conv tail)
    COLB = g.NLP - COLA
    tshardA = nc.dram_tensor("tshardA", [COLA, 128], dt.bfloat16,
                             kind="Internal")
    tshardB = nc.dram_tensor("tshardB", [COLB, 128], dt.bfloat16,
                             kind="Internal")
    tableA = nc.dram_tensor("tableA", [g.C * COLA, 128], dt.bfloat16,
                            kind="Internal", addr_space="Shared")
    tableB = nc.dram_tensor("tableB", [g.C * COLB, 128], dt.bfloat16,
                            kind="Internal", addr_space="Shared")
    gap_in = [nc.dram_tensor(f"gap_in{i}", [96, g.G], dt.float32,
                             kind="Internal") for i in range(3)]
    gap_out = [nc.dram_tensor(f"gap_out{i}", [96, g.G], dt.float32,
                              kind="Internal", addr_space="Shared")
               for i in range(3)]
    gmp_in = [nc.dram_tensor(f"gmp_in{i}", [96, g.G], dt.float32,
                             kind="Internal") for i in range(3)]
    gmp_out = [nc.dram_tensor(f"gmp_out{i}", [96, g.G], dt.float32,
                              kind="Internal", addr_space="Shared")
               for i in range(3)]

    with tile.TileContext(nc) as tc:
        import contextlib
        stk = contextlib.ExitStack()
        pp = stk.enter_context(tc.tile_pool(name="persist", bufs=1))
        wk = stk.enter_context(tc.tile_pool(name="work", bufs=2))
        ep = stk.enter_context(tc.tile_pool(name="epil", bufs=2))
        ps_conv = stk.enter_context(
            tc.tile_pool(name="ps_conv", bufs=2, space="PSUM"))
        ps_tr = stk.enter_context(
            tc.tile_pool(name="ps_tr", bufs=2, space="PSUM"))
        ps_agg = stk.enter_context(
            tc.tile_pool(name="ps_agg", bufs=4, space="PSUM"))

        def load(pool, dram, shape, dtype, tag, bcast=None):
            t = pool.tile(shape, dtype, tag=tag, name=tag)
            src = dram.ap() if bcast is None else dram.ap().to_broadcast(bcast)
            nc.sync.dma_start(out=t[:], in_=src)
            return t

        # constants
        iota_sb = load(pp, iota_d, [128, CHUNK_WINDOWS * WIN], dt.float32,
                       "iota")
        id96_sb = load(pp, id96_d, [96, 96], dt.float32, "id96")
        id128b_sb = load(pp, id128b_d, [128, 128], dt.bfloat16, "id128b")
        dinv_sbT = load(pp, dinv_d, [128, g.NWIN], dt.float32, "dinv")
        invc_bc = load(pp, invc_d, [96, g.G], dt.float32, "invc",
                       bcast=(96, g.G))
        maskneg_sb = load(pp, maskneg_d, [96, g.NLP], dt.bfloat16, "maskn",
                          bcast=(96, g.NLP))
        eidx_sb = load(pp, eidx_d, [128, Ttot * 8], dt.int16, "eidx")
        dst_sb = load(pp, dstslot_d, [128, Ttot], dt.float32, "dstslot")
        gap_last_sb = load(pp, gap_last_d, [96, g.G // 16], dt.int16, "glast")
        maxcol_sb = load(pp, maxcol_d, [96, g.G // 16], dt.int16, "maxcol")
        Urows = load(pp, U1rows_d, [128, 4, H], dt.bfloat16, "Urows")
        tabkeep = pp.tile([128, g.NWIN, 96], dt.bfloat16, tag="tabkeep",
                          name="tabkeep")
        W_sb = {n: load(pp, W_d[n], list(W_d[n].shape), dt.bfloat16, n)
                for n in W_d}
        b_bc = {n: load(pp, b_d[n], [128, H], dt.float32, n, bcast=(128, H))
                for n in b_d}
        Wl1a_sb = load(pp, Wl1a_d, [H, H], dt.float32, "Wl1a")
        Wl1b_sb = load(pp, Wl1b_d, [H, H], dt.float32, "Wl1b")
        Wl2_sb = load(pp, Wl2_d, [H, H2], dt.float32, "Wl2")
        Wl3_sb = load(pp, Wl3_d, [H2, O], dt.float32, "Wl3")
        bl1_sb = load(pp, bl1_d, [H, 1], dt.float32, "bl1")
        bl2_sb = load(pp, bl2_d, [H2, 1], dt.float32, "bl2")
        bl3_sb = load(pp, bl3_d, [O, 1], dt.float32, "bl3")

        xbuf = [pp.tile([128, g.NLP], dt.bfloat16, tag=f"xbuf{i}", name=f"xbuf{i}")
                for i in range(2)]
        nc.sync.dma_start(out=xbuf[0][:g.F_IN, :], in_=xT_d.ap())

        scano = pp.tile([96, g.NLP], dt.float32, tag="scano", name="scano")
        gmp_all = pp.tile([96, 3 * g.G], dt.float32, tag="gmp_all", name="gmp_all")
        mg_sb = [pp.tile([96, g.G], dt.float32, tag=f"mg{i}", name=f"mg{i}")
                 for i in range(3)]
        gapar_sb = [pp.tile([96, g.G], dt.float32, tag=f"gapar{i}", name=f"gapar{i}")
                    for i in range(3)]

        zinit = wk.tile([128, Tmax, 128], dt.bfloat16, tag="gath", name="zinit")
        nc.gpsimd.memset(zinit[:, :g.NWIN, :], 0.0)
        nc.sync.dma_start(
            out=tshardA.ap().rearrange("(w p) c -> p w c", p=128),
            in_=zinit[:, :WINA, :])
        nc.sync.dma_start(
            out=tshardB.ap().rearrange("(w p) c -> p w c", p=128),
            in_=zinit[:, WINA:g.NWIN, :])

        layer_W = [("W1a", "W1b", "b1_row"), ("W2a", "W2b", "b2_row"),
                   ("W3a", "W3b", "b3_row")]

        for l in range(dbg_layers):
            F = g.F_IN if l == 0 else H
            Wa = W_sb[layer_W[l][0]]
            Wb = W_sb[layer_W[l][1]]
            bias = b_bc[layer_W[l][2]]
            x_src, x_dst = xbuf[l % 2], xbuf[(l + 1) % 2]

            # ---- conv + gather-table build ----
            for ci, (cs, cw) in enumerate(g.conv_chunks):
                ohc = wk.tile([128, 4, CONV_CHUNK], dt.bfloat16, tag="ohc",
                              name="ohc")
                nc.sync.dma_start(out=ohc[:],
                                  in_=ohconv_d.ap()[:, 4 * ci:4 * ci + 4, :])
                psc = ps_conv.tile([96, CONV_CHUNK], dt.float32, tag="psc", name="psc")
                nc.tensor.matmul(out=psc[:, :cw], lhsT=Wa[:],
                                 rhs=x_src[:F, cs:cs + cw],
                                 start=True, stop=False)
                for b in range(4):
                    nc.tensor.matmul(out=psc[:, :cw], lhsT=Urows[:, b, :],
                                     rhs=ohc[:, b, :cw], start=False,
                                     stop=b == 3)
                hT = wk.tile([96, CONV_CHUNK], dt.float32, tag="hT", name="hT")
                nc.vector.tensor_copy(out=hT[:, :cw], in_=psc[:, :cw])
                for wo in range(0, cw, 128):
                    w = (cs + wo) // 128
                    pt = ps_tr.tile([128, 128], dt.float32, tag="ptr", name="ptr")
                    nc.tensor.transpose(out=pt[:, :96],
                                        in_=hT[:, wo:wo + 128],
                                        identity=id96_sb[:])
                    nc.scalar.activation(out=tabkeep[:, w, :], in_=pt[:, :96],
                                         func=Act.Copy,
                                         scale=dinv_sbT[:, w:w + 1])
                    if w < WINA:
                        tgt = tshardA.ap()[w * 128:(w + 1) * 128, :96]
                    else:
                        wb = w - WINA
                        tgt = tshardB.ap()[wb * 128:(wb + 1) * 128, :96]
                    nc.sync.dma_start(out=tgt, in_=tabkeep[:, w, :])
                if cs + cw >= COLA and cs < COLA:
                    # windows 0..WINA-1 written: AG half A overlaps conv tail
                    nc.gpsimd.collective_compute(
                        "AllGather", Alu.bypass, replica_groups=rg,
                        ins=[tshardA.ap()], outs=[tableA.ap()])

            if dbg_stage == "conv":
                break
            # ---- all-gather table half B ----
            nc.gpsimd.collective_compute(
                "AllGather", Alu.bypass, replica_groups=rg,
                ins=[tshardB.ap()], outs=[tableB.ap()])

            if dbg_stage == "ag":
                break
            # ---- edge aggregation ----
            t_base = 0
            for (w0, nw, Tlo, Thi, rng_l, rng_h) in chunk_plan:
                T = Tlo + Thi
                gath = wk.tile([128, Tmax, 128], dt.bfloat16, tag="gath", name="gath")
                # dma_gather is limited to 1024 indices (8 tiles) per call
                for (goff, tstart, cnt, hi) in (
                        (0, t_base, Tlo, False),
                        (Tlo, t_base + Tlo, Thi, True)):
                    if cnt == 0:
                        continue
                    tbl_ap = tableB.ap() if hi else tableA.ap()
                    done = 0
                    while done < cnt:
                        nt = min(8, cnt - done)
                        nc.gpsimd.dma_gather(
                            gath[:, goff + done:goff + done + nt, :],
                            tbl_ap,
                            eidx_sb[:, 8 * (tstart + done):
                                    8 * (tstart + done + nt)],
                            num_idxs=nt * 128, num_idxs_reg=nt * 128,
                            elem_size=128)
                        done += nt
                if dbg_agg == "gather":
                    t_base += T
                    continue
                # per window: masked one-hot matmuls over its tile ranges
                blocks = []   # (wi, gath offset of first tile, ntiles)
                for wi in range(nw):
                    if rng_l[wi] is not None:
                        blocks.append((wi, rng_l[wi][0],
                                       rng_l[wi][1] - rng_l[wi][0] + 1))
                    if rng_h[wi] is not None:
                        blocks.append((wi, Tlo + rng_h[wi][0],
                                       rng_h[wi][1] - rng_h[wi][0] + 1))
                pags = [ps_agg.tile([96, 128], dt.float32, tag="pag", name="pag")
                        for _ in range(nw)]
                remaining = [0] * nw
                for (wi, _, Tb) in blocks:
                    remaining[wi] += Tb
                for wi in range(nw):
                    # self-loop term: psum += table rows of this window
                    nc.tensor.matmul(out=pags[wi][:],
                                     lhsT=tabkeep[:, w0 + wi, :],
                                     rhs=id128b_sb[:],
                                     start=True, stop=remaining[wi] == 0)
                for (wi, toff, Tb) in blocks:
                    oh = wk.tile([128, Tblk, 128], dt.bfloat16, tag="oh",
                                 name="oh", bufs=4)
                    a = t_base + toff
                    nc.vector.tensor_tensor(
                        out=oh[:, 0:Tb, :],
                        in0=dst_sb[:, a:a + Tb, None]
                            .to_broadcast((128, Tb, 128)),
                        in1=iota_sb[:, None, wi * 128:(wi + 1) * 128]
                            .to_broadcast((128, Tb, 128)),
                        op=Alu.is_equal)
                    for t in range(Tb):
                        if dbg_agg == "onehot":
                            continue
                        remaining[wi] -= 1
                        nc.tensor.matmul(out=pags[wi][:],
                                         lhsT=gath[:, toff + t, :96],
                                         rhs=oh[:, t, :],
                                         start=False,
                                         stop=remaining[wi] == 0)

                for wi in range(nw):
                    if dbg_agg in ("onehot", "mm"):
                        break
                    w = w0 + wi
                    sb1 = ep.tile([96, 128], dt.float32, tag="ep1", name="ep1")
                    nc.vector.tensor_copy(out=sb1[:], in_=pags[wi][:])
                    pt2 = ps_tr.tile([128, 128], dt.float32, tag="ptr", name="ptr")
                    nc.tensor.transpose(out=pt2[:, :96], in_=sb1[:],
                                        identity=id96_sb[:])
                    sb2 = ep.tile([128, 96], dt.float32, tag="ep2", name="ep2")
                    nc.scalar.activation(out=sb2[:], in_=pt2[:, :96],
                                         func=Act.Copy,
                                         scale=dinv_sbT[:, w:w + 1])
                    sb3 = ep.tile([128, 96], dt.bfloat16, tag="ep3", name="ep3")
                    if l < 2:
                        tmp = ep.tile([128, 96], dt.float32, tag="ep3f", name="ep3f")
                        nc.vector.tensor_add(out=tmp[:], in0=sb2[:],
                                             in1=bias[:, :96])
                        nc.vector.tensor_scalar_max(out=sb3[:], in0=tmp[:],
                                                    scalar1=0.0)
                    else:
                        nc.vector.tensor_add(out=sb3[:], in0=sb2[:],
                                             in1=bias[:, :96])
                    pt3 = ps_tr.tile([128, 128], dt.bfloat16, tag="ptr", name="ptr")
                    nc.tensor.transpose(out=pt3[:96, :], in_=sb3[:],
                                        identity=id128b_sb[:])
                    nc.vector.tensor_copy(
                        out=x_dst[:96, w * 128:(w + 1) * 128],
                        in_=pt3[:96, :])
                t_base += T

            nc.gpsimd.memset(x_dst[:96, g.NL:g.NLP], PAD_VAL)

            if dbg_stage == "agg":
                break
            # ---- pooling ----
            nc.vector.memset(scano[:, 0:1], 0.0)
            nc.vector.tensor_tensor_scan(
                out=scano[:, 1:g.NL + 1], data0=x_dst[:96, 0:g.NL],
                data1=x_dst[:96, 0:g.NL],
                initial=0.0, op0=Alu.add, op1=Alu.bypass)
            cum_l = ep.tile([96, g.G + 1], dt.float32, tag="cuml", name="cuml",
                            bufs=1)
            nc.vector.memset(cum_l[:, 0:1], 0.0)
            nc.gpsimd.ap_gather(cum_l[:, 1:g.G + 1], scano[:], gap_last_sb[:],
                                channels=96, num_elems=g.NLP, d=1,
                                num_idxs=g.G)
            gaps = ep.tile([96, g.G], dt.float32, tag="gaps", name="gaps", bufs=1)
            nc.vector.tensor_tensor(out=gaps[:], in0=cum_l[:, 1:g.G + 1],
                                    in1=cum_l[:, 0:g.G],
                                    op=Alu.subtract)
            nc.sync.dma_start(out=gap_in[l].ap(), in_=gaps[:])
            nc.gpsimd.collective_compute(
                "AllReduce", Alu.add, replica_groups=rg,
                ins=[gap_in[l].ap()], outs=[gap_out[l].ap()])
            nc.sync.dma_start(out=gapar_sb[l][:], in_=gap_out[l].ap())

            nc.vector.tensor_tensor_scan(
                out=scano[:], data0=maskneg_sb[:], data1=x_dst[:96, :],
                initial=0.0, op0=Alu.add, op1=Alu.max)
            nc.gpsimd.ap_gather(gmp_all[:, l * g.G:(l + 1) * g.G],
                                scano[:], maxcol_sb[:],
                                channels=96, num_elems=g.NLP, d=1,
                                num_idxs=g.G)
            # per-layer gmp AllReduce: layers 0-1 overlap the next layer's
            # edge gathers; only layer 2's is exposed
            nc.sync.dma_start(out=gmp_in[l].ap(),
                              in_=gmp_all[:, l * g.G:(l + 1) * g.G])
            nc.gpsimd.collective_compute(
                "AllReduce", Alu.max, replica_groups=rg,
                ins=[gmp_in[l].ap()], outs=[gmp_out[l].ap()])

            nc.vector.tensor_mul(out=mg_sb[l][:], in0=gapar_sb[l][:],
                                 in1=invc_bc[:])

            if l < 2:
                # build Urows for next layer: U = Wb_{l+1}^T @ mg,
                # transposed into [128 graph, 4 block, 96] for the conv
                Wbn = W_sb[layer_W[l + 1][1]]
                mgb = ep.tile([96, g.G], dt.bfloat16, tag="mgb", name="mgb",
                              bufs=1)
                nc.scalar.copy(out=mgb[:], in_=mg_sb[l][:])
                psU = ps_conv.tile([96, g.G], dt.float32, tag="psc", name="psc")
                nc.tensor.matmul(out=psU[:], lhsT=Wbn[:], rhs=mgb[:],
                                 start=True, stop=True)
                Usb = ep.tile([96, g.G], dt.bfloat16, tag="Usb", name="Usb",
                              bufs=1)
                nc.vector.tensor_copy(out=Usb[:], in_=psU[:])
                for b in range(4):
                    ptU = ps_tr.tile([128, 128], dt.bfloat16, tag="ptr",
                                     name="ptr")
                    nc.tensor.transpose(out=ptU[:, :96],
                                        in_=Usb[:, b * 128:(b + 1) * 128],
                                        identity=id128b_sb[:96, :96])
                    nc.scalar.copy(out=Urows[:, b, :], in_=ptU[:, :96])

        debug_cut = dbg_stage != "full" or dbg_layers < 3
        if debug_cut:
            nc.gpsimd.dma_start(out=out_d.ap(), in_=xbuf[0][:O, :g.G])
        # ---- final readout MLP (f32) ----
        if not debug_cut:
            gmpar = pp.tile([96, 3 * g.G], dt.float32, tag="gmpar", name="gmpar")
            for i in range(3):
                nc.sync.dma_start(out=gmpar[:, i * g.G:(i + 1) * g.G],
                                  in_=gmp_out[i].ap())

            hTa = pp.tile([96, g.G], dt.float32, tag="hTa", name="hTa")
            hTb = pp.tile([96, g.G], dt.float32, tag="hTb", name="hTb")
            nc.vector.tensor_add(out=hTa[:], in0=gmpar[:, 0:g.G],
                                 in1=gmpar[:, g.G:2 * g.G])
            nc.vector.tensor_add(out=hTa[:], in0=hTa[:],
                                 in1=gmpar[:, 2 * g.G:3 * g.G])
            nc.vector.tensor_add(out=hTb[:], in0=mg_sb[0][:], in1=mg_sb[1][:])
            nc.vector.tensor_add(out=hTb[:], in0=hTb[:], in1=mg_sb[2][:])

            ps1 = ps_conv.tile([96, g.G], dt.float32, tag="psc", name="psc")
            nc.tensor.matmul(out=ps1[:], lhsT=Wl1a_sb[:], rhs=hTa[:],
                             start=True, stop=False)
            nc.tensor.matmul(out=ps1[:], lhsT=Wl1b_sb[:], rhs=hTb[:],
                             start=False, stop=True)
            o1 = pp.tile([96, g.G], dt.float32, tag="o1", name="o1")
            nc.scalar.activation(out=o1[:], in_=ps1[:], func=Act.Relu,
                                 bias=bl1_sb[:])
            ps2 = ps_conv.tile([96, g.G], dt.float32, tag="psc", name="psc")
            nc.tensor.matmul(out=ps2[:H2, :], lhsT=Wl2_sb[:], rhs=o1[:],
                             start=True, stop=True)
            o2 = pp.tile([H2, g.G], dt.float32, tag="o2", name="o2")
            nc.scalar.activation(out=o2[:], in_=ps2[:H2, :], func=Act.Relu,
                                 bias=bl2_sb[:])
            ps3 = ps_conv.tile([96, g.G], dt.float32, tag="psc", name="psc")
            nc.tensor.matmul(out=ps3[:O, :], lhsT=Wl3_sb[:], rhs=o2[:],
                             start=True, stop=True)
            o3 = pp.tile([O, g.G], dt.float32, tag="o3", name="o3")
            nc.scalar.activation(out=o3[:], in_=ps3[:O, :], func=Act.Identity,
                                 bias=bl3_sb[:])
            nc.sync.dma_start(out=out_d.ap(), in_=o3[:])

        stk.close()

    nc.compile()
    return nc


_CACHE = {}


def _get_program(geo, meta, n_cores):
    key = (repr(sorted(geo.__dict__.items(), key=str)),
           repr(meta["chunk_plan"]), n_cores)
    if key not in _CACHE:
        _CACHE[key] = build_program(geo, meta, n_cores)
    return _CACHE[key]


def kernel(**inputs):
    from concourse.bass_utils import run_bass_kernel_spmd

    geo = Geo(CFG)
    inputs = {k: np.asarray(v) for k, v in inputs.items()}
    per_core, meta = prep(geo, **inputs)
    nc = _get_program(geo, meta, geo.C)
    res = run_bass_kernel_spmd(nc, per_core, core_ids=list(range(geo.C)))
    out = np.asarray(res.results[0]["out"], f32)   # [OUT, G]
    return np.ascontiguousarray(out.T)             # [G, OUT] float32



# revision 42
# speedup vs baseline: 1.2883x; 1.0012x over previous
"""Trainium2 Bass kernel for nn_BenchGNN_29300266893894 (3-layer GCN with
global-feature concat + global mean/max pooling readout + MLP head).

Self-contained: host-side sharding/packing prep + SPMD Bass/Tile program on
8 NeuronCores via run_bass_kernel_spmd.

Sharding: nodes are split into 8 contiguous shards of 6250 (padded to 6272 =
49*128). Edges are partitioned by dst owner and sorted by dst window; weight
matrices are replicated. Per layer, each core computes h = x_cat @ W for its
nodes (the global-mean term enters as U = Wb^T @ mean matmul'd against
static one-hot graph-block matrices - no gather), builds a bf16 table of
h' = h*dinv rows (256 B each, kept in SBUF as tabkeep AND written to DRAM
in two halves whose AllGathers overlap the conv tail), then aggregates
edges with dma_gather + masked one-hot matmuls accumulating per-window PSUM
tiles; edge tiles are packed at 4-window-chunk granularity and self-loops
are added via one identity matmul per window from tabkeep. Graph pooling
uses DVE scans + one boundary ap_gather per pool (gap via shift-subtract)
with per-layer AllReduces overlapped into the next layer's gather stream.
"""
import sys
import numpy as np
import ml_dtypes

sys.path.insert(0, "/opt/trn_rl_repo")

bf16 = ml_dtypes.bfloat16
f32 = np.float32

# ---------------- problem geometry (hardcoded) ----------------
CFG = dict(N=50000, E=800000, G=512, F_IN=128, HID=96, OUT=10, C=8)

WIN = 128
WINA = 25            # windows per core in table half A
COLA = WINA * WIN    # 3200
CHUNK_WINDOWS = 4
CONV_CHUNK = 512
PAD_SLOT = 999.0
PAD_VAL = -60000.0


class Geo:
    def __init__(self, cfg):
        self.__dict__.update(cfg)
        assert self.N % self.C == 0
        self.NL = self.N // self.C
        self.NWIN = -(-self.NL // WIN)
        self.NLP = self.NWIN * WIN
        # pooling pad-column tricks need at least one pad column per core
        assert self.NLP > self.NL
        self.NP = self.NLP * self.C
        self.conv_chunks = [(o, min(CONV_CHUNK, self.NLP - o))
                            for o in range(0, self.NLP, CONV_CHUNK)]


def _wrap16(idx, channels):
    idx = np.asarray(idx, np.int16)
    assert len(idx) % 16 == 0
    idx16 = idx.reshape(-1, 16).T
    return np.ascontiguousarray(np.tile(idx16, (channels // 16, 1)))


def prep(geo, x, edge_index, batch, W1, b1, W2, b2, W3, b3,
         Wl1, bl1, Wl2, bl2, Wl3, bl3):
    g = geo
    x = np.asarray(x, f32)
    src = np.asarray(edge_index[0], np.int64)
    dst = np.asarray(edge_index[1], np.int64)
    batch = np.asarray(batch, np.int64)

    def gid_of(n):
        return (n // g.NL) * g.NLP + (n % g.NL)

    deg = np.bincount(dst, minlength=g.N).astype(f32) + 1.0
    dinv = (1.0 / np.sqrt(deg)).astype(f32)
    counts = np.bincount(batch, minlength=g.G).astype(f32)
    invc = (1.0 / np.maximum(counts, 1.0)).astype(f32)

    sums1 = np.zeros((g.G, g.F_IN), f32)
    np.add.at(sums1, batch, x)
    mean1 = (sums1 * invc[:, None]).astype(f32)          # [G, F_IN]
    U1rows = (mean1 @ np.asarray(W1[g.F_IN:], f32))      # [G, HID]

    src_g = gid_of(src)

    core_win = []
    for k in range(g.C):
        lo_n, hi_n = k * g.NL, (k + 1) * g.NL
        sel = (dst >= lo_n) & (dst < hi_n)
        es, ed = src_g[sel], (dst[sel] - lo_n)
        order = np.argsort(ed, kind="stable")
        es, ed = es[order], ed[order]
        wstart = np.searchsorted(ed, np.arange(0, g.NLP + 1, WIN))
        wins = []
        COLB = g.NLP - COLA
        for w in range(g.NWIN):
            a, b = wstart[w], wstart[w + 1]
            ws, wd = es[a:b], ed[a:b] - w * WIN
            c_, off = ws // g.NLP, ws % g.NLP
            a_m = off < COLA
            wins.append((c_[a_m] * COLA + off[a_m], wd[a_m],
                         c_[~a_m] * COLB + (off[~a_m] - COLA), wd[~a_m]))
        core_win.append(wins)

    # chunk plan: per chunk of CHUNK_WINDOWS windows, edges of all windows are
    # packed contiguously (chunk-local slot ids); tile counts are chunk-level
    # cross-core maxima, and each window gets a [t0, t1] tile range (union
    # over cores) whose tiles feed its psum via masked one-hot matmuls.
    def _ranges(lens_kw, nw):
        out = []
        for wi in range(nw):
            t0s, t1s = [], []
            for k in range(g.C):
                s = sum(lens_kw[k][:wi])
                L = lens_kw[k][wi]
                if L:
                    t0s.append(s // 128)
                    t1s.append((s + L - 1) // 128)
            out.append((min(t0s), max(t1s)) if t0s else None)
        return out

    chunk_plan = []   # (w0, nw, TC_lo, TC_hi, rng_lo, rng_hi)
    for w0 in range(0, g.NWIN, CHUNK_WINDOWS):
        nw = min(CHUNK_WINDOWS, g.NWIN - w0)
        lens_lo = [[len(core_win[k][w0 + wi][0]) for wi in range(nw)]
                   for k in range(g.C)]
        lens_hi = [[len(core_win[k][w0 + wi][2]) for wi in range(nw)]
                   for k in range(g.C)]
        TC_lo = -(-max(sum(r) for r in lens_lo) // 128)
        TC_hi = -(-max(sum(r) for r in lens_hi) // 128)
        chunk_plan.append((w0, nw, TC_lo, TC_hi,
                           _ranges(lens_lo, nw), _ranges(lens_hi, nw)))

    per_core = []
    for k in range(g.C):
        lo_n, hi_n = k * g.NL, (k + 1) * g.NL
        idx_list, slot_list = [], []
        for (w0, nw, TCl, TCh, _, _) in chunk_plan:
            for half, TC in ((0, TCl), (2, TCh)):
                s_arr = np.concatenate(
                    [core_win[k][w0 + wi][half] for wi in range(nw)]
                    + [np.zeros(0, np.int64)])
                d_arr = np.concatenate(
                    [core_win[k][w0 + wi][half + 1] + 128 * wi
                     for wi in range(nw)] + [np.zeros(0, np.int64)])
                pad = TC * 128 - len(s_arr)
                idx_list.append(np.concatenate(
                    [s_arr, np.zeros(pad, np.int64)]))
                slot_list.append(np.concatenate(
                    [d_arr, np.full(pad, PAD_SLOT)]))
        idx_all = np.concatenate(idx_list)
        slot_all = np.concatenate(slot_list).astype(f32)
        idx_sb = _wrap16(idx_all, 128)
        dst_sb = np.ascontiguousarray(
            slot_all.reshape(-1, 128).T.astype(f32))

        bl = batch[lo_n:hi_n]
        # one-hot graph-block matrices for the conv global-feature term:
        # ohconv[p, c, b, j] = 1{batch_local[cs_c + j] == 128*b + p}
        nch = len(g.conv_chunks)
        ohconv = np.zeros((128, nch, 4, CONV_CHUNK), bf16)
        for c, (cs, cw) in enumerate(g.conv_chunks):
            seg = bl[cs:min(cs + cw, g.NL)]
            for j, gid in enumerate(seg):
                ohconv[gid % 128, c, gid // 128, j] = 1
        bcol = np.searchsorted(bl, np.arange(g.G), side="right") - 1
        gap_last = _wrap16(np.where(bcol < 0, 0, bcol + 1), 96)
        present = np.zeros(g.G, bool)
        present[np.unique(bl)] = True
        maxcol = _wrap16(np.where(present, bcol, g.NLP - 1), 96)

        gstart = np.searchsorted(bl, np.arange(g.G), side="left")
        maskneg = np.zeros(g.NLP, f32)
        maskneg[np.unique(gstart[present])] = -1e30
        maskneg[g.NL] = -1e30
        maskneg_row = maskneg.reshape(1, g.NLP).astype(bf16)

        xT = np.zeros((g.F_IN, g.NLP), f32)
        xT[:, :g.NL] = x[lo_n:hi_n].T
        dinv_loc = np.zeros(g.NLP, f32)
        dinv_loc[:g.NL] = dinv[lo_n:hi_n]
        dinv_sb = np.ascontiguousarray(dinv_loc.reshape(g.NWIN, WIN).T)

        H, H2, O = g.HID, g.HID // 2, g.OUT
        inp = {
            "xT": xT.astype(bf16),
            "U1rows": np.ascontiguousarray(
                U1rows.reshape(4, 128, H).transpose(1, 0, 2)).astype(bf16),
            "ohconv": np.ascontiguousarray(
                ohconv.reshape(128, 4 * nch, CONV_CHUNK)),
            "gap_last": gap_last, "maxcol": maxcol,
            "maskneg_row": maskneg_row,
            "eidx": idx_sb,
            "dstslot": dst_sb,
            "dinv_sb": dinv_sb,
            "iota512": np.tile(np.arange(CHUNK_WINDOWS * WIN, dtype=f32),
                               (128, 1)),
            "id96": np.eye(96, dtype=f32),
            "id128b": np.eye(128, dtype=bf16),
            "invc_row": invc.reshape(1, g.G),
            "W1a": np.asarray(W1[:g.F_IN], bf16),
            "W1b": np.asarray(W1[g.F_IN:], bf16),
            "W2a": np.asarray(W2[:H], bf16), "W2b": np.asarray(W2[H:], bf16),
            "W3a": np.asarray(W3[:H], bf16), "W3b": np.asarray(W3[H:], bf16),
            "b1_row": np.asarray(b1, f32).reshape(1, H),
            "b2_row": np.asarray(b2, f32).reshape(1, H),
            "b3_row": np.asarray(b3, f32).reshape(1, H),
            "Wl1a": np.asarray(Wl1[:H], f32), "Wl1b": np.asarray(Wl1[H:], f32),
            "Wl2": np.asarray(Wl2, f32), "Wl3": np.asarray(Wl3, f32),
            "bl1": np.asarray(bl1, f32).reshape(H, 1),
            "bl2": np.asarray(bl2, f32).reshape(H2, 1),
            "bl3": np.asarray(bl3, f32).reshape(O, 1),
        }
        per_core.append(inp)

    meta = {"chunk_plan": chunk_plan,
            "Ttot": sum(c[2] + c[3] for c in chunk_plan)}
    return per_core, meta


# ---------------- device program ----------------


def build_program(geo, meta, n_cores):
    import os
    dbg_layers = int(os.environ.get("KN_LAYERS", "3"))
    dbg_stage = os.environ.get("KN_STAGE", "full")
    dbg_agg = os.environ.get("KN_AGG", "ep")
    import concourse.bacc as bacc
    import concourse.bass as bass
    import concourse.mybir as mybir
    import concourse.tile as tile

    g = geo
    H, H2, O = g.HID, g.HID // 2, g.OUT
    dt = mybir.dt
    Alu = mybir.AluOpType
    Act = mybir.ActivationFunctionType
    chunk_plan = meta["chunk_plan"]
    Ttot = meta["Ttot"]
    Tmax = max(max(c[2] + c[3] for c in chunk_plan), g.NWIN)
    Tblk = max(r[1] - r[0] + 1
               for (_, _, _, _, rl, rh) in chunk_plan
               for r in rl + rh if r is not None)

    nc = bacc.Bacc("TRN2", target_bir_lowering=False, debug=False,
                   num_devices=n_cores)
    rg = [list(range(n_cores))]

    def din(name, shape, dtype):
        return nc.dram_tensor(name, list(shape), dtype, kind="ExternalInput")

    xT_d = din("xT", [g.F_IN, g.NLP], dt.bfloat16)
    U1rows_d = din("U1rows", [128, 4, H], dt.bfloat16)
    ohconv_d = din("ohconv", [128, 4 * len(g.conv_chunks), CONV_CHUNK],
                   dt.bfloat16)
    gap_last_d = din("gap_last", [96, g.G // 16], dt.int16)
    maxcol_d = din("maxcol", [96, g.G // 16], dt.int16)
    maskneg_d = din("maskneg_row", [1, g.NLP], dt.bfloat16)
    eidx_d = din("eidx", [128, Ttot * 8], dt.int16)
    dstslot_d = din("dstslot", [128, Ttot], dt.float32)
    dinv_d = din("dinv_sb", [128, g.NWIN], dt.float32)
    iota_d = din("iota512", [128, CHUNK_WINDOWS * WIN], dt.float32)
    id96_d = din("id96", [96, 96], dt.float32)
    id128b_d = din("id128b", [128, 128], dt.bfloat16)
    invc_d = din("invc_row", [1, g.G], dt.float32)
    W_d = {n: din(n, [g.F_IN if n[1] == "1" else H, H], dt.bfloat16)
           for n in ("W1a", "W1b", "W2a", "W2b", "W3a", "W3b")}
    b_d = {n: din(n, [1, H], dt.float32)
           for n in ("b1_row", "b2_row", "b3_row")}
    Wl1a_d = din("Wl1a", [H, H], dt.float32)
    Wl1b_d = din("Wl1b", [H, H], dt.float32)
    Wl2_d = din("Wl2", [H, H2], dt.float32)
    Wl3_d = din("Wl3", [H2, O], dt.float32)
    bl1_d = din("bl1", [H, 1], dt.float32)
    bl2_d = din("bl2", [H2, 1], dt.float32)
    bl3_d = din("bl3", [O, 1], dt.float32)

    out_d = nc.dram_tensor("out", [O, g.G], dt.float32, kind="ExternalOutput")

    # internal DRAM (table split in two halves so AG-A overlaps conv tail)
    COLB = g.NLP - COLA
    tshardA = nc.dram_tensor("tshardA", [COLA, 128], dt.bfloat16,
                             kind="Internal")
    tshardB = nc.dram_tensor("tshardB", [COLB, 128], dt.bfloat16,
                             kind="Internal")
    tableA = nc.dram_tensor("tableA", [g.C * COLA, 128], dt.bfloat16,
                            kind="Internal", addr_space="Shared")
    tableB = nc.dram_tensor("tableB", [g.C * COLB, 128], dt.bfloat16,
                            kind="Internal", addr_space="Shared")
    gap_in = [nc.dram_tensor(f"gap_in{i}", [96, g.G], dt.float32,
                             kind="Internal") for i in range(3)]
    gap_out = [nc.dram_tensor(f"gap_out{i}", [96, g.G], dt.float32,
                              kind="Internal", addr_space="Shared")
               for i in range(3)]
    gmp_in = [nc.dram_tensor(f"gmp_in{i}", [96, g.G], dt.float32,
                             kind="Internal") for i in range(3)]
    gmp_out = [nc.dram_tensor(f"gmp_out{i}", [96, g.G], dt.float32,
                              kind="Internal", addr_space="Shared")
               for i in range(3)]

    with tile.TileContext(nc) as tc:
        import contextlib
        stk = contextlib.ExitStack()
        pp = stk.enter_context(tc.tile_pool(name="persist", bufs=1))
        wk = stk.enter_context(tc.tile_pool(name="work", bufs=2))
        ep = stk.enter_context(tc.tile_pool(name="epil", bufs=2))
        ps_conv = stk.enter_context(
            tc.tile_pool(name="ps_conv", bufs=2, space="PSUM"))
        ps_tr = stk.enter_context(
            tc.tile_pool(name="ps_tr", bufs=2, space="PSUM"))
        ps_agg = stk.enter_context(
            tc.tile_pool(name="ps_agg", bufs=4, space="PSUM"))

        def load(pool, dram, shape, dtype, tag, bcast=None):
            t = pool.tile(shape, dtype, tag=tag, name=tag)
            src = dram.ap() if bcast is None else dram.ap().to_broadcast(bcast)
            nc.sync.dma_start(out=t[:], in_=src)
            return t

        # constants
        iota_sb = load(pp, iota_d, [128, CHUNK_WINDOWS * WIN], dt.float32,
                       "iota")
        id96_sb = load(pp, id96_d, [96, 96], dt.float32, "id96")
        id128b_sb = load(pp, id128b_d, [128, 128], dt.bfloat16, "id128b")
        dinv_sbT = load(pp, dinv_d, [128, g.NWIN], dt.float32, "dinv")
        invc_bc = load(pp, invc_d, [96, g.G], dt.float32, "invc",
                       bcast=(96, g.G))
        maskneg_sb = load(pp, maskneg_d, [96, g.NLP], dt.bfloat16, "maskn",
                          bcast=(96, g.NLP))
        eidx_sb = load(pp, eidx_d, [128, Ttot * 8], dt.int16, "eidx")
        dst_sb = load(pp, dstslot_d, [128, Ttot], dt.float32, "dstslot")
        gap_last_sb = load(pp, gap_last_d, [96, g.G // 16], dt.int16, "glast")
        maxcol_sb = load(pp, maxcol_d, [96, g.G // 16], dt.int16, "maxcol")
        Urows = load(pp, U1rows_d, [128, 4, H], dt.bfloat16, "Urows")
        tabkeep = pp.tile([128, g.NWIN, 96], dt.bfloat16, tag="tabkeep",
                          name="tabkeep")
        W_sb = {n: load(pp, W_d[n], list(W_d[n].shape), dt.bfloat16, n)
                for n in W_d}
        b_bc = {n: load(pp, b_d[n], [128, H], dt.float32, n, bcast=(128, H))
                for n in b_d}
        Wl1a_sb = load(pp, Wl1a_d, [H, H], dt.float32, "Wl1a")
        Wl1b_sb = load(pp, Wl1b_d, [H, H], dt.float32, "Wl1b")
        Wl2_sb = load(pp, Wl2_d, [H, H2], dt.float32, "Wl2")
        Wl3_sb = load(pp, Wl3_d, [H2, O], dt.float32, "Wl3")
        bl1_sb = load(pp, bl1_d, [H, 1], dt.float32, "bl1")
        bl2_sb = load(pp, bl2_d, [H2, 1], dt.float32, "bl2")
        bl3_sb = load(pp, bl3_d, [O, 1], dt.float32, "bl3")

        xbuf = [pp.tile([128, g.NLP], dt.bfloat16, tag=f"xbuf{i}", name=f"xbuf{i}")
                for i in range(2)]
        nc.sync.dma_start(out=xbuf[0][:g.F_IN, :], in_=xT_d.ap())

        scano = pp.tile([96, g.NLP], dt.float32, tag="scano", name="scano")
        gmp_all = pp.tile([96, 3 * g.G], dt.float32, tag="gmp_all", name="gmp_all")
        mg_sb = [pp.tile([96, g.G], dt.float32, tag=f"mg{i}", name=f"mg{i}")
                 for i in range(3)]
        gapar_sb = [pp.tile([96, g.G], dt.float32, tag=f"gapar{i}", name=f"gapar{i}")
                    for i in range(3)]

        zinit = wk.tile([128, Tmax, 128], dt.bfloat16, tag="gath", name="zinit")
        nc.gpsimd.memset(zinit[:, :g.NWIN, :], 0.0)
        nc.sync.dma_start(
            out=tshardA.ap().rearrange("(w p) c -> p w c", p=128),
            in_=zinit[:, :WINA, :])
        nc.sync.dma_start(
            out=tshardB.ap().rearrange("(w p) c -> p w c", p=128),
            in_=zinit[:, WINA:g.NWIN, :])

        layer_W = [("W1a", "W1b", "b1_row"), ("W2a", "W2b", "b2_row"),
                   ("W3a", "W3b", "b3_row")]

        for l in range(dbg_layers):
            F = g.F_IN if l == 0 else H
            Wa = W_sb[layer_W[l][0]]
            Wb = W_sb[layer_W[l][1]]
            bias = b_bc[layer_W[l][2]]
            x_src, x_dst = xbuf[l % 2], xbuf[(l + 1) % 2]

            # ---- conv + gather-table build ----
            for ci, (cs, cw) in enumerate(g.conv_chunks):
                ohc = wk.tile([128, 4, CONV_CHUNK], dt.bfloat16, tag="ohc",
                              name="ohc")
                nc.sync.dma_start(out=ohc[:],
                                  in_=ohconv_d.ap()[:, 4 * ci:4 * ci + 4, :])
                psc = ps_conv.tile([96, CONV_CHUNK], dt.float32, tag="psc", name="psc")
                nc.tensor.matmul(out=psc[:, :cw], lhsT=Wa[:],
                                 rhs=x_src[:F, cs:cs + cw],
                                 start=True, stop=False)
                for b in range(4):
                    nc.tensor.matmul(out=psc[:, :cw], lhsT=Urows[:, b, :],
                                     rhs=ohc[:, b, :cw], start=False,
                                     stop=b == 3)
                hT = wk.tile([96, CONV_CHUNK], dt.float32, tag="hT", name="hT")
                nc.vector.tensor_copy(out=hT[:, :cw], in_=psc[:, :cw])
                for wo in range(0, cw, 128):
                    w = (cs + wo) // 128
                    pt = ps_tr.tile([128, 128], dt.float32, tag="ptr", name="ptr")
                    nc.tensor.transpose(out=pt[:, :96],
                                        in_=hT[:, wo:wo + 128],
                                        identity=id96_sb[:])
                    nc.scalar.activation(out=tabkeep[:, w, :], in_=pt[:, :96],
                                         func=Act.Copy,
                                         scale=dinv_sbT[:, w:w + 1])
                    if w < WINA:
                        tgt = tshardA.ap()[w * 128:(w + 1) * 128, :96]
                    else:
                        wb = w - WINA
                        tgt = tshardB.ap()[wb * 128:(wb + 1) * 128, :96]
                    nc.sync.dma_start(out=tgt, in_=tabkeep[:, w, :])
                if cs + cw >= COLA and cs < COLA:
                    # windows 0..WINA-1 written: AG half A overlaps conv tail
                    nc.gpsimd.collective_compute(
                        "AllGather", Alu.bypass, replica_groups=rg,
                        ins=[tshardA.ap()], outs=[tableA.ap()])

            if dbg_stage == "conv":
                break
            # ---- all-gather table half B ----
            nc.gpsimd.collective_compute(
                "AllGather", Alu.bypass, replica_groups=rg,
                ins=[tshardB.ap()], outs=[tableB.ap()])

            if dbg_stage == "ag":
                break
            # ---- edge aggregation ----
            t_base = 0
            for (w0, nw, Tlo, Thi, rng_l, rng_h) in chunk_plan:
                T = Tlo + Thi
                gath = wk.tile([128, Tmax, 128], dt.bfloat16, tag="gath", name="gath")
                # dma_gather is limited to 1024 indices (8 tiles) per call
                for (goff, tstart, cnt, hi) in (
                        (0, t_base, Tlo, False),
                        (Tlo, t_base + Tlo, Thi, True)):
                    if cnt == 0:
                        continue
                    tbl_ap = tableB.ap() if hi else tableA.ap()
                    done = 0
                    while done < cnt:
                        nt = min(8, cnt - done)
                        nc.gpsimd.dma_gather(
                            gath[:, goff + done:goff + done + nt, :],
                            tbl_ap,
                            eidx_sb[:, 8 * (tstart + done):
                                    8 * (tstart + done + nt)],
                            num_idxs=nt * 128, num_idxs_reg=nt * 128,
                            elem_size=128)
                        done += nt
                if dbg_agg == "gather":
                    t_base += T
                    continue
                # per window: masked one-hot matmuls over its tile ranges
                blocks = []   # (wi, gath offset of first tile, ntiles)
                for wi in range(nw):
                    if rng_l[wi] is not None:
                        blocks.append((wi, rng_l[wi][0],
                                       rng_l[wi][1] - rng_l[wi][0] + 1))
                    if rng_h[wi] is not None:
                        blocks.append((wi, Tlo + rng_h[wi][0],
                                       rng_h[wi][1] - rng_h[wi][0] + 1))
                pags = [ps_agg.tile([96, 128], dt.float32, tag="pag", name="pag")
                        for _ in range(nw)]
                remaining = [0] * nw
                for (wi, _, Tb) in blocks:
                    remaining[wi] += Tb
                for wi in range(nw):
                    # self-loop term: psum += table rows of this window
                    nc.tensor.matmul(out=pags[wi][:],
                                     lhsT=tabkeep[:, w0 + wi, :],
                                     rhs=id128b_sb[:],
                                     start=True, stop=remaining[wi] == 0)
                for (wi, toff, Tb) in blocks:
                    oh = wk.tile([128, Tblk, 128], dt.bfloat16, tag="oh",
                                 name="oh", bufs=4)
                    a = t_base + toff
                    nc.vector.tensor_tensor(
                        out=oh[:, 0:Tb, :],
                        in0=dst_sb[:, a:a + Tb, None]
                            .to_broadcast((128, Tb, 128)),
                        in1=iota_sb[:, None, wi * 128:(wi + 1) * 128]
                            .to_broadcast((128, Tb, 128)),
                        op=Alu.is_equal)
                    for t in range(Tb):
                        if dbg_agg == "onehot":
                            continue
                        remaining[wi] -= 1
                        nc.tensor.matmul(out=pags[wi][:],
                                         lhsT=gath[:, toff + t, :96],
                                         rhs=oh[:, t, :],
                                         start=False,
                                         stop=remaining[wi] == 0)

                for wi in range(nw):
                    if dbg_agg in ("onehot", "mm"):
                        break
                    w = w0 + wi
                    sb1 = ep.tile([96, 128], dt.float32, tag="ep1", name="ep1")
                    nc.vector.tensor_copy(out=sb1[:], in_=pags[wi][:])
                    pt2 = ps_tr.tile([128, 128], dt.float32, tag="ptr", name="ptr")
                    nc.tensor.transpose(out=pt2[:, :96], in_=sb1[:],
                                        identity=id96_sb[:])
                    sb2 = ep.tile([128, 96], dt.float32, tag="ep2", name="ep2")
                    nc.scalar.activation(out=sb2[:], in_=pt2[:, :96],
                                         func=Act.Copy,
                                         scale=dinv_sbT[:, w:w + 1])
                    sb3 = ep.tile([128, 96], dt.bfloat16, tag="ep3", name="ep3")
                    if l < 2:
                        tmp = ep.tile([128, 96], dt.float32, tag="ep3f", name="ep3f")
                        nc.vector.tensor_add(out=tmp[:], in0=sb2[:],
                                             in1=bias[:, :96])
                        nc.vector.tensor_scalar_max(out=sb3[:], in0=tmp[:],
                                                    scalar1=0.0)
                    else:
                        nc.vector.tensor_add(out=sb3[:], in0=sb2[:],
                                             in1=bias[:, :96])
                    pt3 = ps_tr.tile([128, 128], dt.bfloat16, tag="ptr", name="ptr")
                    nc.tensor.transpose(out=pt3[:96, :], in_=sb3[:],
                                        identity=id128b_sb[:])
                    nc.vector.tensor_copy(
                        out=x_dst[:96, w * 128:(w + 1) * 128],
                        in_=pt3[:96, :])
                t_base += T

            nc.gpsimd.memset(x_dst[:96, g.NL:g.NLP], PAD_VAL)

            if dbg_stage == "agg":
                break
            # ---- pooling ----
            nc.vector.memset(scano[:, 0:1], 0.0)
            nc.vector.tensor_tensor_scan(
                out=scano[:, 1:g.NL + 1], data0=x_dst[:96, 0:g.NL],
                data1=x_dst[:96, 0:g.NL],
                initial=0.0, op0=Alu.add, op1=Alu.bypass)
            cum_l = ep.tile([96, g.G + 1], dt.float32, tag="cuml", name="cuml",
                            bufs=1)
            nc.vector.memset(cum_l[:, 0:1], 0.0)
            nc.gpsimd.ap_gather(cum_l[:, 1:g.G + 1], scano[:], gap_last_sb[:],
                                channels=96, num_elems=g.NLP, d=1,
                                num_idxs=g.G)
            gaps = ep.tile([96, g.G], dt.float32, tag="gaps", name="gaps", bufs=1)
            nc.vector.tensor_tensor(out=gaps[:], in0=cum_l[:, 1:g.G + 1],
                                    in1=cum_l[:, 0:g.G],
                                    op=Alu.subtract)
            nc.sync.dma_start(out=gap_in[l].ap(), in_=gaps[:])
            nc.gpsimd.collective_compute(
                "AllReduce", Alu.add, replica_groups=rg,
                ins=[gap_in[l].ap()], outs=[gap_out[l].ap()])
            nc.sync.dma_start(out=gapar_sb[l][:], in_=gap_out[l].ap())

            nc.vector.tensor_tensor_scan(
                out=scano[:], data0=maskneg_sb[:], data1=x_dst[:96, :],
                initial=0.0, op0=Alu.add, op1=Alu.max)
            nc.gpsimd.ap_gather(gmp_all[:, l * g.G:(l + 1) * g.G],
                                scano[:], maxcol_sb[:],
                                channels=96, num_elems=g.NLP, d=1,
                                num_idxs=g.G)
            # per-layer gmp AllReduce: layers 0-1 overlap the next layer's
            # edge gathers; only layer 2's is exposed
            nc.sync.dma_start(out=gmp_in[l].ap(),
                              in_=gmp_all[:, l * g.G:(l + 1) * g.G])
            nc.gpsimd.collective_compute(
                "AllReduce", Alu.max, replica_groups=rg,
                ins=[gmp_in[l].ap()], outs=[gmp_out[l].ap()])

            nc.vector.tensor_mul(out=mg_sb[l][:], in0=gapar_sb[l][:],
                                 in1=invc_bc[:])

            if l < 2:
                # build Urows for next layer: U = Wb_{l+1}^T @ mg,
                # transposed into [128 graph, 4 block, 96] for the conv
                Wbn = W_sb[layer_W[l + 1][1]]
                mgb = ep.tile([96, g.G], dt.bfloat16, tag="mgb", name="mgb",
                              bufs=1)
                nc.scalar.copy(out=mgb[:], in_=mg_sb[l][:])
                psU = ps_conv.tile([96, g.G], dt.float32, tag="psc", name="psc")
                nc.tensor.matmul(out=psU[:], lhsT=Wbn[:], rhs=mgb[:],
                                 start=True, stop=True)
                Usb = ep.tile([96, g.G], dt.bfloat16, tag="Usb", name="Usb",
                              bufs=1)
                nc.vector.tensor_copy(out=Usb[:], in_=psU[:])
                for b in range(4):
                    ptU = ps_tr.tile([128, 128], dt.bfloat16, tag="ptr",
                                     name="ptr")
                    nc.tensor.transpose(out=ptU[:, :96],
                                        in_=Usb[:, b * 128:(b + 1) * 128],
                                        identity=id128b_sb[:96, :96])
                    nc.scalar.copy(out=Urows[:, b, :], in_=ptU[:, :96])

        debug_cut = dbg_stage != "full" or dbg_layers < 3
        if debug_cut:
            nc.gpsimd.dma_start(out=out_d.ap(), in_=xbuf[0][:O, :g.G])
        # ---- final readout MLP (f32) ----
        if not debug_cut:
            gmpar = pp.tile([96, 3 * g.G], dt.float32, tag="gmpar", name="gmpar")
            for i in range(3):
                nc.sync.dma_start(out=gmpar[:, i * g.G:(i + 1) * g.G],
                                  in_=gmp_out[i].ap())

            hTa = pp.tile([96, g.G], dt.float32, tag="hTa", name="hTa")
            hTb = pp.tile([96, g.G], dt.float32, tag="hTb", name="hTb")
            nc.vector.tensor_add(out=hTa[:], in0=gmpar[:, 0:g.G],
                                 in1=gmpar[:, g.G:2 * g.G])
            nc.vector.tensor_add(out=hTa[:], in0=hTa[:],
                                 in1=gmpar[:, 2 * g.G:3 * g.G])
            nc.vector.tensor_add(out=hTb[:], in0=mg_sb[0][:], in1=mg_sb[1][:])
            nc.vector.tensor_add(out=hTb[:], in0=hTb[:], in1=mg_sb[2][:])

            ps1 = ps_conv.tile([96, g.G], dt.float32, tag="psc", name="psc")
            nc.tensor.matmul(out=ps1[:], lhsT=Wl1a_sb[:], rhs=hTa[:],
                             start=True, stop=False)
            nc.tensor.matmul(out=ps1[:], lhsT=Wl1b_sb[:], rhs=hTb[:],
                             start=False, stop=True)
            o1 = pp.tile([96, g.G], dt.float32, tag="o1", name="o1")
            nc.scalar.activation(out=o1[:], in_=ps1[:], func=Act.Relu,
                                 bias=bl1_sb[:])
            ps2 = ps_conv.tile([96, g.G], dt.float32, tag="psc", name="psc")
            nc.tensor.matmul(out=ps2[:H2, :], lhsT=Wl2_sb[:], rhs=o1[:],
                             start=True, stop=True)
            o2 = pp.tile([H2, g.G], dt.float32, tag="o2", name="o2")
            nc.scalar.activation(out=o2[:], in_=ps2[:H2, :], func=Act.Relu,
                                 bias=bl2_sb[:])
            ps3 = ps_conv.tile([96, g.G], dt.float32, tag="psc", name="psc")
            nc.tensor.matmul(out=ps3[:O, :], lhsT=Wl3_sb[:], rhs=o2[:],
                             start=True, stop=True)
            o3 = pp.tile([O, g.G], dt.float32, tag="o3", name="o3")
            nc.scalar.activation(out=o3[:], in_=ps3[:O, :], func=Act.Identity,
                                 bias=bl3_sb[:])
            nc.sync.dma_start(out=out_d.ap(), in_=o3[:])

        stk.close()

    nc.compile()
    return nc


_CACHE = {}


def _get_program(geo, meta, n_cores):
    key = (repr(sorted(geo.__dict__.items(), key=str)),
           repr(meta["chunk_plan"]), n_cores)
    if key not in _CACHE:
        _CACHE[key] = build_program(geo, meta, n_cores)
    return _CACHE[key]


def kernel(**inputs):
    from concourse.bass_utils import run_bass_kernel_spmd

    geo = Geo(CFG)
    inputs = {k: np.asarray(v) for k, v in inputs.items()}
    per_core, meta = prep(geo, **inputs)
    nc = _get_program(geo, meta, geo.C)
    res = run_bass_kernel_spmd(nc, per_core, core_ids=list(range(geo.C)))
    out = np.asarray(res.results[0]["out"], f32)   # [OUT, G]
    return np.ascontiguousarray(out.T)             # [G, OUT] float32



# revision 48
# speedup vs baseline: 1.2942x; 1.0046x over previous
"""Trainium2 Bass kernel for nn_BenchGNN_29300266893894 (3-layer GCN with
global-feature concat + global mean/max pooling readout + MLP head).

Self-contained: host-side sharding/packing prep + SPMD Bass/Tile program on
8 NeuronCores via run_bass_kernel_spmd.

Sharding: nodes are split into 8 contiguous shards of 6250 (padded to 6272 =
49*128). Edges are partitioned by dst owner and sorted by dst window; weight
matrices are replicated. Per layer, each core computes h = x_cat @ W for its
nodes (the global-mean term enters as U = Wb^T @ mean matmul'd against
static one-hot graph-block matrices - no gather), builds a bf16 table of
h' = h*dinv rows (256 B each, kept in SBUF as tabkeep AND written to DRAM
in two halves whose AllGathers overlap the conv tail), then aggregates
edges with dma_gather + masked one-hot matmuls accumulating per-window PSUM
tiles; edge tiles are packed at 4-window-chunk granularity and self-loops
are added via one identity matmul per window from tabkeep. Graph pooling
uses DVE scans + one boundary ap_gather per pool (gap via shift-subtract)
with per-layer AllReduces overlapped into the next layer's gather stream.
"""
import sys
import numpy as np
import ml_dtypes

sys.path.insert(0, "/opt/trn_rl_repo")

bf16 = ml_dtypes.bfloat16
f32 = np.float32

# ---------------- problem geometry (hardcoded) ----------------
CFG = dict(N=50000, E=800000, G=512, F_IN=128, HID=96, OUT=10, C=8)

WIN = 128
WINA = 25            # windows per core in table half A
COLA = WINA * WIN    # 3200
CHUNK_WINDOWS = 4
CONV_CHUNK = 512
PAD_SLOT = 999.0
PAD_VAL = -60000.0


class Geo:
    def __init__(self, cfg):
        self.__dict__.update(cfg)
        assert self.N % self.C == 0
        self.NL = self.N // self.C
        self.NWIN = -(-self.NL // WIN)
        self.NLP = self.NWIN * WIN
        # pooling pad-column tricks need at least one pad column per core
        assert self.NLP > self.NL
        self.NP = self.NLP * self.C
        self.conv_chunks = [(o, min(CONV_CHUNK, self.NLP - o))
                            for o in range(0, self.NLP, CONV_CHUNK)]


def _wrap16(idx, channels):
    idx = np.asarray(idx, np.int16)
    assert len(idx) % 16 == 0
    idx16 = idx.reshape(-1, 16).T
    return np.ascontiguousarray(np.tile(idx16, (channels // 16, 1)))


def prep(geo, x, edge_index, batch, W1, b1, W2, b2, W3, b3,
         Wl1, bl1, Wl2, bl2, Wl3, bl3):
    g = geo
    x = np.asarray(x, f32)
    src = np.asarray(edge_index[0], np.int64)
    dst = np.asarray(edge_index[1], np.int64)
    batch = np.asarray(batch, np.int64)

    def gid_of(n):
        return (n // g.NL) * g.NLP + (n % g.NL)

    deg = np.bincount(dst, minlength=g.N).astype(f32) + 1.0
    dinv = (1.0 / np.sqrt(deg)).astype(f32)
    counts = np.bincount(batch, minlength=g.G).astype(f32)
    invc = (1.0 / np.maximum(counts, 1.0)).astype(f32)

    sums1 = np.zeros((g.G, g.F_IN), f32)
    np.add.at(sums1, batch, x)
    mean1 = (sums1 * invc[:, None]).astype(f32)          # [G, F_IN]
    U1rows = (mean1 @ np.asarray(W1[g.F_IN:], f32))      # [G, HID]

    src_g = gid_of(src)

    core_win = []
    for k in range(g.C):
        lo_n, hi_n = k * g.NL, (k + 1) * g.NL
        sel = (dst >= lo_n) & (dst < hi_n)
        es, ed = src_g[sel], (dst[sel] - lo_n)
        order = np.argsort(ed, kind="stable")
        es, ed = es[order], ed[order]
        wstart = np.searchsorted(ed, np.arange(0, g.NLP + 1, WIN))
        wins = []
        COLB = g.NLP - COLA
        for w in range(g.NWIN):
            a, b = wstart[w], wstart[w + 1]
            ws, wd = es[a:b], ed[a:b] - w * WIN
            c_, off = ws // g.NLP, ws % g.NLP
            a_m = off < COLA
            wins.append((c_[a_m] * COLA + off[a_m], wd[a_m],
                         c_[~a_m] * COLB + (off[~a_m] - COLA), wd[~a_m]))
        core_win.append(wins)

    # chunk plan: per chunk of CHUNK_WINDOWS windows, edges of all windows are
    # packed contiguously (chunk-local slot ids); tile counts are chunk-level
    # cross-core maxima, and each window gets a [t0, t1] tile range (union
    # over cores) whose tiles feed its psum via masked one-hot matmuls.
    def _ranges(lens_kw, nw):
        out = []
        for wi in range(nw):
            t0s, t1s = [], []
            for k in range(g.C):
                s = sum(lens_kw[k][:wi])
                L = lens_kw[k][wi]
                if L:
                    t0s.append(s // 128)
                    t1s.append((s + L - 1) // 128)
            out.append((min(t0s), max(t1s)) if t0s else None)
        return out

    chunk_plan = []   # (w0, nw, TC_lo, TC_hi, rng_lo, rng_hi)
    for w0 in range(0, g.NWIN, CHUNK_WINDOWS):
        nw = min(CHUNK_WINDOWS, g.NWIN - w0)
        lens_lo = [[len(core_win[k][w0 + wi][0]) for wi in range(nw)]
                   for k in range(g.C)]
        lens_hi = [[len(core_win[k][w0 + wi][2]) for wi in range(nw)]
                   for k in range(g.C)]
        TC_lo = -(-max(sum(r) for r in lens_lo) // 128)
        TC_hi = -(-max(sum(r) for r in lens_hi) // 128)
        chunk_plan.append((w0, nw, TC_lo, TC_hi,
                           _ranges(lens_lo, nw), _ranges(lens_hi, nw)))

    per_core = []
    for k in range(g.C):
        lo_n, hi_n = k * g.NL, (k + 1) * g.NL
        idx_list, slot_list = [], []
        for (w0, nw, TCl, TCh, _, _) in chunk_plan:
            for half, TC in ((0, TCl), (2, TCh)):
                s_arr = np.concatenate(
                    [core_win[k][w0 + wi][half] for wi in range(nw)]
                    + [np.zeros(0, np.int64)])
                d_arr = np.concatenate(
                    [core_win[k][w0 + wi][half + 1] + 128 * wi
                     for wi in range(nw)] + [np.zeros(0, np.int64)])
                pad = TC * 128 - len(s_arr)
                idx_list.append(np.concatenate(
                    [s_arr, np.zeros(pad, np.int64)]))
                slot_list.append(np.concatenate(
                    [d_arr, np.full(pad, PAD_SLOT)]))
        idx_all = np.concatenate(idx_list)
        slot_all = np.concatenate(slot_list).astype(f32)
        idx_sb = _wrap16(idx_all, 128)
        dst_sb = np.ascontiguousarray(
            slot_all.reshape(-1, 128).T.astype(f32))

        bl = batch[lo_n:hi_n]
        # one-hot graph-block matrices for the conv global-feature term:
        # ohconv[p, c, b, j] = 1{batch_local[cs_c + j] == 128*b + p}
        nch = len(g.conv_chunks)
        ohconv = np.zeros((128, nch, 4, CONV_CHUNK), bf16)
        for c, (cs, cw) in enumerate(g.conv_chunks):
            seg = bl[cs:min(cs + cw, g.NL)]
            for j, gid in enumerate(seg):
                ohconv[gid % 128, c, gid // 128, j] = 1
        bcol = np.searchsorted(bl, np.arange(g.G), side="right") - 1
        gap_last = _wrap16(np.where(bcol < 0, 0, bcol + 1), 96)
        present = np.zeros(g.G, bool)
        present[np.unique(bl)] = True
        maxcol = _wrap16(np.where(present, bcol, g.NLP - 1), 96)

        gstart = np.searchsorted(bl, np.arange(g.G), side="left")
        maskneg = np.zeros(g.NLP, f32)
        maskneg[np.unique(gstart[present])] = -1e30
        maskneg[g.NL] = -1e30
        maskneg_row = maskneg.reshape(1, g.NLP).astype(bf16)

        xT = np.zeros((g.F_IN, g.NLP), f32)
        xT[:, :g.NL] = x[lo_n:hi_n].T
        dinv_loc = np.zeros(g.NLP, f32)
        dinv_loc[:g.NL] = dinv[lo_n:hi_n]
        dinv_sb = np.ascontiguousarray(dinv_loc.reshape(g.NWIN, WIN).T)

        H, H2, O = g.HID, g.HID // 2, g.OUT
        inp = {
            "xT": xT.astype(bf16),
            "U1rows": np.ascontiguousarray(
                U1rows.reshape(4, 128, H).transpose(1, 0, 2)).astype(bf16),
            "ohconv": np.ascontiguousarray(
                ohconv.reshape(128, 4 * nch, CONV_CHUNK)),
            "gap_last": gap_last, "maxcol": maxcol,
            "maskneg_row": maskneg_row,
            "eidx": idx_sb,
            "dstslot": dst_sb,
            "dinv_sb": dinv_sb,
            "iota512": np.tile(np.arange(CHUNK_WINDOWS * WIN, dtype=f32),
                               (128, 1)),
            "id96": np.eye(96, dtype=f32),
            "id128b": np.eye(128, dtype=bf16),
            "invc_row": invc.reshape(1, g.G),
            "W1a": np.asarray(W1[:g.F_IN], bf16),
            "W1b": np.asarray(W1[g.F_IN:], bf16),
            "W2a": np.asarray(W2[:H], bf16), "W2b": np.asarray(W2[H:], bf16),
            "W3a": np.asarray(W3[:H], bf16), "W3b": np.asarray(W3[H:], bf16),
            "b1_row": np.asarray(b1, f32).reshape(1, H),
            "b2_row": np.asarray(b2, f32).reshape(1, H),
            "b3_row": np.asarray(b3, f32).reshape(1, H),
            "Wl1a": np.asarray(Wl1[:H], f32), "Wl1b": np.asarray(Wl1[H:], f32),
            "Wl2": np.asarray(Wl2, f32), "Wl3": np.asarray(Wl3, f32),
            "bl1": np.asarray(bl1, f32).reshape(H, 1),
            "bl2": np.asarray(bl2, f32).reshape(H2, 1),
            "bl3": np.asarray(bl3, f32).reshape(O, 1),
        }
        per_core.append(inp)

    meta = {"chunk_plan": chunk_plan,
            "Ttot": sum(c[2] + c[3] for c in chunk_plan)}
    return per_core, meta


# ---------------- device program ----------------


def build_program(geo, meta, n_cores):
    import os
    dbg_layers = int(os.environ.get("KN_LAYERS", "3"))
    dbg_stage = os.environ.get("KN_STAGE", "full")
    dbg_agg = os.environ.get("KN_AGG", "ep")
    import concourse.bacc as bacc
    import concourse.bass as bass
    import concourse.mybir as mybir
    import concourse.tile as tile

    g = geo
    H, H2, O = g.HID, g.HID // 2, g.OUT
    dt = mybir.dt
    Alu = mybir.AluOpType
    Act = mybir.ActivationFunctionType
    chunk_plan = meta["chunk_plan"]
    Ttot = meta["Ttot"]
    Tmax = max(max(c[2] + c[3] for c in chunk_plan), g.NWIN)
    Tblk = max(r[1] - r[0] + 1
               for (_, _, _, _, rl, rh) in chunk_plan
               for r in rl + rh if r is not None)

    nc = bacc.Bacc("TRN2", target_bir_lowering=False, debug=False,
                   num_devices=n_cores)
    rg = [list(range(n_cores))]

    def din(name, shape, dtype):
        return nc.dram_tensor(name, list(shape), dtype, kind="ExternalInput")

    xT_d = din("xT", [g.F_IN, g.NLP], dt.bfloat16)
    U1rows_d = din("U1rows", [128, 4, H], dt.bfloat16)
    ohconv_d = din("ohconv", [128, 4 * len(g.conv_chunks), CONV_CHUNK],
                   dt.bfloat16)
    gap_last_d = din("gap_last", [96, g.G // 16], dt.int16)
    maxcol_d = din("maxcol", [96, g.G // 16], dt.int16)
    maskneg_d = din("maskneg_row", [1, g.NLP], dt.bfloat16)
    eidx_d = din("eidx", [128, Ttot * 8], dt.int16)
    dstslot_d = din("dstslot", [128, Ttot], dt.float32)
    dinv_d = din("dinv_sb", [128, g.NWIN], dt.float32)
    iota_d = din("iota512", [128, CHUNK_WINDOWS * WIN], dt.float32)
    id96_d = din("id96", [96, 96], dt.float32)
    id128b_d = din("id128b", [128, 128], dt.bfloat16)
    invc_d = din("invc_row", [1, g.G], dt.float32)
    W_d = {n: din(n, [g.F_IN if n[1] == "1" else H, H], dt.bfloat16)
           for n in ("W1a", "W1b", "W2a", "W2b", "W3a", "W3b")}
    b_d = {n: din(n, [1, H], dt.float32)
           for n in ("b1_row", "b2_row", "b3_row")}
    Wl1a_d = din("Wl1a", [H, H], dt.float32)
    Wl1b_d = din("Wl1b", [H, H], dt.float32)
    Wl2_d = din("Wl2", [H, H2], dt.float32)
    Wl3_d = din("Wl3", [H2, O], dt.float32)
    bl1_d = din("bl1", [H, 1], dt.float32)
    bl2_d = din("bl2", [H2, 1], dt.float32)
    bl3_d = din("bl3", [O, 1], dt.float32)

    out_d = nc.dram_tensor("out", [O, g.G], dt.float32, kind="ExternalOutput")

    # internal DRAM (table split in two halves so AG-A overlaps conv tail)
    COLB = g.NLP - COLA
    tshardA = nc.dram_tensor("tshardA", [COLA, 128], dt.bfloat16,
                             kind="Internal")
    tshardB = nc.dram_tensor("tshardB", [COLB, 128], dt.bfloat16,
                             kind="Internal")
    tableA = nc.dram_tensor("tableA", [g.C * COLA, 128], dt.bfloat16,
                            kind="Internal", addr_space="Shared")
    tableB = nc.dram_tensor("tableB", [g.C * COLB, 128], dt.bfloat16,
                            kind="Internal", addr_space="Shared")
    gap_in = [nc.dram_tensor(f"gap_in{i}", [96, g.G], dt.float32,
                             kind="Internal") for i in range(3)]
    gap_out = [nc.dram_tensor(f"gap_out{i}", [96, g.G], dt.float32,
                              kind="Internal", addr_space="Shared")
               for i in range(3)]
    gmp_in = [nc.dram_tensor(f"gmp_in{i}", [96, g.G], dt.float32,
                             kind="Internal") for i in range(3)]
    gmp_out = [nc.dram_tensor(f"gmp_out{i}", [96, g.G], dt.float32,
                              kind="Internal", addr_space="Shared")
               for i in range(3)]

    with tile.TileContext(nc) as tc:
        import contextlib
        stk = contextlib.ExitStack()
        pp = stk.enter_context(tc.tile_pool(name="persist", bufs=1))
        wk = stk.enter_context(tc.tile_pool(name="work", bufs=2))
        ep = stk.enter_context(tc.tile_pool(name="epil", bufs=2))
        ps_conv = stk.enter_context(
            tc.tile_pool(name="ps_conv", bufs=2, space="PSUM"))
        ps_tr = stk.enter_context(
            tc.tile_pool(name="ps_tr", bufs=2, space="PSUM"))
        ps_agg = stk.enter_context(
            tc.tile_pool(name="ps_agg", bufs=4, space="PSUM"))

        def load(pool, dram, shape, dtype, tag, bcast=None):
            t = pool.tile(shape, dtype, tag=tag, name=tag)
            src = dram.ap() if bcast is None else dram.ap().to_broadcast(bcast)
            nc.sync.dma_start(out=t[:], in_=src)
            return t

        # constants
        iota_sb = load(pp, iota_d, [128, CHUNK_WINDOWS * WIN], dt.float32,
                       "iota")
        id96_sb = load(pp, id96_d, [96, 96], dt.float32, "id96")
        id128b_sb = load(pp, id128b_d, [128, 128], dt.bfloat16, "id128b")
        dinv_sbT = load(pp, dinv_d, [128, g.NWIN], dt.float32, "dinv")
        invc_bc = load(pp, invc_d, [96, g.G], dt.float32, "invc",
                       bcast=(96, g.G))
        maskneg_sb = load(pp, maskneg_d, [96, g.NLP], dt.bfloat16, "maskn",
                          bcast=(96, g.NLP))

        gap_last_sb = load(pp, gap_last_d, [96, g.G // 16], dt.int16, "glast")
        maxcol_sb = load(pp, maxcol_d, [96, g.G // 16], dt.int16, "maxcol")
        Urows = load(pp, U1rows_d, [128, 4, H], dt.bfloat16, "Urows")
        tabkeep = pp.tile([128, g.NWIN, 96], dt.bfloat16, tag="tabkeep",
                          name="tabkeep")
        W_sb = {n: load(pp, W_d[n], list(W_d[n].shape), dt.bfloat16, n)
                for n in W_d}
        b_bc = {n: load(pp, b_d[n], [128, H], dt.float32, n, bcast=(128, H))
                for n in b_d}
        Wl1a_sb = load(pp, Wl1a_d, [H, H], dt.float32, "Wl1a")
        Wl1b_sb = load(pp, Wl1b_d, [H, H], dt.float32, "Wl1b")
        Wl2_sb = load(pp, Wl2_d, [H, H2], dt.float32, "Wl2")
        Wl3_sb = load(pp, Wl3_d, [H2, O], dt.float32, "Wl3")
        bl1_sb = load(pp, bl1_d, [H, 1], dt.float32, "bl1")
        bl2_sb = load(pp, bl2_d, [H2, 1], dt.float32, "bl2")
        bl3_sb = load(pp, bl3_d, [O, 1], dt.float32, "bl3")

        xbuf = [pp.tile([128, g.NLP], dt.bfloat16, tag=f"xbuf{i}", name=f"xbuf{i}")
                for i in range(2)]
        nc.sync.dma_start(out=xbuf[0][:g.F_IN, :], in_=xT_d.ap())
        # edge-gather tables load after the conv-critical inputs
        eidx_sb = load(pp, eidx_d, [128, Ttot * 8], dt.int16, "eidx")
        dst_sb = load(pp, dstslot_d, [128, Ttot], dt.float32, "dstslot")

        scano = pp.tile([96, g.NLP], dt.float32, tag="scano", name="scano")
        gmp_all = pp.tile([96, 3 * g.G], dt.float32, tag="gmp_all", name="gmp_all")
        mg_sb = [pp.tile([96, g.G], dt.float32, tag=f"mg{i}", name=f"mg{i}")
                 for i in range(3)]
        gapar_sb = [pp.tile([96, g.G], dt.float32, tag=f"gapar{i}", name=f"gapar{i}")
                    for i in range(3)]

        zinit = wk.tile([128, Tmax, 128], dt.bfloat16, tag="gath", name="zinit")
        nc.gpsimd.memset(zinit[:, :g.NWIN, :], 0.0)
        nc.sync.dma_start(
            out=tshardA.ap().rearrange("(w p) c -> p w c", p=128),
            in_=zinit[:, :WINA, :])
        nc.sync.dma_start(
            out=tshardB.ap().rearrange("(w p) c -> p w c", p=128),
            in_=zinit[:, WINA:g.NWIN, :])

        layer_W = [("W1a", "W1b", "b1_row"), ("W2a", "W2b", "b2_row"),
                   ("W3a", "W3b", "b3_row")]

        for l in range(dbg_layers):
            F = g.F_IN if l == 0 else H
            Wa = W_sb[layer_W[l][0]]
            Wb = W_sb[layer_W[l][1]]
            bias = b_bc[layer_W[l][2]]
            x_src, x_dst = xbuf[l % 2], xbuf[(l + 1) % 2]

            # ---- conv + gather-table build ----
            nc.vector.memset(scano[:, 0:1], 0.0)
            for ci, (cs, cw) in enumerate(g.conv_chunks):
                ohc = wk.tile([128, 4, CONV_CHUNK], dt.bfloat16, tag="ohc",
                              name="ohc")
                nc.sync.dma_start(out=ohc[:],
                                  in_=ohconv_d.ap()[:, 4 * ci:4 * ci + 4, :])
                for wo in range(0, cw, 128):
                    w = (cs + wo) // 128
                    # node-major conv: psn[n, f] = x_cat[:, n]^T W + U[:, batch_n]
                    psn = ps_tr.tile([128, 128], dt.float32, tag="ptr",
                                     name="ptr")
                    nc.tensor.matmul(out=psn[:, :96],
                                     lhsT=x_src[:F, cs + wo:cs + wo + 128],
                                     rhs=Wa[:], start=True, stop=False)
                    for b in range(4):
                        nc.tensor.matmul(out=psn[:, :96],
                                         lhsT=ohc[:, b, wo:wo + 128],
                                         rhs=Urows[:, b, :],
                                         start=False, stop=b == 3)
                    nc.scalar.activation(out=tabkeep[:, w, :], in_=psn[:, :96],
                                         func=Act.Copy,
                                         scale=dinv_sbT[:, w:w + 1])
                    if w < WINA:
                        tgt = tshardA.ap()[w * 128:(w + 1) * 128, :96]
                    else:
                        wb = w - WINA
                        tgt = tshardB.ap()[wb * 128:(wb + 1) * 128, :96]
                    nc.sync.dma_start(out=tgt, in_=tabkeep[:, w, :])
                if cs + cw >= COLA and cs < COLA:
                    # windows 0..WINA-1 written: AG half A overlaps conv tail
                    nc.gpsimd.collective_compute(
                        "AllGather", Alu.bypass, replica_groups=rg,
                        ins=[tshardA.ap()], outs=[tableA.ap()])

            if dbg_stage == "conv":
                break
            # ---- all-gather table half B ----
            nc.gpsimd.collective_compute(
                "AllGather", Alu.bypass, replica_groups=rg,
                ins=[tshardB.ap()], outs=[tableB.ap()])

            if dbg_stage == "ag":
                break
            # ---- edge aggregation ----
            t_base = 0
            for (w0, nw, Tlo, Thi, rng_l, rng_h) in chunk_plan:
                T = Tlo + Thi
                gath = wk.tile([128, Tmax, 128], dt.bfloat16, tag="gath", name="gath")
                # dma_gather is limited to 1024 indices (8 tiles) per call
                for (goff, tstart, cnt, hi) in (
                        (0, t_base, Tlo, False),
                        (Tlo, t_base + Tlo, Thi, True)):
                    if cnt == 0:
                        continue
                    tbl_ap = tableB.ap() if hi else tableA.ap()
                    done = 0
                    while done < cnt:
                        nt = min(8, cnt - done)
                        nc.gpsimd.dma_gather(
                            gath[:, goff + done:goff + done + nt, :],
                            tbl_ap,
                            eidx_sb[:, 8 * (tstart + done):
                                    8 * (tstart + done + nt)],
                            num_idxs=nt * 128, num_idxs_reg=nt * 128,
                            elem_size=128)
                        done += nt
                if dbg_agg == "gather":
                    t_base += T
                    continue
                # per window: masked one-hot matmuls over its tile ranges
                blocks = []   # (wi, gath offset of first tile, ntiles)
                for wi in range(nw):
                    if rng_l[wi] is not None:
                        blocks.append((wi, rng_l[wi][0],
                                       rng_l[wi][1] - rng_l[wi][0] + 1))
                    if rng_h[wi] is not None:
                        blocks.append((wi, Tlo + rng_h[wi][0],
                                       rng_h[wi][1] - rng_h[wi][0] + 1))
                pags = [ps_agg.tile([96, 128], dt.float32, tag="pag", name="pag")
                        for _ in range(nw)]
                remaining = [0] * nw
                for (wi, _, Tb) in blocks:
                    remaining[wi] += Tb
                for wi in range(nw):
                    # self-loop term: psum += table rows of this window
                    nc.tensor.matmul(out=pags[wi][:],
                                     lhsT=tabkeep[:, w0 + wi, :],
                                     rhs=id128b_sb[:],
                                     start=True, stop=remaining[wi] == 0)
                for (wi, toff, Tb) in blocks:
                    oh = wk.tile([128, Tblk, 128], dt.bfloat16, tag="oh",
                                 name="oh", bufs=4)
                    a = t_base + toff
                    nc.vector.tensor_tensor(
                        out=oh[:, 0:Tb, :],
                        in0=dst_sb[:, a:a + Tb, None]
                            .to_broadcast((128, Tb, 128)),
                        in1=iota_sb[:, None, wi * 128:(wi + 1) * 128]
                            .to_broadcast((128, Tb, 128)),
                        op=Alu.is_equal)
                    for t in range(Tb):
                        if dbg_agg == "onehot":
                            continue
                        remaining[wi] -= 1
                        nc.tensor.matmul(out=pags[wi][:],
                                         lhsT=gath[:, toff + t, :96],
                                         rhs=oh[:, t, :],
                                         start=False,
                                         stop=remaining[wi] == 0)

                for wi in range(nw):
                    if dbg_agg in ("onehot", "mm"):
                        break
                    w = w0 + wi
                    sb1 = ep.tile([96, 128], dt.float32, tag="ep1", name="ep1")
                    nc.vector.tensor_copy(out=sb1[:], in_=pags[wi][:])
                    pt2 = ps_tr.tile([128, 128], dt.float32, tag="ptr", name="ptr")
                    nc.tensor.transpose(out=pt2[:, :96], in_=sb1[:],
                                        identity=id96_sb[:])
                    sb2 = ep.tile([128, 96], dt.float32, tag="ep2", name="ep2")
                    nc.scalar.activation(out=sb2[:], in_=pt2[:, :96],
                                         func=Act.Copy,
                                         scale=dinv_sbT[:, w:w + 1])
                    sb3 = ep.tile([128, 96], dt.bfloat16, tag="ep3", name="ep3")
                    if l < 2:
                        tmp = ep.tile([128, 96], dt.float32, tag="ep3f", name="ep3f")
                        nc.vector.tensor_add(out=tmp[:], in0=sb2[:],
                                             in1=bias[:, :96])
                        nc.vector.tensor_scalar_max(out=sb3[:], in0=tmp[:],
                                                    scalar1=0.0)
                    else:
                        nc.vector.tensor_add(out=sb3[:], in0=sb2[:],
                                             in1=bias[:, :96])
                    pt3 = ps_tr.tile([128, 128], dt.bfloat16, tag="ptr", name="ptr")
                    nc.tensor.transpose(out=pt3[:96, :], in_=sb3[:],
                                        identity=id128b_sb[:])
                    nc.vector.tensor_copy(
                        out=x_dst[:96, w * 128:(w + 1) * 128],
                        in_=pt3[:96, :])
                # chained gap cumsum over this chunk's finished columns: the
                # pooling scan rides under the remaining edge gathers
                c_lo, c_hi = w0 * 128, min((w0 + nw) * 128, g.NL)
                if c_lo < g.NL and dbg_agg == "ep":
                    nc.vector.tensor_tensor_scan(
                        out=scano[:, c_lo + 1:c_hi + 1],
                        data0=x_dst[:96, c_lo:c_hi],
                        data1=x_dst[:96, c_lo:c_hi],
                        initial=(0.0 if w0 == 0
                                 else scano[:, c_lo:c_lo + 1]),
                        op0=Alu.add, op1=Alu.bypass)
                t_base += T

            nc.gpsimd.memset(x_dst[:96, g.NL:g.NLP], PAD_VAL)

            if dbg_stage == "agg":
                break
            # ---- pooling ----
            cum_l = ep.tile([96, g.G + 1], dt.float32, tag="cuml", name="cuml",
                            bufs=1)
            nc.vector.memset(cum_l[:, 0:1], 0.0)
            nc.gpsimd.ap_gather(cum_l[:, 1:g.G + 1], scano[:], gap_last_sb[:],
                                channels=96, num_elems=g.NLP, d=1,
                                num_idxs=g.G)
            gaps = ep.tile([96, g.G], dt.float32, tag="gaps", name="gaps", bufs=1)
            nc.vector.tensor_tensor(out=gaps[:], in0=cum_l[:, 1:g.G + 1],
                                    in1=cum_l[:, 0:g.G],
                                    op=Alu.subtract)
            nc.sync.dma_start(out=gap_in[l].ap(), in_=gaps[:])
            nc.gpsimd.collective_compute(
                "AllReduce", Alu.add, replica_groups=rg,
                ins=[gap_in[l].ap()], outs=[gap_out[l].ap()])
            nc.sync.dma_start(out=gapar_sb[l][:], in_=gap_out[l].ap())

            nc.vector.tensor_tensor_scan(
                out=scano[:], data0=maskneg_sb[:], data1=x_dst[:96, :],
                initial=0.0, op0=Alu.add, op1=Alu.max)
            nc.gpsimd.ap_gather(gmp_all[:, l * g.G:(l + 1) * g.G],
                                scano[:], maxcol_sb[:],
                                channels=96, num_elems=g.NLP, d=1,
                                num_idxs=g.G)
            # per-layer gmp AllReduce: layers 0-1 overlap the next layer's
            # edge gathers; only layer 2's is exposed
            nc.sync.dma_start(out=gmp_in[l].ap(),
                              in_=gmp_all[:, l * g.G:(l + 1) * g.G])
            nc.gpsimd.collective_compute(
                "AllReduce", Alu.max, replica_groups=rg,
                ins=[gmp_in[l].ap()], outs=[gmp_out[l].ap()])

            nc.vector.tensor_mul(out=mg_sb[l][:], in0=gapar_sb[l][:],
                                 in1=invc_bc[:])

            if l < 2:
                # build Urows for next layer: U = Wb_{l+1}^T @ mg,
                # transposed into [128 graph, 4 block, 96] for the conv
                Wbn = W_sb[layer_W[l + 1][1]]
                mgb = ep.tile([96, g.G], dt.bfloat16, tag="mgb", name="mgb",
                              bufs=1)
                nc.scalar.copy(out=mgb[:], in_=mg_sb[l][:])
                psU = ps_conv.tile([96, g.G], dt.float32, tag="psc", name="psc")
                nc.tensor.matmul(out=psU[:], lhsT=Wbn[:], rhs=mgb[:],
                                 start=True, stop=True)
                Usb = ep.tile([96, g.G], dt.bfloat16, tag="Usb", name="Usb",
                              bufs=1)
                nc.vector.tensor_copy(out=Usb[:], in_=psU[:])
                for b in range(4):
                    ptU = ps_tr.tile([128, 128], dt.bfloat16, tag="ptr",
                                     name="ptr")
                    nc.tensor.transpose(out=ptU[:, :96],
                                        in_=Usb[:, b * 128:(b + 1) * 128],
                                        identity=id128b_sb[:96, :96])
                    nc.scalar.copy(out=Urows[:, b, :], in_=ptU[:, :96])

        debug_cut = dbg_stage != "full" or dbg_layers < 3
        if debug_cut:
            nc.gpsimd.dma_start(out=out_d.ap(), in_=xbuf[0][:O, :g.G])
        # ---- final readout MLP (f32) ----
        if not debug_cut:
            gmpar = pp.tile([96, 3 * g.G], dt.float32, tag="gmpar", name="gmpar")
            for i in range(3):
                nc.sync.dma_start(out=gmpar[:, i * g.G:(i + 1) * g.G],
                                  in_=gmp_out[i].ap())

            hTa = pp.tile([96, g.G], dt.float32, tag="hTa", name="hTa")
            hTb = pp.tile([96, g.G], dt.float32, tag="hTb", name="hTb")
            nc.vector.tensor_add(out=hTa[:], in0=gmpar[:, 0:g.G],
                                 in1=gmpar[:, g.G:2 * g.G])
            nc.vector.tensor_add(out=hTa[:], in0=hTa[:],
                                 in1=gmpar[:, 2 * g.G:3 * g.G])
            nc.vector.tensor_add(out=hTb[:], in0=mg_sb[0][:], in1=mg_sb[1][:])
            nc.vector.tensor_add(out=hTb[:], in0=hTb[:], in1=mg_sb[2][:])

            ps1 = ps_conv.tile([96, g.G], dt.float32, tag="psc", name="psc")
            nc.tensor.matmul(out=ps1[:], lhsT=Wl1a_sb[:], rhs=hTa[:],
                             start=True, stop=False)
            nc.tensor.matmul(out=ps1[:], lhsT=Wl1b_sb[:], rhs=hTb[:],
                             start=False, stop=True)
            o1 = pp.tile([96, g.G], dt.float32, tag="o1", name="o1")
            nc.scalar.activation(out=o1[:], in_=ps1[:], func=Act.Relu,
                                 bias=bl1_sb[:])
            ps2 = ps_conv.tile([96, g.G], dt.float32, tag="psc", name="psc")
            nc.tensor.matmul(out=ps2[:H2, :], lhsT=Wl2_sb[:], rhs=o1[:],
                             start=True, stop=True)
            o2 = pp.tile([H2, g.G], dt.float32, tag="o2", name="o2")
            nc.scalar.activation(out=o2[:], in_=ps2[:H2, :], func=Act.Relu,
                                 bias=bl2_sb[:])
            ps3 = ps_conv.tile([96, g.G], dt.float32, tag="psc", name="psc")
            nc.tensor.matmul(out=ps3[:O, :], lhsT=Wl3_sb[:], rhs=o2[:],
                             start=True, stop=True)
            o3 = pp.tile([O, g.G], dt.float32, tag="o3", name="o3")
            nc.scalar.activation(out=o3[:], in_=ps3[:O, :], func=Act.Identity,
                                 bias=bl3_sb[:])
            nc.sync.dma_start(out=out_d.ap(), in_=o3[:])

        stk.close()

    nc.compile()
    return nc


_CACHE = {}


def _get_program(geo, meta, n_cores):
    key = (repr(sorted(geo.__dict__.items(), key=str)),
           repr(meta["chunk_plan"]), n_cores)
    if key not in _CACHE:
        _CACHE[key] = build_program(geo, meta, n_cores)
    return _CACHE[key]


def kernel(**inputs):
    from concourse.bass_utils import run_bass_kernel_spmd

    geo = Geo(CFG)
    inputs = {k: np.asarray(v) for k, v in inputs.items()}
    per_core, meta = prep(geo, **inputs)
    nc = _get_program(geo, meta, geo.C)
    res = run_bass_kernel_spmd(nc, per_core, core_ids=list(range(geo.C)))
    out = np.asarray(res.results[0]["out"], f32)   # [OUT, G]
    return np.ascontiguousarray(out.T)             # [G, OUT] float32



# revision 50
# speedup vs baseline: 1.3016x; 1.0057x over previous
"""Trainium2 Bass kernel for nn_BenchGNN_29300266893894 (3-layer GCN with
global-feature concat + global mean/max pooling readout + MLP head).

Self-contained: host-side sharding/packing prep + SPMD Bass/Tile program on
8 NeuronCores via run_bass_kernel_spmd.

Sharding: nodes are split into 8 contiguous shards of 6250 (padded to 6272 =
49*128). Edges are partitioned by dst owner and sorted by dst window; weight
matrices are replicated. Per layer, each core computes h = x_cat @ W for its
nodes (the global-mean term enters as U = Wb^T @ mean matmul'd against
static one-hot graph-block matrices - no gather), builds a bf16 table of
h' = h*dinv rows (256 B each, kept in SBUF as tabkeep AND written to DRAM
in two halves whose AllGathers overlap the conv tail), then aggregates
edges with dma_gather + masked one-hot matmuls accumulating per-window PSUM
tiles; edge tiles are packed at 4-window-chunk granularity and self-loops
are added via one identity matmul per window from tabkeep. Graph pooling
uses DVE scans + one boundary ap_gather per pool (gap via shift-subtract)
with per-layer AllReduces overlapped into the next layer's gather stream.
"""
import sys
import numpy as np
import ml_dtypes

sys.path.insert(0, "/opt/trn_rl_repo")

bf16 = ml_dtypes.bfloat16
f32 = np.float32

# ---------------- problem geometry (hardcoded) ----------------
CFG = dict(N=50000, E=800000, G=512, F_IN=128, HID=96, OUT=10, C=8)

WIN = 128
WINA = 25            # windows per core in table half A
COLA = WINA * WIN    # 3200
CHUNK_WINDOWS = 4
CONV_CHUNK = 512
PAD_SLOT = 999.0
PAD_VAL = -60000.0


class Geo:
    def __init__(self, cfg):
        self.__dict__.update(cfg)
        assert self.N % self.C == 0
        self.NL = self.N // self.C
        self.NWIN = -(-self.NL // WIN)
        self.NLP = self.NWIN * WIN
        # pooling pad-column tricks need at least one pad column per core
        assert self.NLP > self.NL
        self.NP = self.NLP * self.C
        self.conv_chunks = [(o, min(CONV_CHUNK, self.NLP - o))
                            for o in range(0, self.NLP, CONV_CHUNK)]


def _wrap16(idx, channels):
    idx = np.asarray(idx, np.int16)
    assert len(idx) % 16 == 0
    idx16 = idx.reshape(-1, 16).T
    return np.ascontiguousarray(np.tile(idx16, (channels // 16, 1)))


def prep(geo, x, edge_index, batch, W1, b1, W2, b2, W3, b3,
         Wl1, bl1, Wl2, bl2, Wl3, bl3):
    g = geo
    x = np.asarray(x, f32)
    src = np.asarray(edge_index[0], np.int64)
    dst = np.asarray(edge_index[1], np.int64)
    batch = np.asarray(batch, np.int64)

    def gid_of(n):
        return (n // g.NL) * g.NLP + (n % g.NL)

    deg = np.bincount(dst, minlength=g.N).astype(f32) + 1.0
    dinv = (1.0 / np.sqrt(deg)).astype(f32)
    counts = np.bincount(batch, minlength=g.G).astype(f32)
    invc = (1.0 / np.maximum(counts, 1.0)).astype(f32)

    sums1 = np.zeros((g.G, g.F_IN), f32)
    np.add.at(sums1, batch, x)
    mean1 = (sums1 * invc[:, None]).astype(f32)          # [G, F_IN]
    U1rows = (mean1 @ np.asarray(W1[g.F_IN:], f32))      # [G, HID]

    src_g = gid_of(src)

    core_win = []
    for k in range(g.C):
        lo_n, hi_n = k * g.NL, (k + 1) * g.NL
        sel = (dst >= lo_n) & (dst < hi_n)
        es, ed = src_g[sel], (dst[sel] - lo_n)
        order = np.argsort(ed, kind="stable")
        es, ed = es[order], ed[order]
        wstart = np.searchsorted(ed, np.arange(0, g.NLP + 1, WIN))
        wins = []
        COLB = g.NLP - COLA
        for w in range(g.NWIN):
            a, b = wstart[w], wstart[w + 1]
            ws, wd = es[a:b], ed[a:b] - w * WIN
            c_, off = ws // g.NLP, ws % g.NLP
            a_m = off < COLA
            wins.append((c_[a_m] * COLA + off[a_m], wd[a_m],
                         c_[~a_m] * COLB + (off[~a_m] - COLA), wd[~a_m]))
        core_win.append(wins)

    # chunk plan: per chunk of CHUNK_WINDOWS windows, edges of all windows are
    # packed contiguously (chunk-local slot ids); tile counts are chunk-level
    # cross-core maxima, and each window gets a [t0, t1] tile range (union
    # over cores) whose tiles feed its psum via masked one-hot matmuls.
    def _ranges(lens_kw, nw):
        out = []
        for wi in range(nw):
            t0s, t1s = [], []
            for k in range(g.C):
                s = sum(lens_kw[k][:wi])
                L = lens_kw[k][wi]
                if L:
                    t0s.append(s // 128)
                    t1s.append((s + L - 1) // 128)
            out.append((min(t0s), max(t1s)) if t0s else None)
        return out

    chunk_plan = []   # (w0, nw, TC_lo, TC_hi, rng_lo, rng_hi)
    for w0 in range(0, g.NWIN, CHUNK_WINDOWS):
        nw = min(CHUNK_WINDOWS, g.NWIN - w0)
        lens_lo = [[len(core_win[k][w0 + wi][0]) for wi in range(nw)]
                   for k in range(g.C)]
        lens_hi = [[len(core_win[k][w0 + wi][2]) for wi in range(nw)]
                   for k in range(g.C)]
        TC_lo = -(-max(sum(r) for r in lens_lo) // 128)
        TC_hi = -(-max(sum(r) for r in lens_hi) // 128)
        chunk_plan.append((w0, nw, TC_lo, TC_hi,
                           _ranges(lens_lo, nw), _ranges(lens_hi, nw)))

    per_core = []
    for k in range(g.C):
        lo_n, hi_n = k * g.NL, (k + 1) * g.NL
        idx_list, slot_list = [], []
        for (w0, nw, TCl, TCh, _, _) in chunk_plan:
            for half, TC in ((0, TCl), (2, TCh)):
                s_arr = np.concatenate(
                    [core_win[k][w0 + wi][half] for wi in range(nw)]
                    + [np.zeros(0, np.int64)])
                d_arr = np.concatenate(
                    [core_win[k][w0 + wi][half + 1] + 128 * wi
                     for wi in range(nw)] + [np.zeros(0, np.int64)])
                pad = TC * 128 - len(s_arr)
                idx_list.append(np.concatenate(
                    [s_arr, np.zeros(pad, np.int64)]))
                slot_list.append(np.concatenate(
                    [d_arr, np.full(pad, PAD_SLOT)]))
        idx_all = np.concatenate(idx_list)
        slot_all = np.concatenate(slot_list).astype(f32)
        idx_sb = _wrap16(idx_all, 128)
        dst_sb = np.ascontiguousarray(
            slot_all.reshape(-1, 128).T.astype(f32))

        bl = batch[lo_n:hi_n]
        # one-hot graph-block matrices for the conv global-feature term:
        # ohconv[p, c, b, j] = 1{batch_local[cs_c + j] == 128*b + p}
        nch = len(g.conv_chunks)
        ohconv = np.zeros((128, nch, 4, CONV_CHUNK), bf16)
        for c, (cs, cw) in enumerate(g.conv_chunks):
            seg = bl[cs:min(cs + cw, g.NL)]
            for j, gid in enumerate(seg):
                ohconv[gid % 128, c, gid // 128, j] = 1
        bcol = np.searchsorted(bl, np.arange(g.G), side="right") - 1
        gap_last = _wrap16(np.where(bcol < 0, 0, bcol + 1), 96)
        present = np.zeros(g.G, bool)
        present[np.unique(bl)] = True
        maxcol = _wrap16(np.where(present, bcol, g.NLP - 1), 96)

        gstart = np.searchsorted(bl, np.arange(g.G), side="left")
        maskneg = np.zeros(g.NLP, f32)
        maskneg[np.unique(gstart[present])] = -1e30
        maskneg[g.NL] = -1e30
        maskneg_row = maskneg.reshape(1, g.NLP).astype(bf16)

        xT = np.zeros((g.F_IN, g.NLP), f32)
        xT[:, :g.NL] = x[lo_n:hi_n].T
        dinv_loc = np.zeros(g.NLP, f32)
        dinv_loc[:g.NL] = dinv[lo_n:hi_n]
        dinv_sb = np.ascontiguousarray(dinv_loc.reshape(g.NWIN, WIN).T)

        H, H2, O = g.HID, g.HID // 2, g.OUT
        inp = {
            "xT": xT.astype(bf16),
            "U1rows": np.ascontiguousarray(
                U1rows.reshape(4, 128, H).transpose(1, 0, 2)).astype(bf16),
            "ohconv": np.ascontiguousarray(
                ohconv.reshape(128, 4 * nch, CONV_CHUNK)),
            "gap_last": gap_last, "maxcol": maxcol,
            "maskneg_row": maskneg_row,
            "eidx": idx_sb,
            "dstslot": dst_sb,
            "dinv_sb": dinv_sb,
            "iota512": np.tile(np.arange(CHUNK_WINDOWS * WIN, dtype=f32),
                               (128, 1)),
            "id96": np.eye(96, dtype=f32),
            "id128b": np.eye(128, dtype=bf16),
            "invc_row": invc.reshape(1, g.G),
            "W1a": np.asarray(W1[:g.F_IN], bf16),
            "W1b": np.asarray(W1[g.F_IN:], bf16),
            "W2a": np.asarray(W2[:H], bf16), "W2b": np.asarray(W2[H:], bf16),
            "W3a": np.asarray(W3[:H], bf16), "W3b": np.asarray(W3[H:], bf16),
            "b1_row": np.asarray(b1, f32).reshape(1, H),
            "b2_row": np.asarray(b2, f32).reshape(1, H),
            "b3_row": np.asarray(b3, f32).reshape(1, H),
            "Wl1a": np.asarray(Wl1[:H], f32), "Wl1b": np.asarray(Wl1[H:], f32),
            "Wl2": np.asarray(Wl2, f32), "Wl3": np.asarray(Wl3, f32),
            "bl1": np.asarray(bl1, f32).reshape(H, 1),
            "bl2": np.asarray(bl2, f32).reshape(H2, 1),
            "bl3": np.asarray(bl3, f32).reshape(O, 1),
        }
        per_core.append(inp)

    meta = {"chunk_plan": chunk_plan,
            "Ttot": sum(c[2] + c[3] for c in chunk_plan)}
    return per_core, meta


# ---------------- device program ----------------


def build_program(geo, meta, n_cores):
    import os
    dbg_layers = int(os.environ.get("KN_LAYERS", "3"))
    dbg_stage = os.environ.get("KN_STAGE", "full")
    dbg_agg = os.environ.get("KN_AGG", "ep")
    import concourse.bacc as bacc
    import concourse.bass as bass
    import concourse.mybir as mybir
    import concourse.tile as tile

    g = geo
    H, H2, O = g.HID, g.HID // 2, g.OUT
    dt = mybir.dt
    Alu = mybir.AluOpType
    Act = mybir.ActivationFunctionType
    chunk_plan = meta["chunk_plan"]
    Ttot = meta["Ttot"]
    Tmax = max(max(c[2] + c[3] for c in chunk_plan), g.NWIN)
    Tblk = max(r[1] - r[0] + 1
               for (_, _, _, _, rl, rh) in chunk_plan
               for r in rl + rh if r is not None)

    nc = bacc.Bacc("TRN2", target_bir_lowering=False, debug=False,
                   num_devices=n_cores)
    rg = [list(range(n_cores))]

    def din(name, shape, dtype):
        return nc.dram_tensor(name, list(shape), dtype, kind="ExternalInput")

    xT_d = din("xT", [g.F_IN, g.NLP], dt.bfloat16)
    U1rows_d = din("U1rows", [128, 4, H], dt.bfloat16)
    ohconv_d = din("ohconv", [128, 4 * len(g.conv_chunks), CONV_CHUNK],
                   dt.bfloat16)
    gap_last_d = din("gap_last", [96, g.G // 16], dt.int16)
    maxcol_d = din("maxcol", [96, g.G // 16], dt.int16)
    maskneg_d = din("maskneg_row", [1, g.NLP], dt.bfloat16)
    eidx_d = din("eidx", [128, Ttot * 8], dt.int16)
    dstslot_d = din("dstslot", [128, Ttot], dt.float32)
    dinv_d = din("dinv_sb", [128, g.NWIN], dt.float32)
    iota_d = din("iota512", [128, CHUNK_WINDOWS * WIN], dt.float32)
    id96_d = din("id96", [96, 96], dt.float32)
    id128b_d = din("id128b", [128, 128], dt.bfloat16)
    invc_d = din("invc_row", [1, g.G], dt.float32)
    W_d = {n: din(n, [g.F_IN if n[1] == "1" else H, H], dt.bfloat16)
           for n in ("W1a", "W1b", "W2a", "W2b", "W3a", "W3b")}
    b_d = {n: din(n, [1, H], dt.float32)
           for n in ("b1_row", "b2_row", "b3_row")}
    Wl1a_d = din("Wl1a", [H, H], dt.float32)
    Wl1b_d = din("Wl1b", [H, H], dt.float32)
    Wl2_d = din("Wl2", [H, H2], dt.float32)
    Wl3_d = din("Wl3", [H2, O], dt.float32)
    bl1_d = din("bl1", [H, 1], dt.float32)
    bl2_d = din("bl2", [H2, 1], dt.float32)
    bl3_d = din("bl3", [O, 1], dt.float32)

    out_d = nc.dram_tensor("out", [O, g.G], dt.float32, kind="ExternalOutput")

    # internal DRAM (table split in two halves so AG-A overlaps conv tail)
    COLB = g.NLP - COLA
    tshardA = nc.dram_tensor("tshardA", [COLA, 128], dt.bfloat16,
                             kind="Internal")
    tshardB = nc.dram_tensor("tshardB", [COLB, 128], dt.bfloat16,
                             kind="Internal")
    tableA = nc.dram_tensor("tableA", [g.C * COLA, 128], dt.bfloat16,
                            kind="Internal", addr_space="Shared")
    tableB = nc.dram_tensor("tableB", [g.C * COLB, 128], dt.bfloat16,
                            kind="Internal", addr_space="Shared")
    gap_in = [nc.dram_tensor(f"gap_in{i}", [96, g.G], dt.float32,
                             kind="Internal") for i in range(3)]
    gap_out = [nc.dram_tensor(f"gap_out{i}", [96, g.G], dt.float32,
                              kind="Internal", addr_space="Shared")
               for i in range(3)]
    gmp_in = [nc.dram_tensor(f"gmp_in{i}", [96, g.G], dt.float32,
                             kind="Internal") for i in range(3)]
    gmp_out = [nc.dram_tensor(f"gmp_out{i}", [96, g.G], dt.float32,
                              kind="Internal", addr_space="Shared")
               for i in range(3)]

    with tile.TileContext(nc) as tc:
        import contextlib
        stk = contextlib.ExitStack()
        pp = stk.enter_context(tc.tile_pool(name="persist", bufs=1))
        wk = stk.enter_context(tc.tile_pool(name="work", bufs=2))
        ep = stk.enter_context(tc.tile_pool(name="epil", bufs=2))
        ps_conv = stk.enter_context(
            tc.tile_pool(name="ps_conv", bufs=2, space="PSUM"))
        ps_tr = stk.enter_context(
            tc.tile_pool(name="ps_tr", bufs=2, space="PSUM"))
        ps_agg = stk.enter_context(
            tc.tile_pool(name="ps_agg", bufs=4, space="PSUM"))

        def load(pool, dram, shape, dtype, tag, bcast=None):
            t = pool.tile(shape, dtype, tag=tag, name=tag)
            src = dram.ap() if bcast is None else dram.ap().to_broadcast(bcast)
            nc.sync.dma_start(out=t[:], in_=src)
            return t

        # constants
        iota_sb = load(pp, iota_d, [128, CHUNK_WINDOWS * WIN], dt.float32,
                       "iota")
        id96_sb = load(pp, id96_d, [96, 96], dt.float32, "id96")
        id128b_sb = load(pp, id128b_d, [128, 128], dt.bfloat16, "id128b")
        dinv_sbT = load(pp, dinv_d, [128, g.NWIN], dt.float32, "dinv")
        invc_bc = load(pp, invc_d, [96, g.G], dt.float32, "invc",
                       bcast=(96, g.G))
        maskneg_sb = load(pp, maskneg_d, [96, g.NLP], dt.bfloat16, "maskn",
                          bcast=(96, g.NLP))

        gap_last_sb = load(pp, gap_last_d, [96, g.G // 16], dt.int16, "glast")
        maxcol_sb = load(pp, maxcol_d, [96, g.G // 16], dt.int16, "maxcol")
        Urows = load(pp, U1rows_d, [128, 4, H], dt.bfloat16, "Urows")
        tabkeep = pp.tile([128, g.NWIN, 96], dt.bfloat16, tag="tabkeep",
                          name="tabkeep")
        W_sb = {n: load(pp, W_d[n], list(W_d[n].shape), dt.bfloat16, n)
                for n in W_d}
        b_bc = {n: load(pp, b_d[n], [128, H], dt.float32, n, bcast=(128, H))
                for n in b_d}
        Wl1a_sb = load(pp, Wl1a_d, [H, H], dt.float32, "Wl1a")
        Wl1b_sb = load(pp, Wl1b_d, [H, H], dt.float32, "Wl1b")
        Wl2_sb = load(pp, Wl2_d, [H, H2], dt.float32, "Wl2")
        Wl3_sb = load(pp, Wl3_d, [H2, O], dt.float32, "Wl3")
        bl1_sb = load(pp, bl1_d, [H, 1], dt.float32, "bl1")
        bl2_sb = load(pp, bl2_d, [H2, 1], dt.float32, "bl2")
        bl3_sb = load(pp, bl3_d, [O, 1], dt.float32, "bl3")

        xbuf = [pp.tile([128, g.NLP], dt.bfloat16, tag=f"xbuf{i}", name=f"xbuf{i}")
                for i in range(2)]
        nc.sync.dma_start(out=xbuf[0][:g.F_IN, :], in_=xT_d.ap())
        # edge-gather tables load after the conv-critical inputs
        eidx_sb = load(pp, eidx_d, [128, Ttot * 8], dt.int16, "eidx")
        dst_sb = load(pp, dstslot_d, [128, Ttot], dt.float32, "dstslot")

        scano = pp.tile([96, g.NLP], dt.float32, tag="scano", name="scano")
        gmp_all = pp.tile([96, 3 * g.G], dt.float32, tag="gmp_all", name="gmp_all")
        mg_sb = [pp.tile([96, g.G], dt.float32, tag=f"mg{i}", name=f"mg{i}")
                 for i in range(3)]
        gapar_sb = [pp.tile([96, g.G], dt.float32, tag=f"gapar{i}", name=f"gapar{i}")
                    for i in range(3)]

        zinit = wk.tile([128, Tmax, 128], dt.bfloat16, tag="gath", name="zinit")
        nc.gpsimd.memset(zinit[:, :g.NWIN, :], 0.0)
        nc.sync.dma_start(
            out=tshardA.ap().rearrange("(w p) c -> p w c", p=128),
            in_=zinit[:, :WINA, :])
        nc.sync.dma_start(
            out=tshardB.ap().rearrange("(w p) c -> p w c", p=128),
            in_=zinit[:, WINA:g.NWIN, :])

        layer_W = [("W1a", "W1b", "b1_row"), ("W2a", "W2b", "b2_row"),
                   ("W3a", "W3b", "b3_row")]

        for l in range(dbg_layers):
            F = g.F_IN if l == 0 else H
            Wa = W_sb[layer_W[l][0]]
            Wb = W_sb[layer_W[l][1]]
            bias = b_bc[layer_W[l][2]]
            x_src, x_dst = xbuf[l % 2], xbuf[(l + 1) % 2]

            # ---- conv + gather-table build ----
            nc.vector.memset(scano[:, 0:1], 0.0)
            for ci, (cs, cw) in enumerate(g.conv_chunks):
                ohc = wk.tile([128, 4, CONV_CHUNK], dt.bfloat16, tag="ohc",
                              name="ohc")
                nc.sync.dma_start(out=ohc[:],
                                  in_=ohconv_d.ap()[:, 4 * ci:4 * ci + 4, :])
                for wo in range(0, cw, 128):
                    w = (cs + wo) // 128
                    # node-major conv: psn[n, f] = x_cat[:, n]^T W + U[:, batch_n]
                    psn = ps_tr.tile([128, 128], dt.float32, tag="ptr",
                                     name="ptr")
                    nc.tensor.matmul(out=psn[:, :96],
                                     lhsT=x_src[:F, cs + wo:cs + wo + 128],
                                     rhs=Wa[:], start=True, stop=False)
                    for b in range(4):
                        nc.tensor.matmul(out=psn[:, :96],
                                         lhsT=ohc[:, b, wo:wo + 128],
                                         rhs=Urows[:, b, :],
                                         start=False, stop=b == 3)
                    nc.scalar.activation(out=tabkeep[:, w, :], in_=psn[:, :96],
                                         func=Act.Copy,
                                         scale=dinv_sbT[:, w:w + 1])
                    if w < WINA:
                        tgt = tshardA.ap()[w * 128:(w + 1) * 128, :96]
                    else:
                        wb = w - WINA
                        tgt = tshardB.ap()[wb * 128:(wb + 1) * 128, :96]
                    nc.sync.dma_start(out=tgt, in_=tabkeep[:, w, :])
                if cs + cw >= COLA and cs < COLA:
                    # windows 0..WINA-1 written: AG half A overlaps conv tail
                    nc.gpsimd.collective_compute(
                        "AllGather", Alu.bypass, replica_groups=rg,
                        ins=[tshardA.ap()], outs=[tableA.ap()])

            if dbg_stage == "conv":
                break
            # ---- all-gather table half B ----
            nc.gpsimd.collective_compute(
                "AllGather", Alu.bypass, replica_groups=rg,
                ins=[tshardB.ap()], outs=[tableB.ap()])

            if dbg_stage == "ag":
                break
            # ---- edge aggregation ----
            # software-pipelined gather issue: A-calls of chunk c, then
            # B-calls of chunk c-1, so the AG-B flight hides behind a full
            # chunk of A-half gather work before the first B-call.
            def gcalls(gath, goff, tstart, cnt, hi):
                tbl_ap = tableB.ap() if hi else tableA.ap()
                done = 0
                while done < cnt:
                    nt = min(8, cnt - done)
                    nc.gpsimd.dma_gather(
                        gath[:, goff + done:goff + done + nt, :],
                        tbl_ap,
                        eidx_sb[:, 8 * (tstart + done):
                                8 * (tstart + done + nt)],
                        num_idxs=nt * 128, num_idxs_reg=nt * 128,
                        elem_size=128)
                    done += nt

            nchunks = len(chunk_plan)
            tiles, tbases = {}, {}
            tb_accum = 0
            for ci in range(nchunks + 1):
                if ci < nchunks:
                    (_, _, cTlo, cThi, _, _) = chunk_plan[ci]
                    gath_new = wk.tile([128, Tmax, 128], dt.bfloat16,
                                       tag="gath", name="gath")
                    tiles[ci], tbases[ci] = gath_new, tb_accum
                    if cTlo:
                        gcalls(gath_new, 0, tb_accum, cTlo, False)
                    tb_accum += cTlo + cThi
                if ci == 0:
                    continue
                (w0, nw, Tlo, Thi, rng_l, rng_h) = chunk_plan[ci - 1]
                gath, t_base = tiles.pop(ci - 1), tbases.pop(ci - 1)
                T = Tlo + Thi
                if Thi:
                    gcalls(gath, Tlo, t_base + Tlo, Thi, True)
                if dbg_agg == "gather":
                    continue
                # per window: masked one-hot matmuls over its tile ranges
                blocks = []   # (wi, gath offset of first tile, ntiles)
                for wi in range(nw):
                    if rng_l[wi] is not None:
                        blocks.append((wi, rng_l[wi][0],
                                       rng_l[wi][1] - rng_l[wi][0] + 1))
                    if rng_h[wi] is not None:
                        blocks.append((wi, Tlo + rng_h[wi][0],
                                       rng_h[wi][1] - rng_h[wi][0] + 1))
                pags = [ps_agg.tile([96, 128], dt.float32, tag="pag", name="pag")
                        for _ in range(nw)]
                remaining = [0] * nw
                for (wi, _, Tb) in blocks:
                    remaining[wi] += Tb
                for wi in range(nw):
                    # self-loop term: psum += table rows of this window
                    nc.tensor.matmul(out=pags[wi][:],
                                     lhsT=tabkeep[:, w0 + wi, :],
                                     rhs=id128b_sb[:],
                                     start=True, stop=remaining[wi] == 0)
                for (wi, toff, Tb) in blocks:
                    oh = wk.tile([128, Tblk, 128], dt.bfloat16, tag="oh",
                                 name="oh", bufs=4)
                    a = t_base + toff
                    nc.vector.tensor_tensor(
                        out=oh[:, 0:Tb, :],
                        in0=dst_sb[:, a:a + Tb, None]
                            .to_broadcast((128, Tb, 128)),
                        in1=iota_sb[:, None, wi * 128:(wi + 1) * 128]
                            .to_broadcast((128, Tb, 128)),
                        op=Alu.is_equal)
                    for t in range(Tb):
                        if dbg_agg == "onehot":
                            continue
                        remaining[wi] -= 1
                        nc.tensor.matmul(out=pags[wi][:],
                                         lhsT=gath[:, toff + t, :96],
                                         rhs=oh[:, t, :],
                                         start=False,
                                         stop=remaining[wi] == 0)

                for wi in range(nw):
                    if dbg_agg in ("onehot", "mm"):
                        break
                    w = w0 + wi
                    sb1 = ep.tile([96, 128], dt.float32, tag="ep1", name="ep1")
                    nc.vector.tensor_copy(out=sb1[:], in_=pags[wi][:])
                    pt2 = ps_tr.tile([128, 128], dt.float32, tag="ptr", name="ptr")
                    nc.tensor.transpose(out=pt2[:, :96], in_=sb1[:],
                                        identity=id96_sb[:])
                    sb2 = ep.tile([128, 96], dt.float32, tag="ep2", name="ep2")
                    nc.scalar.activation(out=sb2[:], in_=pt2[:, :96],
                                         func=Act.Copy,
                                         scale=dinv_sbT[:, w:w + 1])
                    sb3 = ep.tile([128, 96], dt.bfloat16, tag="ep3", name="ep3")
                    if l < 2:
                        tmp = ep.tile([128, 96], dt.float32, tag="ep3f", name="ep3f")
                        nc.vector.tensor_add(out=tmp[:], in0=sb2[:],
                                             in1=bias[:, :96])
                        nc.vector.tensor_scalar_max(out=sb3[:], in0=tmp[:],
                                                    scalar1=0.0)
                    else:
                        nc.vector.tensor_add(out=sb3[:], in0=sb2[:],
                                             in1=bias[:, :96])
                    pt3 = ps_tr.tile([128, 128], dt.bfloat16, tag="ptr", name="ptr")
                    nc.tensor.transpose(out=pt3[:96, :], in_=sb3[:],
                                        identity=id128b_sb[:])
                    nc.vector.tensor_copy(
                        out=x_dst[:96, w * 128:(w + 1) * 128],
                        in_=pt3[:96, :])
                # chained gap cumsum over this chunk's finished columns: the
                # pooling scan rides under the remaining edge gathers
                c_lo, c_hi = w0 * 128, min((w0 + nw) * 128, g.NL)
                if c_lo < g.NL and dbg_agg == "ep":
                    nc.vector.tensor_tensor_scan(
                        out=scano[:, c_lo + 1:c_hi + 1],
                        data0=x_dst[:96, c_lo:c_hi],
                        data1=x_dst[:96, c_lo:c_hi],
                        initial=(0.0 if w0 == 0
                                 else scano[:, c_lo:c_lo + 1]),
                        op0=Alu.add, op1=Alu.bypass)

            nc.gpsimd.memset(x_dst[:96, g.NL:g.NLP], PAD_VAL)

            if dbg_stage == "agg":
                break
            # ---- pooling ----
            cum_l = ep.tile([96, g.G + 1], dt.float32, tag="cuml", name="cuml",
                            bufs=1)
            nc.vector.memset(cum_l[:, 0:1], 0.0)
            nc.gpsimd.ap_gather(cum_l[:, 1:g.G + 1], scano[:], gap_last_sb[:],
                                channels=96, num_elems=g.NLP, d=1,
                                num_idxs=g.G)
            gaps = ep.tile([96, g.G], dt.float32, tag="gaps", name="gaps", bufs=1)
            nc.vector.tensor_tensor(out=gaps[:], in0=cum_l[:, 1:g.G + 1],
                                    in1=cum_l[:, 0:g.G],
                                    op=Alu.subtract)
            nc.sync.dma_start(out=gap_in[l].ap(), in_=gaps[:])
            nc.gpsimd.collective_compute(
                "AllReduce", Alu.add, replica_groups=rg,
                ins=[gap_in[l].ap()], outs=[gap_out[l].ap()])
            nc.sync.dma_start(out=gapar_sb[l][:], in_=gap_out[l].ap())

            nc.vector.tensor_tensor_scan(
                out=scano[:], data0=maskneg_sb[:], data1=x_dst[:96, :],
                initial=0.0, op0=Alu.add, op1=Alu.max)
            nc.gpsimd.ap_gather(gmp_all[:, l * g.G:(l + 1) * g.G],
                                scano[:], maxcol_sb[:],
                                channels=96, num_elems=g.NLP, d=1,
                                num_idxs=g.G)
            # per-layer gmp AllReduce: layers 0-1 overlap the next layer's
            # edge gathers; only layer 2's is exposed
            nc.sync.dma_start(out=gmp_in[l].ap(),
                              in_=gmp_all[:, l * g.G:(l + 1) * g.G])
            nc.gpsimd.collective_compute(
                "AllReduce", Alu.max, replica_groups=rg,
                ins=[gmp_in[l].ap()], outs=[gmp_out[l].ap()])

            nc.vector.tensor_mul(out=mg_sb[l][:], in0=gapar_sb[l][:],
                                 in1=invc_bc[:])

            if l < 2:
                # build Urows for next layer: U = Wb_{l+1}^T @ mg,
                # transposed into [128 graph, 4 block, 96] for the conv
                Wbn = W_sb[layer_W[l + 1][1]]
                mgb = ep.tile([96, g.G], dt.bfloat16, tag="mgb", name="mgb",
                              bufs=1)
                nc.scalar.copy(out=mgb[:], in_=mg_sb[l][:])
                psU = ps_conv.tile([96, g.G], dt.float32, tag="psc", name="psc")
                nc.tensor.matmul(out=psU[:], lhsT=Wbn[:], rhs=mgb[:],
                                 start=True, stop=True)
                Usb = ep.tile([96, g.G], dt.bfloat16, tag="Usb", name="Usb",
                              bufs=1)
                nc.vector.tensor_copy(out=Usb[:], in_=psU[:])
                for b in range(4):
                    ptU = ps_tr.tile([128, 128], dt.bfloat16, tag="ptr",
                                     name="ptr")
                    nc.tensor.transpose(out=ptU[:, :96],
                                        in_=Usb[:, b * 128:(b + 1) * 128],
                                        identity=id128b_sb[:96, :96])
                    nc.scalar.copy(out=Urows[:, b, :], in_=ptU[:, :96])

        debug_cut = dbg_stage != "full" or dbg_layers < 3
        if debug_cut:
            nc.gpsimd.dma_start(out=out_d.ap(), in_=xbuf[0][:O, :g.G])
        # ---- final readout MLP (f32) ----
        if not debug_cut:
            gmpar = pp.tile([96, 3 * g.G], dt.float32, tag="gmpar", name="gmpar")
            for i in range(3):
                nc.sync.dma_start(out=gmpar[:, i * g.G:(i + 1) * g.G],
                                  in_=gmp_out[i].ap())

            hTa = pp.tile([96, g.G], dt.float32, tag="hTa", name="hTa")
            hTb = pp.tile([96, g.G], dt.float32, tag="hTb", name="hTb")
            nc.vector.tensor_add(out=hTa[:], in0=gmpar[:, 0:g.G],
                                 in1=gmpar[:, g.G:2 * g.G])
            nc.vector.tensor_add(out=hTa[:], in0=hTa[:],
                                 in1=gmpar[:, 2 * g.G:3 * g.G])
            nc.vector.tensor_add(out=hTb[:], in0=mg_sb[0][:], in1=mg_sb[1][:])
            nc.vector.tensor_add(out=hTb[:], in0=hTb[:], in1=mg_sb[2][:])

            ps1 = ps_conv.tile([96, g.G], dt.float32, tag="psc", name="psc")
            nc.tensor.matmul(out=ps1[:], lhsT=Wl1a_sb[:], rhs=hTa[:],
                             start=True, stop=False)
            nc.tensor.matmul(out=ps1[:], lhsT=Wl1b_sb[:], rhs=hTb[:],
                             start=False, stop=True)
            o1 = pp.tile([96, g.G], dt.float32, tag="o1", name="o1")
            nc.scalar.activation(out=o1[:], in_=ps1[:], func=Act.Relu,
                                 bias=bl1_sb[:])
            ps2 = ps_conv.tile([96, g.G], dt.float32, tag="psc", name="psc")
            nc.tensor.matmul(out=ps2[:H2, :], lhsT=Wl2_sb[:], rhs=o1[:],
                             start=True, stop=True)
            o2 = pp.tile([H2, g.G], dt.float32, tag="o2", name="o2")
            nc.scalar.activation(out=o2[:], in_=ps2[:H2, :], func=Act.Relu,
                                 bias=bl2_sb[:])
            ps3 = ps_conv.tile([96, g.G], dt.float32, tag="psc", name="psc")
            nc.tensor.matmul(out=ps3[:O, :], lhsT=Wl3_sb[:], rhs=o2[:],
                             start=True, stop=True)
            o3 = pp.tile([O, g.G], dt.float32, tag="o3", name="o3")
            nc.scalar.activation(out=o3[:], in_=ps3[:O, :], func=Act.Identity,
                                 bias=bl3_sb[:])
            nc.sync.dma_start(out=out_d.ap(), in_=o3[:])

        stk.close()

    nc.compile()
    return nc


_CACHE = {}


def _get_program(geo, meta, n_cores):
    key = (repr(sorted(geo.__dict__.items(), key=str)),
           repr(meta["chunk_plan"]), n_cores)
    if key not in _CACHE:
        _CACHE[key] = build_program(geo, meta, n_cores)
    return _CACHE[key]


def kernel(**inputs):
    from concourse.bass_utils import run_bass_kernel_spmd

    geo = Geo(CFG)
    inputs = {k: np.asarray(v) for k, v in inputs.items()}
    per_core, meta = prep(geo, **inputs)
    nc = _get_program(geo, meta, geo.C)
    res = run_bass_kernel_spmd(nc, per_core, core_ids=list(range(geo.C)))
    out = np.asarray(res.results[0]["out"], f32)   # [OUT, G]
    return np.ascontiguousarray(out.T)             # [G, OUT] float32



# revision 52
# speedup vs baseline: 1.9261x; 1.4798x over previous
"""Trainium2 Bass kernel for nn_BenchGNN_29300266893894 (3-layer GCN with
global-feature concat + global mean/max pooling readout + MLP head).

Self-contained: host-side sharding/packing prep + SPMD Bass/Tile program on
8 NeuronCores via run_bass_kernel_spmd.

Sharding: nodes are split into 8 contiguous shards of 6250 (padded to 6272 =
49*128). Edges are partitioned by dst owner and sorted by dst window; weight
matrices are replicated. Per layer, each core computes h = x_cat @ W for its
nodes (the global-mean term enters as U = Wb^T @ mean matmul'd against
static one-hot graph-block matrices - no gather), builds a bf16 table of
h' = h*dinv rows (256 B each, kept in SBUF as tabkeep AND written to DRAM
in two halves whose AllGathers overlap the conv tail), then aggregates
edges with dma_gather + masked one-hot matmuls accumulating per-window PSUM
tiles; edge tiles are packed at 4-window-chunk granularity and self-loops
are added via one identity matmul per window from tabkeep. Graph pooling
uses DVE scans + one boundary ap_gather per pool (gap via shift-subtract)
with per-layer AllReduces overlapped into the next layer's gather stream.
"""
import sys
import numpy as np
import ml_dtypes

sys.path.insert(0, "/opt/trn_rl_repo")

bf16 = ml_dtypes.bfloat16
f32 = np.float32

# ---------------- problem geometry (hardcoded) ----------------
CFG = dict(N=50000, E=800000, G=512, F_IN=128, HID=96, OUT=10, C=8)

WIN = 128
WINA = 25            # windows per core in table half A
COLA = WINA * WIN    # 3200
CHUNK_WINDOWS = 4
CONV_CHUNK = 512
PAD_SLOT = 999.0
PAD_VAL = -60000.0


class Geo:
    def __init__(self, cfg):
        self.__dict__.update(cfg)
        assert self.N % self.C == 0
        self.NL = self.N // self.C
        self.NWIN = -(-self.NL // WIN)
        self.NLP = self.NWIN * WIN
        # pooling pad-column tricks need at least one pad column per core
        assert self.NLP > self.NL
        self.NP = self.NLP * self.C
        self.conv_chunks = [(o, min(CONV_CHUNK, self.NLP - o))
                            for o in range(0, self.NLP, CONV_CHUNK)]


def _wrap16(idx, channels):
    idx = np.asarray(idx, np.int16)
    assert len(idx) % 16 == 0
    idx16 = idx.reshape(-1, 16).T
    return np.ascontiguousarray(np.tile(idx16, (channels // 16, 1)))


def prep(geo, x, edge_index, batch, W1, b1, W2, b2, W3, b3,
         Wl1, bl1, Wl2, bl2, Wl3, bl3):
    g = geo
    x = np.asarray(x, f32)
    src = np.asarray(edge_index[0], np.int64)
    dst = np.asarray(edge_index[1], np.int64)
    batch = np.asarray(batch, np.int64)

    def gid_of(n):
        return (n // g.NL) * g.NLP + (n % g.NL)

    deg = np.bincount(dst, minlength=g.N).astype(f32) + 1.0
    dinv = (1.0 / np.sqrt(deg)).astype(f32)
    counts = np.bincount(batch, minlength=g.G).astype(f32)
    invc = (1.0 / np.maximum(counts, 1.0)).astype(f32)

    sums1 = np.zeros((g.G, g.F_IN), f32)
    np.add.at(sums1, batch, x)
    mean1 = (sums1 * invc[:, None]).astype(f32)          # [G, F_IN]
    U1rows = (mean1 @ np.asarray(W1[g.F_IN:], f32))      # [G, HID]

    src_g = gid_of(src)

    core_win = []
    for k in range(g.C):
        lo_n, hi_n = k * g.NL, (k + 1) * g.NL
        sel = (dst >= lo_n) & (dst < hi_n)
        es, ed = src_g[sel], (dst[sel] - lo_n)
        order = np.argsort(ed, kind="stable")
        es, ed = es[order], ed[order]
        wstart = np.searchsorted(ed, np.arange(0, g.NLP + 1, WIN))
        wins = []
        COLB = g.NLP - COLA
        for w in range(g.NWIN):
            a, b = wstart[w], wstart[w + 1]
            ws, wd = es[a:b], ed[a:b] - w * WIN
            c_, off = ws // g.NLP, ws % g.NLP
            a_m = off < COLA
            wins.append((c_[a_m] * COLA + off[a_m], wd[a_m],
                         c_[~a_m] * COLB + (off[~a_m] - COLA), wd[~a_m]))
        core_win.append(wins)

    # chunk plan: per chunk of CHUNK_WINDOWS windows, edges of all windows are
    # packed contiguously (chunk-local slot ids); tile counts are chunk-level
    # cross-core maxima, and each window gets a [t0, t1] tile range (union
    # over cores) whose tiles feed its psum via masked one-hot matmuls.
    def _ranges(lens_kw, nw):
        out = []
        for wi in range(nw):
            t0s, t1s = [], []
            for k in range(g.C):
                s = sum(lens_kw[k][:wi])
                L = lens_kw[k][wi]
                if L:
                    t0s.append(s // 128)
                    t1s.append((s + L - 1) // 128)
            out.append((min(t0s), max(t1s)) if t0s else None)
        return out

    chunk_plan = []   # (w0, nw, TC_lo, TC_hi, rng_lo, rng_hi)
    for w0 in range(0, g.NWIN, CHUNK_WINDOWS):
        nw = min(CHUNK_WINDOWS, g.NWIN - w0)
        lens_lo = [[len(core_win[k][w0 + wi][0]) for wi in range(nw)]
                   for k in range(g.C)]
        lens_hi = [[len(core_win[k][w0 + wi][2]) for wi in range(nw)]
                   for k in range(g.C)]
        TC_lo = -(-max(sum(r) for r in lens_lo) // 128)
        TC_hi = -(-max(sum(r) for r in lens_hi) // 128)
        chunk_plan.append((w0, nw, TC_lo, TC_hi,
                           _ranges(lens_lo, nw), _ranges(lens_hi, nw)))

    per_core = []
    for k in range(g.C):
        lo_n, hi_n = k * g.NL, (k + 1) * g.NL
        idx_list, slot_list = [], []
        for (w0, nw, TCl, TCh, _, _) in chunk_plan:
            for half, TC in ((0, TCl), (2, TCh)):
                s_arr = np.concatenate(
                    [core_win[k][w0 + wi][half] for wi in range(nw)]
                    + [np.zeros(0, np.int64)])
                d_arr = np.concatenate(
                    [core_win[k][w0 + wi][half + 1] + 128 * wi
                     for wi in range(nw)] + [np.zeros(0, np.int64)])
                pad = TC * 128 - len(s_arr)
                idx_list.append(np.concatenate(
                    [s_arr, np.zeros(pad, np.int64)]))
                slot_list.append(np.concatenate(
                    [d_arr, np.full(pad, PAD_SLOT)]))
        idx_all = np.concatenate(idx_list)
        slot_all = np.concatenate(slot_list).astype(f32)
        idx_sb = _wrap16(idx_all, 128)
        dst_sb = np.ascontiguousarray(
            slot_all.reshape(-1, 128).T.astype(f32))

        bl = batch[lo_n:hi_n]
        # one-hot graph-block matrices for the conv global-feature term:
        # ohconv[p, c, b, j] = 1{batch_local[cs_c + j] == 128*b + p}
        nch = len(g.conv_chunks)
        ohconv = np.zeros((128, nch, 4, CONV_CHUNK), bf16)
        for c, (cs, cw) in enumerate(g.conv_chunks):
            seg = bl[cs:min(cs + cw, g.NL)]
            for j, gid in enumerate(seg):
                ohconv[gid % 128, c, gid // 128, j] = 1
        bcol = np.searchsorted(bl, np.arange(g.G), side="right") - 1
        gap_last = _wrap16(np.where(bcol < 0, 0, bcol + 1), 96)
        present = np.zeros(g.G, bool)
        present[np.unique(bl)] = True
        maxcol = _wrap16(np.where(present, bcol, g.NLP - 1), 96)

        gstart = np.searchsorted(bl, np.arange(g.G), side="left")
        maskneg = np.zeros(g.NLP, f32)
        maskneg[np.unique(gstart[present])] = -1e30
        maskneg[g.NL] = -1e30
        maskneg_row = maskneg.reshape(1, g.NLP).astype(bf16)

        xT = np.zeros((g.F_IN, g.NLP), f32)
        xT[:, :g.NL] = x[lo_n:hi_n].T
        dinv_loc = np.zeros(g.NLP, f32)
        dinv_loc[:g.NL] = dinv[lo_n:hi_n]
        dinv_sb = np.ascontiguousarray(dinv_loc.reshape(g.NWIN, WIN).T)

        H, H2, O = g.HID, g.HID // 2, g.OUT
        inp = {
            "xT": xT.astype(bf16),
            "U1rows": np.ascontiguousarray(
                U1rows.reshape(4, 128, H).transpose(1, 0, 2)).astype(bf16),
            "ohconv": np.ascontiguousarray(
                ohconv.reshape(128, 4 * nch, CONV_CHUNK)),
            "gap_last": gap_last, "maxcol": maxcol,
            "maskneg_row": maskneg_row,
            "eidx": idx_sb,
            "dstslot": dst_sb,
            "dinv_sb": dinv_sb,
            "iota512": np.tile(np.arange(CHUNK_WINDOWS * WIN, dtype=f32),
                               (128, 1)),
            "id96": np.eye(96, dtype=f32),
            "id128b": np.eye(128, dtype=bf16),
            "invc_row": invc.reshape(1, g.G),
            "W1a": np.asarray(W1[:g.F_IN], bf16),
            "W1b": np.asarray(W1[g.F_IN:], bf16),
            "W2a": np.asarray(W2[:H], bf16), "W2b": np.asarray(W2[H:], bf16),
            "W3a": np.asarray(W3[:H], bf16), "W3b": np.asarray(W3[H:], bf16),
            "b1_row": np.asarray(b1, f32).reshape(1, H),
            "b2_row": np.asarray(b2, f32).reshape(1, H),
            "b3_row": np.asarray(b3, f32).reshape(1, H),
            "Wl1a": np.asarray(Wl1[:H], f32), "Wl1b": np.asarray(Wl1[H:], f32),
            "Wl2": np.asarray(Wl2, f32), "Wl3": np.asarray(Wl3, f32),
            "bl1": np.asarray(bl1, f32).reshape(H, 1),
            "bl2": np.asarray(bl2, f32).reshape(H2, 1),
            "bl3": np.asarray(bl3, f32).reshape(O, 1),
        }
        per_core.append(inp)

    meta = {"chunk_plan": chunk_plan,
            "Ttot": sum(c[2] + c[3] for c in chunk_plan)}
    return per_core, meta


# ---------------- device program ----------------


def build_program(geo, meta, n_cores):
    import os
    dbg_layers = int(os.environ.get("KN_LAYERS", "3"))
    dbg_stage = os.environ.get("KN_STAGE", "full")
    dbg_agg = os.environ.get("KN_AGG", "ep")
    import concourse.bacc as bacc
    import concourse.bass as bass
    import concourse.mybir as mybir
    import concourse.tile as tile

    g = geo
    H, H2, O = g.HID, g.HID // 2, g.OUT
    dt = mybir.dt
    Alu = mybir.AluOpType
    Act = mybir.ActivationFunctionType
    chunk_plan = meta["chunk_plan"]
    Ttot = meta["Ttot"]
    Tmax = max(max(c[2] + c[3] for c in chunk_plan), g.NWIN)
    Tblk = max(r[1] - r[0] + 1
               for (_, _, _, _, rl, rh) in chunk_plan
               for r in rl + rh if r is not None)

    nc = bacc.Bacc("TRN2", target_bir_lowering=False, debug=False,
                   num_devices=n_cores, num_swdge_queues=2)
    rg = [list(range(n_cores))]

    def din(name, shape, dtype):
        return nc.dram_tensor(name, list(shape), dtype, kind="ExternalInput")

    xT_d = din("xT", [g.F_IN, g.NLP], dt.bfloat16)
    U1rows_d = din("U1rows", [128, 4, H], dt.bfloat16)
    ohconv_d = din("ohconv", [128, 4 * len(g.conv_chunks), CONV_CHUNK],
                   dt.bfloat16)
    gap_last_d = din("gap_last", [96, g.G // 16], dt.int16)
    maxcol_d = din("maxcol", [96, g.G // 16], dt.int16)
    maskneg_d = din("maskneg_row", [1, g.NLP], dt.bfloat16)
    eidx_d = din("eidx", [128, Ttot * 8], dt.int16)
    dstslot_d = din("dstslot", [128, Ttot], dt.float32)
    dinv_d = din("dinv_sb", [128, g.NWIN], dt.float32)
    iota_d = din("iota512", [128, CHUNK_WINDOWS * WIN], dt.float32)
    id96_d = din("id96", [96, 96], dt.float32)
    id128b_d = din("id128b", [128, 128], dt.bfloat16)
    invc_d = din("invc_row", [1, g.G], dt.float32)
    W_d = {n: din(n, [g.F_IN if n[1] == "1" else H, H], dt.bfloat16)
           for n in ("W1a", "W1b", "W2a", "W2b", "W3a", "W3b")}
    b_d = {n: din(n, [1, H], dt.float32)
           for n in ("b1_row", "b2_row", "b3_row")}
    Wl1a_d = din("Wl1a", [H, H], dt.float32)
    Wl1b_d = din("Wl1b", [H, H], dt.float32)
    Wl2_d = din("Wl2", [H, H2], dt.float32)
    Wl3_d = din("Wl3", [H2, O], dt.float32)
    bl1_d = din("bl1", [H, 1], dt.float32)
    bl2_d = din("bl2", [H2, 1], dt.float32)
    bl3_d = din("bl3", [O, 1], dt.float32)

    out_d = nc.dram_tensor("out", [O, g.G], dt.float32, kind="ExternalOutput")

    # internal DRAM (table split in two halves so AG-A overlaps conv tail)
    COLB = g.NLP - COLA
    tshardA = nc.dram_tensor("tshardA", [COLA, 128], dt.bfloat16,
                             kind="Internal")
    tshardB = nc.dram_tensor("tshardB", [COLB, 128], dt.bfloat16,
                             kind="Internal")
    tableA = nc.dram_tensor("tableA", [g.C * COLA, 128], dt.bfloat16,
                            kind="Internal", addr_space="Shared")
    tableB = nc.dram_tensor("tableB", [g.C * COLB, 128], dt.bfloat16,
                            kind="Internal", addr_space="Shared")
    gap_in = [nc.dram_tensor(f"gap_in{i}", [96, g.G], dt.float32,
                             kind="Internal") for i in range(3)]
    gap_out = [nc.dram_tensor(f"gap_out{i}", [96, g.G], dt.float32,
                              kind="Internal", addr_space="Shared")
               for i in range(3)]
    gmp_in = [nc.dram_tensor(f"gmp_in{i}", [96, g.G], dt.float32,
                             kind="Internal") for i in range(3)]
    gmp_out = [nc.dram_tensor(f"gmp_out{i}", [96, g.G], dt.float32,
                              kind="Internal", addr_space="Shared")
               for i in range(3)]

    with tile.TileContext(nc) as tc:
        import contextlib
        stk = contextlib.ExitStack()
        pp = stk.enter_context(tc.tile_pool(name="persist", bufs=1))
        wk = stk.enter_context(tc.tile_pool(name="work", bufs=2))
        ep = stk.enter_context(tc.tile_pool(name="epil", bufs=2))
        ps_conv = stk.enter_context(
            tc.tile_pool(name="ps_conv", bufs=2, space="PSUM"))
        ps_tr = stk.enter_context(
            tc.tile_pool(name="ps_tr", bufs=2, space="PSUM"))
        ps_agg = stk.enter_context(
            tc.tile_pool(name="ps_agg", bufs=4, space="PSUM"))

        def load(pool, dram, shape, dtype, tag, bcast=None):
            t = pool.tile(shape, dtype, tag=tag, name=tag)
            src = dram.ap() if bcast is None else dram.ap().to_broadcast(bcast)
            nc.sync.dma_start(out=t[:], in_=src)
            return t

        # constants
        iota_sb = load(pp, iota_d, [128, CHUNK_WINDOWS * WIN], dt.float32,
                       "iota")
        id96_sb = load(pp, id96_d, [96, 96], dt.float32, "id96")
        id128b_sb = load(pp, id128b_d, [128, 128], dt.bfloat16, "id128b")
        dinv_sbT = load(pp, dinv_d, [128, g.NWIN], dt.float32, "dinv")
        invc_bc = load(pp, invc_d, [96, g.G], dt.float32, "invc",
                       bcast=(96, g.G))
        maskneg_sb = load(pp, maskneg_d, [96, g.NLP], dt.bfloat16, "maskn",
                          bcast=(96, g.NLP))

        gap_last_sb = load(pp, gap_last_d, [96, g.G // 16], dt.int16, "glast")
        maxcol_sb = load(pp, maxcol_d, [96, g.G // 16], dt.int16, "maxcol")
        Urows = load(pp, U1rows_d, [128, 4, H], dt.bfloat16, "Urows")
        tabkeep = pp.tile([128, g.NWIN, 96], dt.bfloat16, tag="tabkeep",
                          name="tabkeep")
        W_sb = {n: load(pp, W_d[n], list(W_d[n].shape), dt.bfloat16, n)
                for n in W_d}
        b_bc = {n: load(pp, b_d[n], [128, H], dt.float32, n, bcast=(128, H))
                for n in b_d}
        Wl1a_sb = load(pp, Wl1a_d, [H, H], dt.float32, "Wl1a")
        Wl1b_sb = load(pp, Wl1b_d, [H, H], dt.float32, "Wl1b")
        Wl2_sb = load(pp, Wl2_d, [H, H2], dt.float32, "Wl2")
        Wl3_sb = load(pp, Wl3_d, [H2, O], dt.float32, "Wl3")
        bl1_sb = load(pp, bl1_d, [H, 1], dt.float32, "bl1")
        bl2_sb = load(pp, bl2_d, [H2, 1], dt.float32, "bl2")
        bl3_sb = load(pp, bl3_d, [O, 1], dt.float32, "bl3")

        xbuf = [pp.tile([128, g.NLP], dt.bfloat16, tag=f"xbuf{i}", name=f"xbuf{i}")
                for i in range(2)]
        nc.sync.dma_start(out=xbuf[0][:g.F_IN, :], in_=xT_d.ap())
        # edge-gather tables load after the conv-critical inputs
        eidx_sb = load(pp, eidx_d, [128, Ttot * 8], dt.int16, "eidx")
        dst_sb = load(pp, dstslot_d, [128, Ttot], dt.float32, "dstslot")

        scano = pp.tile([96, g.NLP], dt.float32, tag="scano", name="scano")
        gmp_all = pp.tile([96, 3 * g.G], dt.float32, tag="gmp_all", name="gmp_all")
        mg_sb = [pp.tile([96, g.G], dt.float32, tag=f"mg{i}", name=f"mg{i}")
                 for i in range(3)]
        gapar_sb = [pp.tile([96, g.G], dt.float32, tag=f"gapar{i}", name=f"gapar{i}")
                    for i in range(3)]

        zinit = wk.tile([128, Tmax, 128], dt.bfloat16, tag="gath", name="zinit")
        nc.gpsimd.memset(zinit[:, :g.NWIN, :], 0.0)
        nc.sync.dma_start(
            out=tshardA.ap().rearrange("(w p) c -> p w c", p=128),
            in_=zinit[:, :WINA, :])
        nc.sync.dma_start(
            out=tshardB.ap().rearrange("(w p) c -> p w c", p=128),
            in_=zinit[:, WINA:g.NWIN, :])

        layer_W = [("W1a", "W1b", "b1_row"), ("W2a", "W2b", "b2_row"),
                   ("W3a", "W3b", "b3_row")]

        for l in range(dbg_layers):
            F = g.F_IN if l == 0 else H
            Wa = W_sb[layer_W[l][0]]
            Wb = W_sb[layer_W[l][1]]
            bias = b_bc[layer_W[l][2]]
            x_src, x_dst = xbuf[l % 2], xbuf[(l + 1) % 2]

            # ---- conv + gather-table build ----
            nc.vector.memset(scano[:, 0:1], 0.0)
            for ci, (cs, cw) in enumerate(g.conv_chunks):
                ohc = wk.tile([128, 4, CONV_CHUNK], dt.bfloat16, tag="ohc",
                              name="ohc")
                nc.sync.dma_start(out=ohc[:],
                                  in_=ohconv_d.ap()[:, 4 * ci:4 * ci + 4, :])
                for wo in range(0, cw, 128):
                    w = (cs + wo) // 128
                    # node-major conv: psn[n, f] = x_cat[:, n]^T W + U[:, batch_n]
                    psn = ps_tr.tile([128, 128], dt.float32, tag="ptr",
                                     name="ptr")
                    nc.tensor.matmul(out=psn[:, :96],
                                     lhsT=x_src[:F, cs + wo:cs + wo + 128],
                                     rhs=Wa[:], start=True, stop=False)
                    for b in range(4):
                        nc.tensor.matmul(out=psn[:, :96],
                                         lhsT=ohc[:, b, wo:wo + 128],
                                         rhs=Urows[:, b, :],
                                         start=False, stop=b == 3)
                    nc.scalar.activation(out=tabkeep[:, w, :], in_=psn[:, :96],
                                         func=Act.Copy,
                                         scale=dinv_sbT[:, w:w + 1])
                    if w < WINA:
                        tgt = tshardA.ap()[w * 128:(w + 1) * 128, :96]
                    else:
                        wb = w - WINA
                        tgt = tshardB.ap()[wb * 128:(wb + 1) * 128, :96]
                    nc.sync.dma_start(out=tgt, in_=tabkeep[:, w, :])
                if cs + cw >= COLA and cs < COLA:
                    # windows 0..WINA-1 written: AG half A overlaps conv tail
                    nc.gpsimd.collective_compute(
                        "AllGather", Alu.bypass, replica_groups=rg,
                        ins=[tshardA.ap()], outs=[tableA.ap()])

            if dbg_stage == "conv":
                break
            # ---- all-gather table half B ----
            nc.gpsimd.collective_compute(
                "AllGather", Alu.bypass, replica_groups=rg,
                ins=[tshardB.ap()], outs=[tableB.ap()])

            if dbg_stage == "ag":
                break
            # ---- edge aggregation ----
            # software-pipelined gather issue: A-calls of chunk c, then
            # B-calls of chunk c-1, so the AG-B flight hides behind a full
            # chunk of A-half gather work before the first B-call.
            def gcalls(gath, goff, tstart, cnt, hi):
                tbl_ap = tableB.ap() if hi else tableA.ap()
                done = 0
                while done < cnt:
                    nt = min(8, cnt - done)
                    nc.gpsimd.dma_gather(
                        gath[:, goff + done:goff + done + nt, :],
                        tbl_ap,
                        eidx_sb[:, 8 * (tstart + done):
                                8 * (tstart + done + nt)],
                        num_idxs=nt * 128, num_idxs_reg=nt * 128,
                        elem_size=128,
                        queue_num=self_qn[0])
                    self_qn[0] ^= 1
                    done += nt

            self_qn = [0]

            nchunks = len(chunk_plan)
            tiles, tbases = {}, {}
            tb_accum = 0
            for ci in range(nchunks + 1):
                if ci < nchunks:
                    (_, _, cTlo, cThi, _, _) = chunk_plan[ci]
                    gath_new = wk.tile([128, Tmax, 128], dt.bfloat16,
                                       tag="gath", name="gath")
                    tiles[ci], tbases[ci] = gath_new, tb_accum
                    if cTlo:
                        gcalls(gath_new, 0, tb_accum, cTlo, False)
                    tb_accum += cTlo + cThi
                if ci == 0:
                    continue
                (w0, nw, Tlo, Thi, rng_l, rng_h) = chunk_plan[ci - 1]
                gath, t_base = tiles.pop(ci - 1), tbases.pop(ci - 1)
                T = Tlo + Thi
                if Thi:
                    gcalls(gath, Tlo, t_base + Tlo, Thi, True)
                if dbg_agg == "gather":
                    continue
                # per window: masked one-hot matmuls over its tile ranges
                blocks = []   # (wi, gath offset of first tile, ntiles)
                for wi in range(nw):
                    if rng_l[wi] is not None:
                        blocks.append((wi, rng_l[wi][0],
                                       rng_l[wi][1] - rng_l[wi][0] + 1))
                    if rng_h[wi] is not None:
                        blocks.append((wi, Tlo + rng_h[wi][0],
                                       rng_h[wi][1] - rng_h[wi][0] + 1))
                pags = [ps_agg.tile([96, 128], dt.float32, tag="pag", name="pag")
                        for _ in range(nw)]
                remaining = [0] * nw
                for (wi, _, Tb) in blocks:
                    remaining[wi] += Tb
                for wi in range(nw):
                    # self-loop term: psum += table rows of this window
                    nc.tensor.matmul(out=pags[wi][:],
                                     lhsT=tabkeep[:, w0 + wi, :],
                                     rhs=id128b_sb[:],
                                     start=True, stop=remaining[wi] == 0)
                for (wi, toff, Tb) in blocks:
                    oh = wk.tile([128, Tblk, 128], dt.bfloat16, tag="oh",
                                 name="oh", bufs=4)
                    a = t_base + toff
                    nc.vector.tensor_tensor(
                        out=oh[:, 0:Tb, :],
                        in0=dst_sb[:, a:a + Tb, None]
                            .to_broadcast((128, Tb, 128)),
                        in1=iota_sb[:, None, wi * 128:(wi + 1) * 128]
                            .to_broadcast((128, Tb, 128)),
                        op=Alu.is_equal)
                    for t in range(Tb):
                        if dbg_agg == "onehot":
                            continue
                        remaining[wi] -= 1
                        nc.tensor.matmul(out=pags[wi][:],
                                         lhsT=gath[:, toff + t, :96],
                                         rhs=oh[:, t, :],
                                         start=False,
                                         stop=remaining[wi] == 0)

                for wi in range(nw):
                    if dbg_agg in ("onehot", "mm"):
                        break
                    w = w0 + wi
                    sb1 = ep.tile([96, 128], dt.float32, tag="ep1", name="ep1")
                    nc.vector.tensor_copy(out=sb1[:], in_=pags[wi][:])
                    pt2 = ps_tr.tile([128, 128], dt.float32, tag="ptr", name="ptr")
                    nc.tensor.transpose(out=pt2[:, :96], in_=sb1[:],
                                        identity=id96_sb[:])
                    sb2 = ep.tile([128, 96], dt.float32, tag="ep2", name="ep2")
                    nc.scalar.activation(out=sb2[:], in_=pt2[:, :96],
                                         func=Act.Copy,
                                         scale=dinv_sbT[:, w:w + 1])
                    sb3 = ep.tile([128, 96], dt.bfloat16, tag="ep3", name="ep3")
                    if l < 2:
                        tmp = ep.tile([128, 96], dt.float32, tag="ep3f", name="ep3f")
                        nc.vector.tensor_add(out=tmp[:], in0=sb2[:],
                                             in1=bias[:, :96])
                        nc.vector.tensor_scalar_max(out=sb3[:], in0=tmp[:],
                                                    scalar1=0.0)
                    else:
                        nc.vector.tensor_add(out=sb3[:], in0=sb2[:],
                                             in1=bias[:, :96])
                    pt3 = ps_tr.tile([128, 128], dt.bfloat16, tag="ptr", name="ptr")
                    nc.tensor.transpose(out=pt3[:96, :], in_=sb3[:],
                                        identity=id128b_sb[:])
                    nc.vector.tensor_copy(
                        out=x_dst[:96, w * 128:(w + 1) * 128],
                        in_=pt3[:96, :])
                # chained gap cumsum over this chunk's finished columns: the
                # pooling scan rides under the remaining edge gathers
                c_lo, c_hi = w0 * 128, min((w0 + nw) * 128, g.NL)
                if c_lo < g.NL and dbg_agg == "ep":
                    nc.vector.tensor_tensor_scan(
                        out=scano[:, c_lo + 1:c_hi + 1],
                        data0=x_dst[:96, c_lo:c_hi],
                        data1=x_dst[:96, c_lo:c_hi],
                        initial=(0.0 if w0 == 0
                                 else scano[:, c_lo:c_lo + 1]),
                        op0=Alu.add, op1=Alu.bypass)

            nc.gpsimd.memset(x_dst[:96, g.NL:g.NLP], PAD_VAL)

            if dbg_stage == "agg":
                break
            # ---- pooling ----
            cum_l = ep.tile([96, g.G + 1], dt.float32, tag="cuml", name="cuml",
                            bufs=1)
            nc.vector.memset(cum_l[:, 0:1], 0.0)
            nc.gpsimd.ap_gather(cum_l[:, 1:g.G + 1], scano[:], gap_last_sb[:],
                                channels=96, num_elems=g.NLP, d=1,
                                num_idxs=g.G)
            gaps = ep.tile([96, g.G], dt.float32, tag="gaps", name="gaps", bufs=1)
            nc.vector.tensor_tensor(out=gaps[:], in0=cum_l[:, 1:g.G + 1],
                                    in1=cum_l[:, 0:g.G],
                                    op=Alu.subtract)
            nc.sync.dma_start(out=gap_in[l].ap(), in_=gaps[:])
            nc.gpsimd.collective_compute(
                "AllReduce", Alu.add, replica_groups=rg,
                ins=[gap_in[l].ap()], outs=[gap_out[l].ap()])
            nc.sync.dma_start(out=gapar_sb[l][:], in_=gap_out[l].ap())

            nc.vector.tensor_tensor_scan(
                out=scano[:], data0=maskneg_sb[:], data1=x_dst[:96, :],
                initial=0.0, op0=Alu.add, op1=Alu.max)
            nc.gpsimd.ap_gather(gmp_all[:, l * g.G:(l + 1) * g.G],
                                scano[:], maxcol_sb[:],
                                channels=96, num_elems=g.NLP, d=1,
                                num_idxs=g.G)
            # per-layer gmp AllReduce: layers 0-1 overlap the next layer's
            # edge gathers; only layer 2's is exposed
            nc.sync.dma_start(out=gmp_in[l].ap(),
                              in_=gmp_all[:, l * g.G:(l + 1) * g.G])
            nc.gpsimd.collective_compute(
                "AllReduce", Alu.max, replica_groups=rg,
                ins=[gmp_in[l].ap()], outs=[gmp_out[l].ap()])

            nc.vector.tensor_mul(out=mg_sb[l][:], in0=gapar_sb[l][:],
                                 in1=invc_bc[:])

            if l < 2:
                # build Urows for next layer: U = Wb_{l+1}^T @ mg,
                # transposed into [128 graph, 4 block, 96] for the conv
                Wbn = W_sb[layer_W[l + 1][1]]
                mgb = ep.tile([96, g.G], dt.bfloat16, tag="mgb", name="mgb",
                              bufs=1)
                nc.scalar.copy(out=mgb[:], in_=mg_sb[l][:])
                psU = ps_conv.tile([96, g.G], dt.float32, tag="psc", name="psc")
                nc.tensor.matmul(out=psU[:], lhsT=Wbn[:], rhs=mgb[:],
                                 start=True, stop=True)
                Usb = ep.tile([96, g.G], dt.bfloat16, tag="Usb", name="Usb",
                              bufs=1)
                nc.vector.tensor_copy(out=Usb[:], in_=psU[:])
                for b in range(4):
                    ptU = ps_tr.tile([128, 128], dt.bfloat16, tag="ptr",
                                     name="ptr")
                    nc.tensor.transpose(out=ptU[:, :96],
                                        in_=Usb[:, b * 128:(b + 1) * 128],
                                        identity=id128b_sb[:96, :96])
                    nc.scalar.copy(out=Urows[:, b, :], in_=ptU[:, :96])

        debug_cut = dbg_stage != "full" or dbg_layers < 3
        if debug_cut:
            nc.gpsimd.dma_start(out=out_d.ap(), in_=xbuf[0][:O, :g.G])
        # ---- final readout MLP (f32) ----
        if not debug_cut:
            gmpar = pp.tile([96, 3 * g.G], dt.float32, tag="gmpar", name="gmpar")
            for i in range(3):
                nc.sync.dma_start(out=gmpar[:, i * g.G:(i + 1) * g.G],
                                  in_=gmp_out[i].ap())

            hTa = pp.tile([96, g.G], dt.float32, tag="hTa", name="hTa")
            hTb = pp.tile([96, g.G], dt.float32, tag="hTb", name="hTb")
            nc.vector.tensor_add(out=hTa[:], in0=gmpar[:, 0:g.G],
                                 in1=gmpar[:, g.G:2 * g.G])
            nc.vector.tensor_add(out=hTa[:], in0=hTa[:],
                                 in1=gmpar[:, 2 * g.G:3 * g.G])
            nc.vector.tensor_add(out=hTb[:], in0=mg_sb[0][:], in1=mg_sb[1][:])
            nc.vector.tensor_add(out=hTb[:], in0=hTb[:], in1=mg_sb[2][:])

            ps1 = ps_conv.tile([96, g.G], dt.float32, tag="psc", name="psc")
            nc.tensor.matmul(out=ps1[:], lhsT=Wl1a_sb[:], rhs=hTa[:],
                             start=True, stop=False)
            nc.tensor.matmul(out=ps1[:], lhsT=Wl1b_sb[:], rhs=hTb[:],
                             start=False, stop=True)
            o1 = pp.tile([96, g.G], dt.float32, tag="o1", name="o1")
            nc.scalar.activation(out=o1[:], in_=ps1[:], func=Act.Relu,
                                 bias=bl1_sb[:])
            ps2 = ps_conv.tile([96, g.G], dt.float32, tag="psc", name="psc")
            nc.tensor.matmul(out=ps2[:H2, :], lhsT=Wl2_sb[:], rhs=o1[:],
                             start=True, stop=True)
            o2 = pp.tile([H2, g.G], dt.float32, tag="o2", name="o2")
            nc.scalar.activation(out=o2[:], in_=ps2[:H2, :], func=Act.Relu,
                                 bias=bl2_sb[:])
            ps3 = ps_conv.tile([96, g.G], dt.float32, tag="psc", name="psc")
            nc.tensor.matmul(out=ps3[:O, :], lhsT=Wl3_sb[:], rhs=o2[:],
                             start=True, stop=True)
            o3 = pp.tile([O, g.G], dt.float32, tag="o3", name="o3")
            nc.scalar.activation(out=o3[:], in_=ps3[:O, :], func=Act.Identity,
                                 bias=bl3_sb[:])
            nc.sync.dma_start(out=out_d.ap(), in_=o3[:])

        stk.close()

    nc.compile()
    return nc


_CACHE = {}


def _get_program(geo, meta, n_cores):
    key = (repr(sorted(geo.__dict__.items(), key=str)),
           repr(meta["chunk_plan"]), n_cores)
    if key not in _CACHE:
        _CACHE[key] = build_program(geo, meta, n_cores)
    return _CACHE[key]


def kernel(**inputs):
    from concourse.bass_utils import run_bass_kernel_spmd

    geo = Geo(CFG)
    inputs = {k: np.asarray(v) for k, v in inputs.items()}
    per_core, meta = prep(geo, **inputs)
    nc = _get_program(geo, meta, geo.C)
    res = run_bass_kernel_spmd(nc, per_core, core_ids=list(range(geo.C)))
    out = np.asarray(res.results[0]["out"], f32)   # [OUT, G]
    return np.ascontiguousarray(out.T)             # [G, OUT] float32

